# revision 74
# baseline (speedup 1.0000x reference)
"""BKT model kernel v2 for Trainium2 (8 NeuronCores, Bass/Tile).

Exact 2-state HMM reformulation of the reference's 2^n-trajectory fastBKT
(see kernel v1 docstring).  v2 restructures for the DVE cost model:

- fp16 for the whole matrix chain (obs probs, level matrices, tree products,
  alphas, predictions).  The chain is contractive and sum-normalized, so
  fp16's 2^-11 rounding keeps the final error ~2e-3 << the 2e-2 gate;
  subnormal flushes only hit entries whose contribution is negligible.
- planar 2x2-entry planes (one buffer region per matrix entry) so
  tensor_tensor ops read/write packed last dims -> DVE 2x mode; per-partition
  transition constants ride tensor_scalar (2x/4x) and Act-engine scale APs.
- within-block (8-step) products use the A^T = gamma*I + 1 v^T structure at
  level 1, a "parity-split" pair layout for levels 2-3, and a 3-stage vector
  down-sweep for the per-step alphas.
- the 64-block scan is radix-8: in-group Hillis-Steele matrix prefixes,
  a tiny 8-group matrix scan, then one batched mat-vec to get per-block
  start alphas.
- Act engine absorbs sigmoids/copies/lns (including the bit-reversal
  unpermute via 4-free-dim APs); Pool absorbs reductions off the DVE path.
- log-predictions, cumsum and the ability-collapse stay f32.

Sharding: data-parallel over students (B=512 -> 64 per core); 5 ability
levels x 64 students = 320 rows padded to 3 x 128-partition tiles.
"""

import os
import numpy as np
from contextlib import ExitStack

import concourse.bass as bass
import concourse.bacc as bacc
import concourse.mybir as mybir
from concourse import tile
from concourse.bass_utils import run_bass_kernel_spmd

F32 = mybir.dt.float32
FP16 = mybir.dt.float16
Alu = mybir.AluOpType
Act = mybir.ActivationFunctionType
AX = mybir.AxisListType

N_CORES = 8
B_FULL = 512
T = 512
A_LEV = 5
BL = B_FULL // N_CORES          # students per core = 64
ROWS = A_LEV * BL               # valid rows per core = 320
RT = 3                          # row tiles of 128 (384 rows incl. pad)
NBT = RT * 64                   # blocks spanning tiles = 192
W = RT * T                      # full-plane free width = 1536
H = W // 2                      # half width = 768
ABILITY = np.array([-2.0, -1.0, 0.0, 1.0, 2.0], dtype=np.float32)

_last_results = None
_cached_nc = None


def _ap(base, off, dims):
    """Custom AP on the same tensor as `base`, keeping its partition dim."""
    return bass.AP(base.tensor, base.offset + off, [list(base.ap[0])] + dims)


def _ap_p(base, poff, pcount, off, dims):
    p = list(base.ap[0])
    pstride = p[0]
    return bass.AP(
        base.tensor, base.offset + poff * pstride + off, [[pstride, pcount]] + dims
    )


def _emit(ctx, tc, nc, U0, U1, PG, DM, Y, K, SH, O):
    v = nc.vector
    sc = nc.scalar
    gp = nc.gpsimd
    sy = nc.sync

    keep = ctx.enter_context(tc.tile_pool(name="keep", bufs=1))

    # ---------------- input DMAs ----------------
    # U0/U1 are the normalized per-step observation likelihood diagonals
    # (host-side sigmoids, slot-ordered); PG = P(y=1|unlearned) and
    # DM = P(y=1|learned) - PG feed the predictions.  K first (tiny, the
    # M planes need its scalars), then U0/U1 per row-tile on the HWDGE
    # queue; PG/DM/Y trail on Pool's SWDGE (needed only by the preds).
    es_in = ExitStack()
    io = es_in.enter_context(tc.tile_pool(name="io", bufs=1))
    u0 = io.tile([128, W], FP16, tag="U0")
    u1 = io.tile([128, W], FP16, tag="U1")
    pgs = keep.tile([128, W], FP16, tag="PG")
    dm = keep.tile([128, W], FP16, tag="DM")
    Yt = keep.tile([128, W], FP16, tag="Y")
    Kt = keep.tile([128, RT * 16], F32, tag="K")
    gp.dma_start(_ap(Kt[:], 0, [[16, RT], [1, 16]]),
                 bass.AP(K[:].tensor, 0, [[16, 128], [128 * 16, RT], [1, 16]]))
    for r in range(RT):
        for dram, sb in ((U0, u0), (U1, u1)):
            sy.dma_start(_ap(sb[:], r * T, [[1, T]]),
                         bass.AP(dram[:].tensor, r * 128 * T,
                                 [[T, 128], [1, T]]))
    for r in range(RT):
        for dram, sb in ((PG, pgs), (DM, dm)):
            gp.dma_start(_ap(sb[:], r * T, [[1, T]]),
                         bass.AP(dram[:].tensor, r * 128 * T,
                                 [[T, 128], [1, T]]))
    gp.dma_start(_ap(Yt[:], 0, [[T, RT], [1, T]]),
                 bass.AP(Y[:].tensor, 0, [[T, 128], [128 * T, RT], [1, T]]))
    # partition-half swap matrix SW[i, j] = 1 iff |i-j| == 64: PE matmuls
    # with it (or its left half) replace SBUF->SBUF partition-shift DMAs
    SHt = keep.tile([128, 128], F32, tag="SH")
    sy.dma_start(SHt[:], bass.AP(SH[:].tensor, 0, [[128, 128], [1, 128]]))

    def KC(col):
        """Per-partition scalar AP for K column `col` of row-tile r -- but all
        tiles share the op; K scalars differ per tile, so ops over multi-tile
        widths must pass per-tile slices.  Helper returns slice for tile r."""
        return Kt[:, col:col + 1]

    # K layout (16 cols per tile r at r*16):
    # 0..3 : A^T entries AT00, AT01, AT10, AT11
    # 4..7 : alpha_cp = AT_c0*AT_0p   (order 00,01,10,11)
    # 8..11: beta_cp  = AT_c1*AT_1p
    # 12,13: alpha1 init (s(-lI0), s(lI0))

    # ---------------- split u-halves ----------------
    # U0/U1 arrive from the host with the T axis permuted within each
    # 8-block: storage slot s holds natural step j = bitrev3(s), i.e. slot
    # order j = (0,4,2,6,1,5,3,7).  Slots 0..3 are exactly the even-j
    # "parity-split" order j_even(m) = 4*(m&1)+2*(m>>1) the M planes want,
    # slots 4..7 the odds.  M-plane reads are packed (stride-1 runs of 4)
    # -> DVE 4x, and each row-tile r starts as soon as its U DMAs land.
    Me = keep.tile([128, 4 * H], FP16, tag="Me")
    Mo = keep.tile([128, 4 * H], FP16, tag="Mo")
    ME = [Me[:, i * H:(i + 1) * H] for i in range(4)]
    MO = [Mo[:, i * H:(i + 1) * H] for i in range(4)]

    def m_plane(dst_i, usrc, kcol, joff):
        # dst pos = r*256 + b*4 + m  <-  src pos = r*512 + b*8 + 4*joff + m
        for r in range(RT):
            v.tensor_scalar_mul(
                _ap(dst_i, r * 256, [[4, 64], [1, 4]]),
                _ap(usrc[:], r * T + 4 * joff, [[8, 64], [1, 4]]),
                Kt[:, r * 16 + kcol:r * 16 + kcol + 1])
    for i, (us, kc_) in enumerate(((u0, 0), (u1, 1), (u0, 2), (u1, 3))):
        m_plane(ME[i], us, kc_, 0)
        m_plane(MO[i], us, kc_, 1)

    # ---------------- tree level 1: U2 = Modd @ Meven ----------------
    # U2_cp[B',m] = Mo_c0*Me_0p + Mo_c1*Me_1p, elementwise over (B', m);
    # planes are contiguous so everything is packed (2x fp16).
    es_tree = ExitStack()
    tr = es_tree.enter_context(tc.tile_pool(name="tr", bufs=1))
    U2 = tr.tile([128, 4 * H], FP16, tag="U2")
    g1 = tr.tile([128, 4 * H], FP16, tag="g1")
    g2 = tr.tile([128, 4 * H], FP16, tag="g2")
    for c in range(2):
        # dims (p, B'm): B-side Mo_c0 bcast over p; A-side Me_0p planes
        v.tensor_tensor(_ap(g1[:], 2 * c * H, [[H, 2], [1, H]]),
                        _ap(Mo[:], 2 * c * H, [[0, 2], [1, H]]),
                        _ap(Me[:], 0, [[H, 2], [1, H]]), op=Alu.mult)
        v.tensor_tensor(_ap(g2[:], 2 * c * H, [[H, 2], [1, H]]),
                        _ap(Mo[:], (2 * c + 1) * H, [[0, 2], [1, H]]),
                        _ap(Me[:], 2 * H, [[H, 2], [1, H]]), op=Alu.mult)
    v.tensor_tensor(U2[:], g1[:], g2[:], op=Alu.add)

    # prediction-side mask, chunked so it fills Pool gaps greedily
    Ym = keep.tile([128, W], mybir.dt.uint32, tag="Ym")
    for ch in range(6):
        gp.tensor_scalar(_ap(Ym[:], ch * (W // 6), [[1, W // 6]]),
                         _ap(Yt[:], ch * (W // 6), [[1, W // 6]]),
                         0.5, None, Alu.is_ge)

    # ---------------- tree level 2: U4 ----------------
    # U2 pair-evens at slots {0,1} (contig), odds at {2,3}.
    # U4_cp[B', n] = U2o_c0[B',n]*U2e_0p[B',n] + U2o_c1[B',n]*U2e_1p[B',n]
    # U2 planes: pos(i, B', m) = i*H + B'*4 + m ; even-read: m in {0,1}:
    # [[4,NBT],[1,2]]; odd-read: +2.
    U4 = tr.tile([128, 4 * 2 * NBT], FP16, tag="U4")   # planes cp x (B',n)
    t1 = tr.tile([128, 4 * 2 * NBT], FP16, tag="t1")
    t2 = tr.tile([128, 4 * 2 * NBT], FP16, tag="t2")
    # per c (ISA max 3 free dims), iterate (p, B', n):
    # B-side: U2odd_c{k} at plane (2c+k), slots {2,3}: pos = (2c+k)*H+B'*4+2+n
    # A-side: U2even_{k}p at plane (2k+p), slots {0,1}
    # out t: pos = (2c+p)*2*NBT + B'*2 + n
    for c in range(2):
        dims_out = [[2 * NBT, 2], [2, NBT], [1, 2]]
        v.tensor_tensor(
            _ap(t1[:], c * 2 * 2 * NBT, dims_out),
            _ap(U2[:], 2 * c * H + 2, [[0, 2], [4, NBT], [1, 2]]),
            _ap(U2[:], 0, [[H, 2], [4, NBT], [1, 2]]),
            op=Alu.mult)
        v.tensor_tensor(
            _ap(t2[:], c * 2 * 2 * NBT, dims_out),
            _ap(U2[:], (2 * c + 1) * H + 2, [[0, 2], [4, NBT], [1, 2]]),
            _ap(U2[:], 2 * H, [[H, 2], [4, NBT], [1, 2]]),
            op=Alu.mult)
    v.tensor_tensor(U4[:], t1[:], t2[:], op=Alu.add)

    # ---------------- tree level 3: U8 ----------------
    # U4 planes (B', n) interleaved; strided n-reads (1x), packed add.
    U8 = tr.tile([128, 4 * NBT], FP16, tag="U8")       # planes cp x B'
    t3 = tr.tile([128, 4 * NBT], FP16, tag="t3")
    t4 = tr.tile([128, 4 * NBT], FP16, tag="t4")
    od = [[2 * NBT, 2], [NBT, 2], [1, NBT]]
    v.tensor_tensor(_ap(t3[:], 0, od),
                    _ap(U4[:], 1, [[2 * 2 * NBT, 2], [0, 2], [2, NBT]]),
                    _ap(U4[:], 0, [[0, 2], [2 * NBT, 2], [2, NBT]]),
                    op=Alu.mult)
    v.tensor_tensor(_ap(t4[:], 0, od),
                    _ap(U4[:], 2 * NBT + 1,
                        [[2 * 2 * NBT, 2], [0, 2], [2, NBT]]),
                    _ap(U4[:], 4 * NBT, [[0, 2], [2 * NBT, 2], [2, NBT]]),
                    op=Alu.mult)
    v.tensor_tensor(U8[:], t3[:], t4[:], op=Alu.add)

    # normalize U8 (sum of 4 entries -> 1) to keep radix-8 chains in range
    zn = tr.tile([128, NBT], FP16, tag="zn")
    rz = tr.tile([128, NBT], FP16, tag="rz")
    zn2 = tr.tile([128, NBT], FP16, tag="zn2")
    v.tensor_tensor(_ap(zn[:], 0, [[1, NBT]]),
                    _ap(U8[:], 0, [[1, NBT]]),
                    _ap(U8[:], NBT, [[1, NBT]]), op=Alu.add)
    v.tensor_tensor(_ap(zn2[:], 0, [[1, NBT]]),
                    _ap(U8[:], 2 * NBT, [[1, NBT]]),
                    _ap(U8[:], 3 * NBT, [[1, NBT]]), op=Alu.add)
    v.tensor_tensor(_ap(zn[:], 0, [[1, NBT]]),
                    _ap(zn[:], 0, [[1, NBT]]),
                    _ap(zn2[:], 0, [[1, NBT]]), op=Alu.add)
    v.reciprocal(rz[:], zn[:])
    v.tensor_tensor(_ap(U8[:], 0, [[NBT, 4], [1, NBT]]),
                    _ap(U8[:], 0, [[NBT, 4], [1, NBT]]),
                    _ap(rz[:], 0, [[0, 4], [1, NBT]]), op=Alu.mult)

    # ---------------- radix-8 block scan ----------------
    # blocks b in [0,64) per tile; groups g of 8 blocks (8 groups/tile).
    # Step A: in-group inclusive matrix prefixes P[g, j] (HS shifts 1,2,4).
    # P stored planar like U8: planes cp x (B' = tile*64 + 8g + j).
    es_blk = ExitStack()
    bs = es_blk.enter_context(tc.tile_pool(name="bs", bufs=1))
    P = U8
    for h in (1, 2, 4):
        Pn = bs.tile([128, 4 * NBT], FP16, tag=f"P{h}")
        s1 = bs.tile([128, 4 * NBT], FP16, tag=f"s1_{h}")
        s2 = bs.tile([128, 4 * NBT], FP16, tag=f"s2_{h}")
        n = 8 - h
        # C[i] = P[i] * P[i-h] for i in [h,8) within each group
        # per c: dims (p, g, i); B-side P_c{k}[i] at plane (2c+k)
        go = [[NBT, 2], [8, NBT // 8], [1, n]]
        for c in range(2):
            v.tensor_tensor(
                _ap(s1[:], c * 2 * NBT + h, go),
                _ap(P[:], 2 * c * NBT + h, [[0, 2], [8, NBT // 8], [1, n]]),
                _ap(P[:], 0, [[NBT, 2], [8, NBT // 8], [1, n]]),
                op=Alu.mult)
            v.tensor_tensor(
                _ap(s2[:], c * 2 * NBT + h, go),
                _ap(P[:], (2 * c + 1) * NBT + h,
                    [[0, 2], [8, NBT // 8], [1, n]]),
                _ap(P[:], 2 * NBT, [[NBT, 2], [8, NBT // 8], [1, n]]),
                op=Alu.mult)
        v.tensor_tensor(_ap(Pn[:], h, [[NBT, 4], [8, NBT // 8], [1, n]]),
                        _ap(s1[:], h, [[NBT, 4], [8, NBT // 8], [1, n]]),
                        _ap(s2[:], h, [[NBT, 4], [8, NBT // 8], [1, n]]),
                        op=Alu.add)
        # heads [0,h) copy through (DVE: keeps the chain on one queue --
        # an Act round-trip here costs ~2 sem hops + 185ns SBUF latency)
        v.tensor_copy(out=_ap(Pn[:], 0, [[NBT, 4], [8, NBT // 8], [1, h]]),
                      in_=_ap(P[:], 0, [[NBT, 4], [8, NBT // 8], [1, h]]))
        P = Pn

    # Step B: group totals Tg = P[g,7]; normalize; tiny inclusive scan
    # over the 8 groups per tile (HS 1,2,4); then vg = Escan[g-1] @ alpha1.
    # Tg planar: planes cp x (tile r, g): width 4 * 24.
    NG = RT * 8
    Tg = bs.tile([128, 4 * NG], FP16, tag="Tg")
    v.tensor_copy(out=_ap(Tg[:], 0, [[NG, 4], [1, NG]]),
                  in_=_ap(P[:], 7, [[NBT, 4], [8, NG]]))
    # normalize Tg
    zg = bs.tile([128, NG], FP16, tag="zg")
    rg = bs.tile([128, NG], FP16, tag="rg")
    zg2 = bs.tile([128, NG], FP16, tag="zg2")
    v.tensor_tensor(zg[:], _ap(Tg[:], 0, [[1, NG]]),
                    _ap(Tg[:], NG, [[1, NG]]), op=Alu.add)
    v.tensor_tensor(zg2[:], _ap(Tg[:], 2 * NG, [[1, NG]]),
                    _ap(Tg[:], 3 * NG, [[1, NG]]), op=Alu.add)
    v.tensor_tensor(zg[:], zg[:], zg2[:], op=Alu.add)
    v.reciprocal(rg[:], zg[:])
    v.tensor_tensor(_ap(Tg[:], 0, [[NG, 4], [1, NG]]),
                    _ap(Tg[:], 0, [[NG, 4], [1, NG]]),
                    _ap(rg[:], 0, [[0, 4], [1, NG]]), op=Alu.mult)
    E = Tg
    for h in (1, 2, 4):
        En = bs.tile([128, 4 * NG], FP16, tag=f"E{h}")
        e1 = bs.tile([128, 4 * NG], FP16, tag=f"e1_{h}")
        e2 = bs.tile([128, 4 * NG], FP16, tag=f"e2_{h}")
        n = 8 - h
        go = [[NG, 2], [8, RT], [1, n]]
        for c in range(2):
            v.tensor_tensor(
                _ap(e1[:], c * 2 * NG + h, go),
                _ap(E[:], 2 * c * NG + h, [[0, 2], [8, RT], [1, n]]),
                _ap(E[:], 0, [[NG, 2], [8, RT], [1, n]]),
                op=Alu.mult)
            v.tensor_tensor(
                _ap(e2[:], c * 2 * NG + h, go),
                _ap(E[:], (2 * c + 1) * NG + h, [[0, 2], [8, RT], [1, n]]),
                _ap(E[:], 2 * NG, [[NG, 2], [8, RT], [1, n]]),
                op=Alu.mult)
        v.tensor_tensor(_ap(En[:], h, [[NG, 4], [8, RT], [1, n]]),
                        _ap(e1[:], h, [[NG, 4], [8, RT], [1, n]]),
                        _ap(e2[:], h, [[NG, 4], [8, RT], [1, n]]),
                        op=Alu.add)
        v.tensor_copy(out=_ap(En[:], 0, [[NG, 4], [8, RT], [1, h]]),
                      in_=_ap(E[:], 0, [[NG, 4], [8, RT], [1, h]]))
        E = En

    # vg[g] = E[g-1] @ alpha1 for g>=1 ; vg[0] = alpha1.  alpha1 per-tile
    # scalars K cols 12,13.  v-planes: vg0/vg1 width NG.
    vg = bs.tile([128, 2 * NG], FP16, tag="vg")
    vt = bs.tile([128, 2 * NG], FP16, tag="vt")
    for r in range(RT):
        a0 = Kt[:, r * 16 + 12:r * 16 + 13]
        a1 = Kt[:, r * 16 + 13:r * 16 + 14]
        for comp in range(2):
            # vg[comp][r, g] = E_{comp,0}[g-1]*a0 + E_{comp,1}[g-1]*a1
            seg7 = [[1, 7]]
            v.tensor_scalar_mul(
                _ap(vt[:], comp * NG + r * 8 + 1, seg7),
                _ap(E[:], (2 * comp + 1) * NG + r * 8, seg7), a1)
            v.scalar_tensor_tensor(
                _ap(vg[:], comp * NG + r * 8 + 1, seg7),
                _ap(E[:], (2 * comp) * NG + r * 8, seg7), a0,
                _ap(vt[:], comp * NG + r * 8 + 1, seg7), Alu.mult, Alu.add)
        v.tensor_copy(out=_ap(vg[:], r * 8, [[NG, 2], [1, 1]]),
                      in_=_ap(Kt[:], r * 16 + 12, [[1, 2], [0, 1]]))
    # normalize vg
    zv = bs.tile([128, NG], FP16, tag="zv")
    rv = bs.tile([128, NG], FP16, tag="rv")
    v.tensor_tensor(zv[:], _ap(vg[:], 0, [[1, NG]]),
                    _ap(vg[:], NG, [[1, NG]]), op=Alu.add)
    v.reciprocal(rv[:], zv[:])
    v.tensor_tensor(_ap(vg[:], 0, [[NG, 2], [1, NG]]),
                    _ap(vg[:], 0, [[NG, 2], [1, NG]]),
                    _ap(rv[:], 0, [[0, 2], [1, NG]]), op=Alu.mult)

    # Step C: w_b for all blocks.  w[8g] = vg[g]; w[8g+j] = P[g,j-1] @ vg[g].
    # w planes: w0/w1 width NBT (B'-indexed).
    wb = tr.tile([128, 2 * NBT], FP16, tag="wb")
    wt1 = bs.tile([128, 2 * NBT], FP16, tag="wt1")
    wt2 = bs.tile([128, 2 * NBT], FP16, tag="wt2")
    # dims (comp, g, j in 1..7): w_c = P_c0[g,j-1]*vg_0[g] + P_c1[g,j-1]*vg_1[g]
    wo = [[NBT, 2], [8, NBT // 8], [1, 7]]
    v.tensor_tensor(
        _ap(wt1[:], 1, wo),
        _ap(P[:], 0, [[2 * NBT, 2], [8, NBT // 8], [1, 7]]),
        _ap(vg[:], 0, [[0, 2], [1, NBT // 8], [0, 7]]),
        op=Alu.mult)
    v.tensor_tensor(
        _ap(wt2[:], 1, wo),
        _ap(P[:], NBT, [[2 * NBT, 2], [8, NBT // 8], [1, 7]]),
        _ap(vg[:], NG, [[0, 2], [1, NBT // 8], [0, 7]]),
        op=Alu.mult)
    v.tensor_tensor(_ap(wb[:], 1, wo), _ap(wt1[:], 1, wo),
                    _ap(wt2[:], 1, wo), op=Alu.add)
    v.tensor_copy(out=_ap(wb[:], 0, [[NBT, 2], [8, NBT // 8], [1, 1]]),
                  in_=_ap(vg[:], 0, [[NG, 2], [1, NBT // 8], [0, 1]]))
    # normalize w
    zw = bs.tile([128, NBT], FP16, tag="zw")
    rw = bs.tile([128, NBT], FP16, tag="rw")
    v.tensor_tensor(zw[:], _ap(wb[:], 0, [[1, NBT]]),
                    _ap(wb[:], NBT, [[1, NBT]]), op=Alu.add)
    v.reciprocal(rw[:], zw[:])
    v.tensor_tensor(_ap(wb[:], 0, [[NBT, 2], [1, NBT]]),
                    _ap(wb[:], 0, [[NBT, 2], [1, NBT]]),
                    _ap(rw[:], 0, [[0, 2], [1, NBT]]), op=Alu.mult)
    es_blk.close()

    # ---------------- within-block down-sweep ----------------
    # Alpha planes AL0/AL1, width W, slot layout (B', s: 8),
    # s = bitrev3(j): even slots 0..3 hold j = 0,4,2,6; odd 4..7: 1,5,3,7.
    es_al = ExitStack()
    alp = es_al.enter_context(tc.tile_pool(name="alp", bufs=1))
    AL = keep.tile([128, 2 * W], FP16, tag="AL")  # AL0 | AL1
    a1t = alp.tile([128, 2 * NBT], FP16, tag="a1t")
    a2t = alp.tile([128, 2 * NBT], FP16, tag="a2t")
    # slot 0 (j=0) = w
    sc.copy(_ap(AL[:], 0, [[W, 2], [8, NBT], [1, 1]]),
            _ap(wb[:], 0, [[NBT, 2], [1, NBT], [0, 1]]))
    # stage 1: slot 1 (j=4) = U4[node0] @ w ; U4 node0 = strided n=0 reads
    v.tensor_tensor(
        _ap(a1t[:], 0, [[NBT, 2], [1, NBT]]),
        _ap(U4[:], 0, [[2 * 2 * NBT, 2], [2, NBT]]),
        _ap(wb[:], 0, [[0, 2], [1, NBT]]), op=Alu.mult)
    v.tensor_tensor(
        _ap(a2t[:], 0, [[NBT, 2], [1, NBT]]),
        _ap(U4[:], 2 * NBT, [[2 * 2 * NBT, 2], [2, NBT]]),
        _ap(wb[:], NBT, [[0, 2], [1, NBT]]), op=Alu.mult)
    v.tensor_tensor(_ap(AL[:], 1, [[W, 2], [8, NBT]]),
                    _ap(a1t[:], 0, [[NBT, 2], [1, NBT]]),
                    _ap(a2t[:], 0, [[NBT, 2], [1, NBT]]), op=Alu.add)
    # stage 2: slots 2,3 (j=2,6) = U2[pair-even p1] @ AL[slots 0,1]
    # U2 even-pair slots {0,1}: pos = i*H + B'*4 + m, m in {0,1}
    b1 = alp.tile([128, 2 * 2 * NBT], FP16, tag="b1")
    b2 = alp.tile([128, 2 * 2 * NBT], FP16, tag="b2")
    s2o = [[2 * NBT, 2], [2, NBT], [1, 2]]
    v.tensor_tensor(
        _ap(b1[:], 0, s2o),
        _ap(U2[:], 0, [[2 * H, 2], [4, NBT], [1, 2]]),
        _ap(AL[:], 0, [[0, 2], [8, NBT], [1, 2]]), op=Alu.mult)
    v.tensor_tensor(
        _ap(b2[:], 0, s2o),
        _ap(U2[:], H, [[2 * H, 2], [4, NBT], [1, 2]]),
        _ap(AL[:], W, [[0, 2], [8, NBT], [1, 2]]), op=Alu.mult)
    v.tensor_tensor(_ap(AL[:], 2, [[W, 2], [8, NBT], [1, 2]]),
                    _ap(b1[:], 0, s2o), _ap(b2[:], 0, s2o), op=Alu.add)
    # stage 3: odd slots 4..7 (j=1,5,3,7) = M_even @ AL[even slots]
    c1 = alp.tile([128, 2 * W // 2], FP16, tag="c1")
    c2 = alp.tile([128, 2 * W // 2], FP16, tag="c2")
    # per row-tile so tile-0 predictions can start before tiles 1-2 finish
    for r in range(RT):
        ob4 = r * 256
        oa = r * T
        s3o = [[H, 2], [4, 64], [1, 4]]
        v.tensor_tensor(
            _ap(c1[:], ob4, s3o),
            _ap(Me[:], ob4, [[2 * H, 2], [4, 64], [1, 4]]),
            _ap(AL[:], oa, [[0, 2], [8, 64], [1, 4]]), op=Alu.mult)
        v.tensor_tensor(
            _ap(c2[:], ob4, s3o),
            _ap(Me[:], H + ob4, [[2 * H, 2], [4, 64], [1, 4]]),
            _ap(AL[:], W + oa, [[0, 2], [8, 64], [1, 4]]), op=Alu.mult)
        v.tensor_tensor(_ap(AL[:], 4 + oa, [[W, 2], [8, 64], [1, 4]]),
                        _ap(c1[:], ob4, s3o), _ap(c2[:], ob4, s3o),
                        op=Alu.add)

    # -------- predictions + lp + cumsum + q, pipelined per row-tile --------
    # per tile r: DVE (Za, Ra, rr, q1) -> Act (q0, ln-unpermute) -> DVE
    # (mask, copy-pred, scan, q-adds) -> relayout DMAs; tiles overlap engines.
    es_pr = ExitStack()
    pr = es_pr.enter_context(tc.tile_pool(name="pr", bufs=1))
    Za = pr.tile([128, W], FP16, tag="Za")
    Ra = pr.tile([128, W], FP16, tag="Ra")
    rr = pr.tile([128, W], FP16, tag="rr")
    q1 = pr.tile([128, W], FP16, tag="q1")
    p1 = keep.tile([128, W], F32, tag="p1")
    p0 = keep.tile([128, W], F32, tag="p0")
    lp = keep.tile([128, W], F32, tag="lp")
    # apc2 has one zero column before each tile's T cumsum columns so the
    # q = p + apc[t-1] add runs full-T with no single-element edge copies.
    apc2 = keep.tile([128, W + RT], F32, tag="apc2")
    q1c = keep.tile([128, W], F32, tag="q1c")
    q0c = keep.tile([128, W], F32, tag="q0c")
    # ability planes split in two tiles so the a0-3 max tree isn't blocked
    # on ability 4's late relayout; partitions [0:64) hold k=1, [64:128) k=0
    # (k=1 lower so tile2's identity move is the later-computed q1).
    QA03 = keep.tile([128, 4 * T], F32, tag="QA03")
    QA4 = keep.tile([128, T], F32, tag="QA4")
    psq = ctx.enter_context(tc.tile_pool(name="psq", bufs=1, space="PSUM"))
    gp.memset(_ap(apc2[:], 0, [[T + 1, RT]]), 0.0)
    slp = [[1, T]]

    def pred_front(r):
        o = r * T
        v.tensor_tensor(_ap(Za[:], o, slp), _ap(AL[:], o, slp),
                        _ap(AL[:], W + o, slp), op=Alu.add)
        v.reciprocal(_ap(Ra[:], o, slp), _ap(Za[:], o, slp))
        v.tensor_tensor(_ap(rr[:], o, slp), _ap(AL[:], W + o, slp),
                        _ap(Ra[:], o, slp), op=Alu.mult)
        v.tensor_tensor(_ap(q1[:], o, slp), _ap(rr[:], o, slp),
                        _ap(dm[:], o, slp), op=Alu.mult)
        v.tensor_tensor(_ap(q1[:], o, slp), _ap(q1[:], o, slp),
                        _ap(pgs[:], o, slp), op=Alu.add)
        # ln with unpermute slot->natural (split by j0); p0 = ln(1 - q1)
        # fuses the complement into the activation's scale/bias
        for j0 in range(2):
            sc.activation(
                _ap(p1[:], o + j0, [[8, 64], [2, 2], [4, 2]]),
                _ap(q1[:], o + 4 * j0, [[8, 64], [2, 2], [1, 2]]),
                Act.Ln)
            sc.activation(
                _ap(p0[:], o + j0, [[8, 64], [2, 2], [4, 2]]),
                _ap(q1[:], o + 4 * j0, [[8, 64], [2, 2], [1, 2]]),
                Act.Ln, bias=1.0, scale=-1.0)

    def pred_scan(r):
        o = r * T
        sc.copy(_ap(lp[:], o, slp), _ap(p0[:], o, slp))
        v.copy_predicated(_ap(lp[:], o, slp), _ap(Ym[:], o, slp),
                          _ap(p1[:], o, slp))
        v.tensor_tensor_scan(_ap(apc2[:], r * (T + 1) + 1, slp),
                             _ap(lp[:], o, slp),
                             _ap(lp[:], o, slp),
                             0.0, Alu.add, Alu.bypass)

    def _qa_dst(k, a):
        base, off = (QA4, 0) if a == 4 else (QA03, a * T)
        return _ap_p(base[:], 64 * (1 - k), 64, off, [[1, T]])

    def pred_q(r):
        # q_k = p_k + apc[t-1] (k=0 on DVE, k=1 on Pool for tiles 0/1),
        # then relayout into QA: moves with matching partition ranges
        # (half == 1-k) are on-chip copies (deferred so they don't block
        # the next tile's critical ops); cross moves are HWDGE DMAs --
        # keeping them off Pool's SWDGE avoids descriptor-gen queueing
        # behind Pool's q1 adds.
        o = r * T
        na = 2 if r < 2 else 1   # tile 2 holds only ability 4 (rows 0-63)
        for k, qsrc, psrc in ((0, q0c, p0), (1, q1c, p1)):
            qeng = gp if (k == 1 and r < 2) else v
            qeng.tensor_tensor(_ap(qsrc[:], o, slp),
                               _ap(psrc[:], o, slp),
                               _ap(apc2[:], r * (T + 1), slp), op=Alu.add)
            for half in range(na):
                if half == 1 - k:
                    continue
                sy.dma_start(_qa_dst(k, 2 * r + half),
                             _ap_p(qsrc[:], 64 * half, 64, o, [[1, T]]))

    for r in range(2):
        pred_front(r)
        pred_scan(r)
        pred_q(r)
    pred_front(2)
    pred_scan(2)
    # k=0 identity copies for tiles 0/1, emitted here so they sit in the
    # Act queue ahead of tile 2's q consumers but after its Ln/lp ops
    for r in range(2):
        sc.copy(_qa_dst(0, 2 * r + 1),
                _ap_p(q0c[:], 64, 64, r * T, [[1, T]]))
    pred_q(2)
    # deferred identity copies: k=1 planes for tiles 0/1 on Pool (after
    # both q1 adds), and tile 2's late a4 k=1 move on Act
    for r in range(2):
        gp.tensor_copy(out=_qa_dst(1, 2 * r),
                       in_=_ap_p(q1c[:], 0, 64, r * T, [[1, T]]))
    sc.copy(_qa_dst(1, 4), _ap_p(q1c[:], 0, 64, 2 * T, [[1, T]]))
    es_pr.close()
    es_al.close()
    es_tree.close()
    es_in.close()

    # ---------------- collapse over abilities ----------------
    col2 = ctx.enter_context(tc.tile_pool(name="col2", bufs=1))

    MX = col2.tile([128, T], F32, tag="MX")
    DF = col2.tile([128, A_LEV * T], FP16, tag="DF")
    EX = col2.tile([128, A_LEV * T], FP16, tag="EX")
    SM = col2.tile([128, T], F32, tag="SM")
    un = col2.tile([128, T], F32, tag="un")
    mt = col2.tile([128, 2 * T], F32, tag="mt")
    mth = col2.tile([128, 2 * T], FP16, tag="mth")
    psp = ctx.enter_context(tc.tile_pool(name="psp", bufs=1, space="PSUM"))
    un1s = psp.tile([64, T], F32, tag="un1s")
    dl = col2.tile([64, T], F32, tag="dl")
    ed = col2.tile([64, T], F32, tag="ed")
    sp = col2.tile([64, T], F32, tag="sp")
    OI = col2.tile([64, 2 * T], F32, tag="OI")
    # t-chunked 3-engine pipeline over the collapse.  The a0-3 max tree
    # reads only QA03, so it runs while ability 4's relayout is in flight;
    # only MX/DF wait for QA4.  Partitions [0:64) hold k=1, [64:128) k=0,
    # so dl = un1 - un0 and out0 = -softplus(dl), out1 = dl - softplus(dl).
    NCH = 4
    HT = T // NCH

    def cmax(t0):
        hl = [[1, HT]]
        v.tensor_tensor(_ap(mt[:], t0, [[T, 2], [1, HT]]),
                        _ap(QA03[:], t0, [[T, 2], [1, HT]]),
                        _ap(QA03[:], 2 * T + t0, [[T, 2], [1, HT]]),
                        op=Alu.max)
        v.tensor_tensor(_ap(mt[:], t0, hl), _ap(mt[:], t0, hl),
                        _ap(mt[:], T + t0, hl), op=Alu.max)

    def cdf(t0):
        hl = [[1, HT]]
        v.tensor_tensor(_ap(MX[:], t0, hl), _ap(mt[:], t0, hl),
                        _ap(QA4[:], t0, hl), op=Alu.max)
        v.tensor_tensor(_ap(DF[:], t0, [[T, 4], [1, HT]]),
                        _ap(QA03[:], t0, [[T, 4], [1, HT]]),
                        _ap(MX[:], t0, [[0, 4], [1, HT]]),
                        op=Alu.subtract)
        v.tensor_tensor(_ap(DF[:], 4 * T + t0, hl),
                        _ap(QA4[:], t0, hl),
                        _ap(MX[:], t0, hl), op=Alu.subtract)
        sc.activation(_ap(EX[:], t0, [[T, A_LEV], [1, HT]]),
                      _ap(DF[:], t0, [[T, A_LEV], [1, HT]]), Act.Exp)

    def csum(t0):
        hl = [[1, HT]]
        gp.tensor_tensor(_ap(mth[:], t0, [[T, 2], [1, HT]]),
                         _ap(EX[:], t0, [[T, 2], [1, HT]]),
                         _ap(EX[:], 2 * T + t0, [[T, 2], [1, HT]]),
                         op=Alu.add)
        gp.tensor_tensor(_ap(mth[:], t0, hl), _ap(mth[:], t0, hl),
                         _ap(mth[:], T + t0, hl), op=Alu.add)
        v.tensor_tensor(_ap(SM[:], t0, hl), _ap(mth[:], t0, hl),
                        _ap(EX[:], 4 * T + t0, hl), op=Alu.add)
        sc.activation(_ap(SM[:], t0, hl), _ap(SM[:], t0, hl), Act.Ln)
        v.tensor_tensor(_ap(un[:], t0, hl), _ap(MX[:], t0, hl),
                        _ap(SM[:], t0, hl), op=Alu.add)
        # partition shift via idle PE: un1s[j] = un[64+j] (k=0 half down)
        nc.tensor.matmul(_ap_p(un1s[:], 0, 64, t0, hl),
                         SHt[:, 0:64], _ap_p(un[:], 0, 128, t0, hl),
                         start=True, stop=True)

    def ctail(t0, HL, oq):
        hl = [[1, HL]]
        v.tensor_tensor(_ap_p(dl[:], 0, 64, t0, hl),
                        _ap_p(un[:], 0, 64, t0, hl),
                        _ap_p(un1s[:], 0, 64, t0, hl), op=Alu.subtract)
        sc.activation(_ap_p(ed[:], 0, 64, t0, hl),
                      _ap_p(dl[:], 0, 64, t0, hl), Act.Exp)
        sc.activation(_ap_p(sp[:], 0, 64, t0, hl),
                      _ap_p(ed[:], 0, 64, t0, hl), Act.Ln, bias=1.0)
        gp.tensor_scalar(_ap_p(OI[:], 0, 64, 2 * t0, [[2, HL]]),
                         _ap_p(sp[:], 0, 64, t0, hl),
                         -1.0, None, Alu.mult)
        v.tensor_tensor(_ap_p(OI[:], 0, 64, 2 * t0 + 1, [[2, HL]]),
                        _ap_p(dl[:], 0, 64, t0, hl),
                        _ap_p(sp[:], 0, 64, t0, hl), op=Alu.subtract)
        oq.dma_start(bass.AP(O[:].tensor, 2 * t0, [[2 * T, 64], [1, 2 * HL]]),
                     _ap_p(OI[:], 0, 64, 2 * t0, [[1, 2 * HL]]))

    for c in range(NCH):
        cmax(c * HT)
    for c in range(NCH):
        cdf(c * HT)
    for c in range(NCH):
        csum(c * HT)
    for c in range(NCH):
        ctail(c * HT, HT, gp if c < 2 else sy)


def _steer_act_tables(arch):
    """Keep Exp/Ln claimed by one table set (see kernel v1)."""
    from concourse import hw_specs
    tabs = hw_specs.get_activation_tables(arch)
    for name, funcs in tabs.items():
        if name == "natural_log_exp_and_others":
            continue
        funcs.discard(Act.Exp)
        funcs.discard(Act.Ln)


def _build_program():
    nc = bacc.Bacc()
    _steer_act_tables(nc.m.arch)
    U0 = nc.declare_dram_parameter("U0", [RT * 128, T], FP16, isOutput=False)
    U1 = nc.declare_dram_parameter("U1", [RT * 128, T], FP16, isOutput=False)
    PG = nc.declare_dram_parameter("PG", [RT * 128, T], FP16, isOutput=False)
    DM = nc.declare_dram_parameter("DM", [RT * 128, T], FP16, isOutput=False)
    Y = nc.declare_dram_parameter("Y", [RT * 128, T], FP16, isOutput=False)
    K = nc.declare_dram_parameter("K", [RT * 128, 16], F32, isOutput=False)
    SH = nc.declare_dram_parameter("SH", [128, 128], F32, isOutput=False)
    O = nc.declare_dram_parameter("O", [BL, T, 2], F32, isOutput=True)
    with ExitStack() as ctx:
        tc = ctx.enter_context(tile.TileContext(nc))
        with nc.allow_low_precision(reason="fp16 HMM chain; validated vs gate"):
            _emit(ctx, tc, nc, U0, U1, PG, DM, Y, K, SH, O)
    if not nc.is_finalized():
        nc.finalize()
    return nc


def _pad_rows(x, dtype=np.float32, fill=0.0):
    out = np.full((RT * 128, x.shape[1]), fill, dtype=dtype)
    out[:ROWS] = x
    return out


def kernel(corr, ytrue, problem, kc, dyn_emb, obs_logits_problem,
           obs_logits_kc, ability_levels, traj, trans_ind, pred_ind):
    global _last_results, _cached_nc
    import ml_dtypes
    fp16 = np.float16

    corr = np.asarray(corr, dtype=np.float32)
    ytrue = np.asarray(ytrue, dtype=np.float32)
    problem = np.asarray(problem)
    kc = np.asarray(kc)
    dyn_emb = np.asarray(dyn_emb, dtype=np.float32)
    obs_logits_problem = np.asarray(obs_logits_problem, dtype=np.float32)
    obs_logits_kc = np.asarray(obs_logits_kc, dtype=np.float32)
    ability = np.asarray(ability_levels, dtype=np.float32)

    obs_core = obs_logits_problem[problem] + obs_logits_kc[kc][:, None, :]
    dyn = dyn_emb[kc]
    sig = lambda x: 1.0 / (1.0 + np.exp(-x.astype(np.float64)))
    lL, lF, lI0 = dyn[:, 0], dyn[:, 1], dyn[:, 2]
    AT00, AT01 = sig(-lL), sig(lF)
    AT10, AT11 = sig(lL), sig(-lF)
    al = [AT00, AT01, AT10, AT11]
    alpha = [al[2 * (i // 2)] * al[i % 2] for i in range(4)]
    # alpha_cp = AT_c0*AT_0p: (c,p): c0 entry = AT[c][0] = al[2c], AT[0][p]=al[p]
    alpha = [al[2 * (i // 2)] * al[i % 2] for i in range(4)]
    beta = [al[2 * (i // 2) + 1] * al[2 + i % 2] for i in range(4)]
    Kfull = np.stack(al + alpha + beta +
                     [sig(-lI0), sig(lI0), np.zeros_like(lL),
                      np.zeros_like(lL)], axis=1).astype(np.float32)  # (B,16)

    # permute the T axis within each 8-block so storage slot s holds
    # natural step j = bitrev3(s); Y stays natural (cumsum order).
    perm = (np.arange(T) & ~7) + np.tile(
        np.array([0, 4, 2, 6, 1, 5, 3, 7]), T // 8)

    in_maps = []
    for c in range(N_CORES):
        sl = slice(c * BL, (c + 1) * BL)
        g = obs_core[sl, :, 0][None, :, perm] + ability[:, None, None]
        s = obs_core[sl, :, 1][None, :, perm] - ability[:, None, None]
        ct = np.broadcast_to(corr[sl][:, perm][None], (A_LEV, BL, T))
        yt = np.broadcast_to(ytrue[sl][None], (A_LEV, BL, T))
        # observation likelihood diagonals, normalized to sum 1 per step
        c2 = 2.0 * ct - 1.0
        u0r = sig(c2 * g)
        u1r = sig(-c2 * s)
        zu = u0r + u1r
        u0n = (u0r / zu).astype(np.float32)
        pg = sig(g)
        dmv = (sig(-s) - pg).astype(np.float32)
        kt = np.broadcast_to(Kfull[sl][None], (A_LEV, BL, 16))
        kpad = _pad_rows(kt.reshape(ROWS, 16), np.float32)
        kpad[ROWS:] = 0.5            # benign transition probs on padded rows
        shm = np.zeros((128, 128), np.float32)
        shm[np.arange(64) + 64, np.arange(64)] = 1.0
        shm[np.arange(64), np.arange(64) + 64] = 1.0
        # pad rows get benign 0.5 probabilities so no inf/nan ever forms
        # there (the PE half-swap matmuls contract over all partitions and
        # 0 * nan would poison valid lanes)
        in_maps.append({
            "U0": _pad_rows(u0n.reshape(ROWS, T), fp16, 0.5),
            "U1": _pad_rows((1.0 - u0n).reshape(ROWS, T), fp16, 0.5),
            "PG": _pad_rows(pg.reshape(ROWS, T).astype(np.float32), fp16, 0.5),
            "DM": _pad_rows(dmv.reshape(ROWS, T), fp16),
            "Y": _pad_rows(yt.reshape(ROWS, T), fp16),
            "K": kpad,
            "SH": shm,
        })

    if _cached_nc is None:
        _cached_nc = _build_program()

    res = run_bass_kernel_spmd(
        _cached_nc, in_maps, list(range(N_CORES)),
        trace=bool(os.environ.get("BASS_TRACE")),
    )
    _last_results = res
    out = np.concatenate([res.results[i]["O"] for i in range(N_CORES)], axis=0)
    return out.astype(np.float32)



# revision 75
# speedup vs baseline: 1.0182x; 1.0182x over previous
"""BKT model kernel v2 for Trainium2 (8 NeuronCores, Bass/Tile).

Exact 2-state HMM reformulation of the reference's 2^n-trajectory fastBKT
(see kernel v1 docstring).  v2 restructures for the DVE cost model:

- fp16 for the whole matrix chain (obs probs, level matrices, tree products,
  alphas, predictions).  The chain is contractive and sum-normalized, so
  fp16's 2^-11 rounding keeps the final error ~2e-3 << the 2e-2 gate;
  subnormal flushes only hit entries whose contribution is negligible.
- planar 2x2-entry planes (one buffer region per matrix entry) so
  tensor_tensor ops read/write packed last dims -> DVE 2x mode; per-partition
  transition constants ride tensor_scalar (2x/4x) and Act-engine scale APs.
- within-block (8-step) products use the A^T = gamma*I + 1 v^T structure at
  level 1, a "parity-split" pair layout for levels 2-3, and a 3-stage vector
  down-sweep for the per-step alphas.
- the 64-block scan is radix-8: in-group Hillis-Steele matrix prefixes,
  a tiny 8-group matrix scan, then one batched mat-vec to get per-block
  start alphas.
- Act engine absorbs sigmoids/copies/lns (including the bit-reversal
  unpermute via 4-free-dim APs); Pool absorbs reductions off the DVE path.
- log-predictions, cumsum and the ability-collapse stay f32.

Sharding: data-parallel over students (B=512 -> 64 per core); 5 ability
levels x 64 students = 320 rows padded to 3 x 128-partition tiles.
"""

import os
import numpy as np
from contextlib import ExitStack

import concourse.bass as bass
import concourse.bacc as bacc
import concourse.mybir as mybir
from concourse import tile
from concourse.bass_utils import run_bass_kernel_spmd

F32 = mybir.dt.float32
FP16 = mybir.dt.float16
Alu = mybir.AluOpType
Act = mybir.ActivationFunctionType
AX = mybir.AxisListType

N_CORES = 8
B_FULL = 512
T = 512
A_LEV = 5
BL = B_FULL // N_CORES          # students per core = 64
ROWS = A_LEV * BL               # valid rows per core = 320
RT = 3                          # row tiles of 128 (384 rows incl. pad)
NBT = RT * 64                   # blocks spanning tiles = 192
W = RT * T                      # full-plane free width = 1536
H = W // 2                      # half width = 768
ABILITY = np.array([-2.0, -1.0, 0.0, 1.0, 2.0], dtype=np.float32)

_last_results = None
_cached_nc = None


def _ap(base, off, dims):
    """Custom AP on the same tensor as `base`, keeping its partition dim."""
    return bass.AP(base.tensor, base.offset + off, [list(base.ap[0])] + dims)


def _ap_p(base, poff, pcount, off, dims):
    p = list(base.ap[0])
    pstride = p[0]
    return bass.AP(
        base.tensor, base.offset + poff * pstride + off, [[pstride, pcount]] + dims
    )


def _emit(ctx, tc, nc, U0, U1, PG, DM, Y, K, SH, O):
    v = nc.vector
    sc = nc.scalar
    gp = nc.gpsimd
    sy = nc.sync

    keep = ctx.enter_context(tc.tile_pool(name="keep", bufs=1))

    # ---------------- input DMAs ----------------
    # U0/U1 are the normalized per-step observation likelihood diagonals
    # (host-side sigmoids, slot-ordered); PG = P(y=1|unlearned) and
    # DM = P(y=1|learned) - PG feed the predictions.  K first (tiny, the
    # M planes need its scalars), then U0/U1 per row-tile on the HWDGE
    # queue; PG/DM/Y trail on Pool's SWDGE (needed only by the preds).
    es_in = ExitStack()
    io = es_in.enter_context(tc.tile_pool(name="io", bufs=1))
    u0 = io.tile([128, W], FP16, tag="U0")
    u1 = io.tile([128, W], FP16, tag="U1")
    pgs = keep.tile([128, W], FP16, tag="PG")
    dm = keep.tile([128, W], FP16, tag="DM")
    Yt = keep.tile([128, W], FP16, tag="Y")
    Kt = keep.tile([128, RT * 16], F32, tag="K")
    gp.dma_start(_ap(Kt[:], 0, [[16, RT], [1, 16]]),
                 bass.AP(K[:].tensor, 0, [[16, 128], [128 * 16, RT], [1, 16]]))
    for r in range(RT):
        for dram, sb in ((U0, u0), (U1, u1)):
            sy.dma_start(_ap(sb[:], r * T, [[1, T]]),
                         bass.AP(dram[:].tensor, r * 128 * T,
                                 [[T, 128], [1, T]]))
    for r in range(RT):
        for dram, sb in ((PG, pgs), (DM, dm)):
            gp.dma_start(_ap(sb[:], r * T, [[1, T]]),
                         bass.AP(dram[:].tensor, r * 128 * T,
                                 [[T, 128], [1, T]]))
    gp.dma_start(_ap(Yt[:], 0, [[T, RT], [1, T]]),
                 bass.AP(Y[:].tensor, 0, [[T, 128], [128 * T, RT], [1, T]]))
    # partition-half swap matrix SW[i, j] = 1 iff |i-j| == 64: PE matmuls
    # with it (or its left half) replace SBUF->SBUF partition-shift DMAs
    SHt = keep.tile([128, 128], F32, tag="SH")
    sy.dma_start(SHt[:], bass.AP(SH[:].tensor, 0, [[128, 128], [1, 128]]))

    def KC(col):
        """Per-partition scalar AP for K column `col` of row-tile r -- but all
        tiles share the op; K scalars differ per tile, so ops over multi-tile
        widths must pass per-tile slices.  Helper returns slice for tile r."""
        return Kt[:, col:col + 1]

    # K layout (16 cols per tile r at r*16):
    # 0..3 : A^T entries AT00, AT01, AT10, AT11
    # 4..7 : alpha_cp = AT_c0*AT_0p   (order 00,01,10,11)
    # 8..11: beta_cp  = AT_c1*AT_1p
    # 12,13: alpha1 init (s(-lI0), s(lI0))

    # ---------------- split u-halves ----------------
    # U0/U1 arrive from the host with the T axis permuted within each
    # 8-block: storage slot s holds natural step j = bitrev3(s), i.e. slot
    # order j = (0,4,2,6,1,5,3,7).  Slots 0..3 are exactly the even-j
    # "parity-split" order j_even(m) = 4*(m&1)+2*(m>>1) the M planes want,
    # slots 4..7 the odds.  M-plane reads are packed (stride-1 runs of 4)
    # -> DVE 4x, and each row-tile r starts as soon as its U DMAs land.
    Me = keep.tile([128, 4 * H], FP16, tag="Me")
    Mo = keep.tile([128, 4 * H], FP16, tag="Mo")
    ME = [Me[:, i * H:(i + 1) * H] for i in range(4)]
    MO = [Mo[:, i * H:(i + 1) * H] for i in range(4)]

    def m_plane(dst_i, usrc, kcol, joff):
        # dst pos = r*256 + b*4 + m  <-  src pos = r*512 + b*8 + 4*joff + m
        for r in range(RT):
            v.tensor_scalar_mul(
                _ap(dst_i, r * 256, [[4, 64], [1, 4]]),
                _ap(usrc[:], r * T + 4 * joff, [[8, 64], [1, 4]]),
                Kt[:, r * 16 + kcol:r * 16 + kcol + 1])
    for i, (us, kc_) in enumerate(((u0, 0), (u1, 1), (u0, 2), (u1, 3))):
        m_plane(ME[i], us, kc_, 0)
        m_plane(MO[i], us, kc_, 1)

    # ---------------- tree level 1: U2 = Modd @ Meven ----------------
    # U2_cp[B',m] = Mo_c0*Me_0p + Mo_c1*Me_1p, elementwise over (B', m);
    # planes are contiguous so everything is packed (2x fp16).
    es_tree = ExitStack()
    tr = es_tree.enter_context(tc.tile_pool(name="tr", bufs=1))
    U2 = tr.tile([128, 4 * H], FP16, tag="U2")
    g1 = tr.tile([128, 4 * H], FP16, tag="g1")
    g2 = tr.tile([128, 4 * H], FP16, tag="g2")
    for c in range(2):
        # dims (p, B'm): B-side Mo_c0 bcast over p; A-side Me_0p planes
        v.tensor_tensor(_ap(g1[:], 2 * c * H, [[H, 2], [1, H]]),
                        _ap(Mo[:], 2 * c * H, [[0, 2], [1, H]]),
                        _ap(Me[:], 0, [[H, 2], [1, H]]), op=Alu.mult)
        v.tensor_tensor(_ap(g2[:], 2 * c * H, [[H, 2], [1, H]]),
                        _ap(Mo[:], (2 * c + 1) * H, [[0, 2], [1, H]]),
                        _ap(Me[:], 2 * H, [[H, 2], [1, H]]), op=Alu.mult)
    v.tensor_tensor(U2[:], g1[:], g2[:], op=Alu.add)

    # prediction-side mask, chunked so it fills Pool gaps greedily
    Ym = keep.tile([128, W], mybir.dt.uint32, tag="Ym")
    for ch in range(6):
        gp.tensor_scalar(_ap(Ym[:], ch * (W // 6), [[1, W // 6]]),
                         _ap(Yt[:], ch * (W // 6), [[1, W // 6]]),
                         0.5, None, Alu.is_ge)

    # ---------------- tree level 2: U4 ----------------
    # U2 pair-evens at slots {0,1} (contig), odds at {2,3}.
    # U4_cp[B', n] = U2o_c0[B',n]*U2e_0p[B',n] + U2o_c1[B',n]*U2e_1p[B',n]
    # U2 planes: pos(i, B', m) = i*H + B'*4 + m ; even-read: m in {0,1}:
    # [[4,NBT],[1,2]]; odd-read: +2.
    U4 = tr.tile([128, 4 * 2 * NBT], FP16, tag="U4")   # planes cp x (B',n)
    t1 = tr.tile([128, 4 * 2 * NBT], FP16, tag="t1")
    t2 = tr.tile([128, 4 * 2 * NBT], FP16, tag="t2")
    # per c (ISA max 3 free dims), iterate (p, B', n):
    # B-side: U2odd_c{k} at plane (2c+k), slots {2,3}: pos = (2c+k)*H+B'*4+2+n
    # A-side: U2even_{k}p at plane (2k+p), slots {0,1}
    # out t: pos = (2c+p)*2*NBT + B'*2 + n
    for c in range(2):
        dims_out = [[2 * NBT, 2], [2, NBT], [1, 2]]
        v.tensor_tensor(
            _ap(t1[:], c * 2 * 2 * NBT, dims_out),
            _ap(U2[:], 2 * c * H + 2, [[0, 2], [4, NBT], [1, 2]]),
            _ap(U2[:], 0, [[H, 2], [4, NBT], [1, 2]]),
            op=Alu.mult)
        v.tensor_tensor(
            _ap(t2[:], c * 2 * 2 * NBT, dims_out),
            _ap(U2[:], (2 * c + 1) * H + 2, [[0, 2], [4, NBT], [1, 2]]),
            _ap(U2[:], 2 * H, [[H, 2], [4, NBT], [1, 2]]),
            op=Alu.mult)
    v.tensor_tensor(U4[:], t1[:], t2[:], op=Alu.add)

    # ---------------- tree level 3: U8 ----------------
    # U4 planes (B', n) interleaved; strided n-reads (1x), packed add.
    U8 = tr.tile([128, 4 * NBT], FP16, tag="U8")       # planes cp x B'
    t3 = tr.tile([128, 4 * NBT], FP16, tag="t3")
    t4 = tr.tile([128, 4 * NBT], FP16, tag="t4")
    od = [[2 * NBT, 2], [NBT, 2], [1, NBT]]
    v.tensor_tensor(_ap(t3[:], 0, od),
                    _ap(U4[:], 1, [[2 * 2 * NBT, 2], [0, 2], [2, NBT]]),
                    _ap(U4[:], 0, [[0, 2], [2 * NBT, 2], [2, NBT]]),
                    op=Alu.mult)
    v.tensor_tensor(_ap(t4[:], 0, od),
                    _ap(U4[:], 2 * NBT + 1,
                        [[2 * 2 * NBT, 2], [0, 2], [2, NBT]]),
                    _ap(U4[:], 4 * NBT, [[0, 2], [2 * NBT, 2], [2, NBT]]),
                    op=Alu.mult)
    v.tensor_tensor(U8[:], t3[:], t4[:], op=Alu.add)

    # normalize U8 (sum of 4 entries -> 1) to keep radix-8 chains in range
    zn = tr.tile([128, NBT], FP16, tag="zn")
    rz = tr.tile([128, NBT], FP16, tag="rz")
    zn2 = tr.tile([128, NBT], FP16, tag="zn2")
    v.tensor_tensor(_ap(zn[:], 0, [[1, NBT]]),
                    _ap(U8[:], 0, [[1, NBT]]),
                    _ap(U8[:], NBT, [[1, NBT]]), op=Alu.add)
    v.tensor_tensor(_ap(zn2[:], 0, [[1, NBT]]),
                    _ap(U8[:], 2 * NBT, [[1, NBT]]),
                    _ap(U8[:], 3 * NBT, [[1, NBT]]), op=Alu.add)
    v.tensor_tensor(_ap(zn[:], 0, [[1, NBT]]),
                    _ap(zn[:], 0, [[1, NBT]]),
                    _ap(zn2[:], 0, [[1, NBT]]), op=Alu.add)
    v.reciprocal(rz[:], zn[:])
    v.tensor_tensor(_ap(U8[:], 0, [[NBT, 4], [1, NBT]]),
                    _ap(U8[:], 0, [[NBT, 4], [1, NBT]]),
                    _ap(rz[:], 0, [[0, 4], [1, NBT]]), op=Alu.mult)

    # ---------------- radix-8 block scan ----------------
    # blocks b in [0,64) per tile; groups g of 8 blocks (8 groups/tile).
    # Step A: in-group inclusive matrix prefixes P[g, j] (HS shifts 1,2,4).
    # P stored planar like U8: planes cp x (B' = tile*64 + 8g + j).
    es_blk = ExitStack()
    bs = es_blk.enter_context(tc.tile_pool(name="bs", bufs=1))
    P = U8
    for h in (1, 2, 4):
        Pn = bs.tile([128, 4 * NBT], FP16, tag=f"P{h}")
        s1 = bs.tile([128, 4 * NBT], FP16, tag=f"s1_{h}")
        s2 = bs.tile([128, 4 * NBT], FP16, tag=f"s2_{h}")
        n = 8 - h
        # C[i] = P[i] * P[i-h] for i in [h,8) within each group
        # per c: dims (p, g, i); B-side P_c{k}[i] at plane (2c+k)
        go = [[NBT, 2], [8, NBT // 8], [1, n]]
        for c in range(2):
            v.tensor_tensor(
                _ap(s1[:], c * 2 * NBT + h, go),
                _ap(P[:], 2 * c * NBT + h, [[0, 2], [8, NBT // 8], [1, n]]),
                _ap(P[:], 0, [[NBT, 2], [8, NBT // 8], [1, n]]),
                op=Alu.mult)
            v.tensor_tensor(
                _ap(s2[:], c * 2 * NBT + h, go),
                _ap(P[:], (2 * c + 1) * NBT + h,
                    [[0, 2], [8, NBT // 8], [1, n]]),
                _ap(P[:], 2 * NBT, [[NBT, 2], [8, NBT // 8], [1, n]]),
                op=Alu.mult)
        v.tensor_tensor(_ap(Pn[:], h, [[NBT, 4], [8, NBT // 8], [1, n]]),
                        _ap(s1[:], h, [[NBT, 4], [8, NBT // 8], [1, n]]),
                        _ap(s2[:], h, [[NBT, 4], [8, NBT // 8], [1, n]]),
                        op=Alu.add)
        # heads [0,h) copy through (DVE: keeps the chain on one queue --
        # an Act round-trip here costs ~2 sem hops + 185ns SBUF latency)
        v.tensor_copy(out=_ap(Pn[:], 0, [[NBT, 4], [8, NBT // 8], [1, h]]),
                      in_=_ap(P[:], 0, [[NBT, 4], [8, NBT // 8], [1, h]]))
        P = Pn

    # Step B: group totals Tg = P[g,7]; normalize; tiny inclusive scan
    # over the 8 groups per tile (HS 1,2,4); then vg = Escan[g-1] @ alpha1.
    # Tg planar: planes cp x (tile r, g): width 4 * 24.
    NG = RT * 8
    Tg = bs.tile([128, 4 * NG], FP16, tag="Tg")
    v.tensor_copy(out=_ap(Tg[:], 0, [[NG, 4], [1, NG]]),
                  in_=_ap(P[:], 7, [[NBT, 4], [8, NG]]))
    # normalize Tg
    zg = bs.tile([128, NG], FP16, tag="zg")
    rg = bs.tile([128, NG], FP16, tag="rg")
    zg2 = bs.tile([128, NG], FP16, tag="zg2")
    v.tensor_tensor(zg[:], _ap(Tg[:], 0, [[1, NG]]),
                    _ap(Tg[:], NG, [[1, NG]]), op=Alu.add)
    v.tensor_tensor(zg2[:], _ap(Tg[:], 2 * NG, [[1, NG]]),
                    _ap(Tg[:], 3 * NG, [[1, NG]]), op=Alu.add)
    v.tensor_tensor(zg[:], zg[:], zg2[:], op=Alu.add)
    v.reciprocal(rg[:], zg[:])
    v.tensor_tensor(_ap(Tg[:], 0, [[NG, 4], [1, NG]]),
                    _ap(Tg[:], 0, [[NG, 4], [1, NG]]),
                    _ap(rg[:], 0, [[0, 4], [1, NG]]), op=Alu.mult)
    E = Tg
    for h in (1, 2, 4):
        En = bs.tile([128, 4 * NG], FP16, tag=f"E{h}")
        e1 = bs.tile([128, 4 * NG], FP16, tag=f"e1_{h}")
        e2 = bs.tile([128, 4 * NG], FP16, tag=f"e2_{h}")
        n = 8 - h
        go = [[NG, 2], [8, RT], [1, n]]
        for c in range(2):
            v.tensor_tensor(
                _ap(e1[:], c * 2 * NG + h, go),
                _ap(E[:], 2 * c * NG + h, [[0, 2], [8, RT], [1, n]]),
                _ap(E[:], 0, [[NG, 2], [8, RT], [1, n]]),
                op=Alu.mult)
            v.tensor_tensor(
                _ap(e2[:], c * 2 * NG + h, go),
                _ap(E[:], (2 * c + 1) * NG + h, [[0, 2], [8, RT], [1, n]]),
                _ap(E[:], 2 * NG, [[NG, 2], [8, RT], [1, n]]),
                op=Alu.mult)
        v.tensor_tensor(_ap(En[:], h, [[NG, 4], [8, RT], [1, n]]),
                        _ap(e1[:], h, [[NG, 4], [8, RT], [1, n]]),
                        _ap(e2[:], h, [[NG, 4], [8, RT], [1, n]]),
                        op=Alu.add)
        v.tensor_copy(out=_ap(En[:], 0, [[NG, 4], [8, RT], [1, h]]),
                      in_=_ap(E[:], 0, [[NG, 4], [8, RT], [1, h]]))
        E = En

    # vg[g] = E[g-1] @ alpha1 for g>=1 ; vg[0] = alpha1.  alpha1 per-tile
    # scalars K cols 12,13.  v-planes: vg0/vg1 width NG.
    vg = bs.tile([128, 2 * NG], FP16, tag="vg")
    vt = bs.tile([128, 2 * NG], FP16, tag="vt")
    for r in range(RT):
        a0 = Kt[:, r * 16 + 12:r * 16 + 13]
        a1 = Kt[:, r * 16 + 13:r * 16 + 14]
        for comp in range(2):
            # vg[comp][r, g] = E_{comp,0}[g-1]*a0 + E_{comp,1}[g-1]*a1
            seg7 = [[1, 7]]
            v.tensor_scalar_mul(
                _ap(vt[:], comp * NG + r * 8 + 1, seg7),
                _ap(E[:], (2 * comp + 1) * NG + r * 8, seg7), a1)
            v.scalar_tensor_tensor(
                _ap(vg[:], comp * NG + r * 8 + 1, seg7),
                _ap(E[:], (2 * comp) * NG + r * 8, seg7), a0,
                _ap(vt[:], comp * NG + r * 8 + 1, seg7), Alu.mult, Alu.add)
        v.tensor_copy(out=_ap(vg[:], r * 8, [[NG, 2], [1, 1]]),
                      in_=_ap(Kt[:], r * 16 + 12, [[1, 2], [0, 1]]))
    # normalize vg
    zv = bs.tile([128, NG], FP16, tag="zv")
    rv = bs.tile([128, NG], FP16, tag="rv")
    v.tensor_tensor(zv[:], _ap(vg[:], 0, [[1, NG]]),
                    _ap(vg[:], NG, [[1, NG]]), op=Alu.add)
    v.reciprocal(rv[:], zv[:])
    v.tensor_tensor(_ap(vg[:], 0, [[NG, 2], [1, NG]]),
                    _ap(vg[:], 0, [[NG, 2], [1, NG]]),
                    _ap(rv[:], 0, [[0, 2], [1, NG]]), op=Alu.mult)

    # Step C: w_b for all blocks.  w[8g] = vg[g]; w[8g+j] = P[g,j-1] @ vg[g].
    # w planes: w0/w1 width NBT (B'-indexed).
    wb = tr.tile([128, 2 * NBT], FP16, tag="wb")
    wt1 = bs.tile([128, 2 * NBT], FP16, tag="wt1")
    wt2 = bs.tile([128, 2 * NBT], FP16, tag="wt2")
    # dims (comp, g, j in 1..7): w_c = P_c0[g,j-1]*vg_0[g] + P_c1[g,j-1]*vg_1[g]
    wo = [[NBT, 2], [8, NBT // 8], [1, 7]]
    v.tensor_tensor(
        _ap(wt1[:], 1, wo),
        _ap(P[:], 0, [[2 * NBT, 2], [8, NBT // 8], [1, 7]]),
        _ap(vg[:], 0, [[0, 2], [1, NBT // 8], [0, 7]]),
        op=Alu.mult)
    v.tensor_tensor(
        _ap(wt2[:], 1, wo),
        _ap(P[:], NBT, [[2 * NBT, 2], [8, NBT // 8], [1, 7]]),
        _ap(vg[:], NG, [[0, 2], [1, NBT // 8], [0, 7]]),
        op=Alu.mult)
    v.tensor_tensor(_ap(wb[:], 1, wo), _ap(wt1[:], 1, wo),
                    _ap(wt2[:], 1, wo), op=Alu.add)
    v.tensor_copy(out=_ap(wb[:], 0, [[NBT, 2], [8, NBT // 8], [1, 1]]),
                  in_=_ap(vg[:], 0, [[NG, 2], [1, NBT // 8], [0, 1]]))
    # normalize w
    zw = bs.tile([128, NBT], FP16, tag="zw")
    rw = bs.tile([128, NBT], FP16, tag="rw")
    v.tensor_tensor(zw[:], _ap(wb[:], 0, [[1, NBT]]),
                    _ap(wb[:], NBT, [[1, NBT]]), op=Alu.add)
    v.reciprocal(rw[:], zw[:])
    v.tensor_tensor(_ap(wb[:], 0, [[NBT, 2], [1, NBT]]),
                    _ap(wb[:], 0, [[NBT, 2], [1, NBT]]),
                    _ap(rw[:], 0, [[0, 2], [1, NBT]]), op=Alu.mult)
    es_blk.close()

    # ---------------- within-block down-sweep ----------------
    # Alpha planes AL0/AL1, width W, slot layout (B', s: 8),
    # s = bitrev3(j): even slots 0..3 hold j = 0,4,2,6; odd 4..7: 1,5,3,7.
    es_al = ExitStack()
    alp = es_al.enter_context(tc.tile_pool(name="alp", bufs=1))
    AL = keep.tile([128, 2 * W], FP16, tag="AL")  # AL0 | AL1
    a1t = alp.tile([128, 2 * NBT], FP16, tag="a1t")
    a2t = alp.tile([128, 2 * NBT], FP16, tag="a2t")
    # slot 0 (j=0) = w
    sc.copy(_ap(AL[:], 0, [[W, 2], [8, NBT], [1, 1]]),
            _ap(wb[:], 0, [[NBT, 2], [1, NBT], [0, 1]]))
    # stage 1: slot 1 (j=4) = U4[node0] @ w ; U4 node0 = strided n=0 reads
    v.tensor_tensor(
        _ap(a1t[:], 0, [[NBT, 2], [1, NBT]]),
        _ap(U4[:], 0, [[2 * 2 * NBT, 2], [2, NBT]]),
        _ap(wb[:], 0, [[0, 2], [1, NBT]]), op=Alu.mult)
    v.tensor_tensor(
        _ap(a2t[:], 0, [[NBT, 2], [1, NBT]]),
        _ap(U4[:], 2 * NBT, [[2 * 2 * NBT, 2], [2, NBT]]),
        _ap(wb[:], NBT, [[0, 2], [1, NBT]]), op=Alu.mult)
    v.tensor_tensor(_ap(AL[:], 1, [[W, 2], [8, NBT]]),
                    _ap(a1t[:], 0, [[NBT, 2], [1, NBT]]),
                    _ap(a2t[:], 0, [[NBT, 2], [1, NBT]]), op=Alu.add)
    # stage 2: slots 2,3 (j=2,6) = U2[pair-even p1] @ AL[slots 0,1]
    # U2 even-pair slots {0,1}: pos = i*H + B'*4 + m, m in {0,1}
    b1 = alp.tile([128, 2 * 2 * NBT], FP16, tag="b1")
    b2 = alp.tile([128, 2 * 2 * NBT], FP16, tag="b2")
    s2o = [[2 * NBT, 2], [2, NBT], [1, 2]]
    v.tensor_tensor(
        _ap(b1[:], 0, s2o),
        _ap(U2[:], 0, [[2 * H, 2], [4, NBT], [1, 2]]),
        _ap(AL[:], 0, [[0, 2], [8, NBT], [1, 2]]), op=Alu.mult)
    v.tensor_tensor(
        _ap(b2[:], 0, s2o),
        _ap(U2[:], H, [[2 * H, 2], [4, NBT], [1, 2]]),
        _ap(AL[:], W, [[0, 2], [8, NBT], [1, 2]]), op=Alu.mult)
    v.tensor_tensor(_ap(AL[:], 2, [[W, 2], [8, NBT], [1, 2]]),
                    _ap(b1[:], 0, s2o), _ap(b2[:], 0, s2o), op=Alu.add)
    # stage 3: odd slots 4..7 (j=1,5,3,7) = M_even @ AL[even slots]
    c1 = alp.tile([128, 2 * W // 2], FP16, tag="c1")
    c2 = alp.tile([128, 2 * W // 2], FP16, tag="c2")
    # per row-tile so tile-0 predictions can start before tiles 1-2 finish
    for r in range(RT):
        ob4 = r * 256
        oa = r * T
        s3o = [[H, 2], [4, 64], [1, 4]]
        v.tensor_tensor(
            _ap(c1[:], ob4, s3o),
            _ap(Me[:], ob4, [[2 * H, 2], [4, 64], [1, 4]]),
            _ap(AL[:], oa, [[0, 2], [8, 64], [1, 4]]), op=Alu.mult)
        v.tensor_tensor(
            _ap(c2[:], ob4, s3o),
            _ap(Me[:], H + ob4, [[2 * H, 2], [4, 64], [1, 4]]),
            _ap(AL[:], W + oa, [[0, 2], [8, 64], [1, 4]]), op=Alu.mult)
        v.tensor_tensor(_ap(AL[:], 4 + oa, [[W, 2], [8, 64], [1, 4]]),
                        _ap(c1[:], ob4, s3o), _ap(c2[:], ob4, s3o),
                        op=Alu.add)

    # -------- predictions + lp + cumsum + q, pipelined per row-tile --------
    # per tile r: DVE (Za, Ra, rr, q1) -> Act (q0, ln-unpermute) -> DVE
    # (mask, copy-pred, scan, q-adds) -> relayout DMAs; tiles overlap engines.
    es_pr = ExitStack()
    pr = es_pr.enter_context(tc.tile_pool(name="pr", bufs=1))
    Za = pr.tile([128, W], FP16, tag="Za")
    Ra = pr.tile([128, W], FP16, tag="Ra")
    rr = pr.tile([128, W], FP16, tag="rr")
    q1 = pr.tile([128, W], FP16, tag="q1")
    p1 = keep.tile([128, W], F32, tag="p1")
    p0 = keep.tile([128, W], F32, tag="p0")
    lp = keep.tile([128, W], F32, tag="lp")
    # apc2 has one zero column before each tile's T cumsum columns so the
    # q = p + apc[t-1] add runs full-T with no single-element edge copies.
    apc2 = keep.tile([128, W + RT], F32, tag="apc2")
    q1c = keep.tile([128, W], F32, tag="q1c")
    q0c = keep.tile([128, W], F32, tag="q0c")
    # ability planes split in two tiles so the a0-3 max tree isn't blocked
    # on ability 4's late relayout; partitions [0:64) hold k=1, [64:128) k=0
    # (k=1 lower so tile2's identity move is the later-computed q1).
    QA03 = keep.tile([128, 4 * T], F32, tag="QA03")
    QA4 = keep.tile([128, T], F32, tag="QA4")
    psq = ctx.enter_context(tc.tile_pool(name="psq", bufs=1, space="PSUM"))
    gp.memset(_ap(apc2[:], 0, [[T + 1, RT]]), 0.0)
    slp = [[1, T]]

    def pred_front(r):
        o = r * T
        v.tensor_tensor(_ap(Za[:], o, slp), _ap(AL[:], o, slp),
                        _ap(AL[:], W + o, slp), op=Alu.add)
        v.reciprocal(_ap(Ra[:], o, slp), _ap(Za[:], o, slp))
        v.tensor_tensor(_ap(rr[:], o, slp), _ap(AL[:], W + o, slp),
                        _ap(Ra[:], o, slp), op=Alu.mult)
        v.tensor_tensor(_ap(q1[:], o, slp), _ap(rr[:], o, slp),
                        _ap(dm[:], o, slp), op=Alu.mult)
        v.tensor_tensor(_ap(q1[:], o, slp), _ap(q1[:], o, slp),
                        _ap(pgs[:], o, slp), op=Alu.add)
        # ln with unpermute slot->natural (split by j0); p0 = ln(1 - q1)
        # fuses the complement into the activation's scale/bias
        for j0 in range(2):
            sc.activation(
                _ap(p1[:], o + j0, [[8, 64], [2, 2], [4, 2]]),
                _ap(q1[:], o + 4 * j0, [[8, 64], [2, 2], [1, 2]]),
                Act.Ln)
            sc.activation(
                _ap(p0[:], o + j0, [[8, 64], [2, 2], [4, 2]]),
                _ap(q1[:], o + 4 * j0, [[8, 64], [2, 2], [1, 2]]),
                Act.Ln, bias=1.0, scale=-1.0)

    def pred_scan(r):
        o = r * T
        sc.copy(_ap(lp[:], o, slp), _ap(p0[:], o, slp))
        v.copy_predicated(_ap(lp[:], o, slp), _ap(Ym[:], o, slp),
                          _ap(p1[:], o, slp))
        v.tensor_tensor_scan(_ap(apc2[:], r * (T + 1) + 1, slp),
                             _ap(lp[:], o, slp),
                             _ap(lp[:], o, slp),
                             0.0, Alu.add, Alu.bypass)

    def _qa_dst(k, a):
        base, off = (QA4, 0) if a == 4 else (QA03, a * T)
        return _ap_p(base[:], 64 * (1 - k), 64, off, [[1, T]])

    def pred_q(r):
        # q_k = p_k + apc[t-1] (k=0 on DVE, k=1 on Pool for tiles 0/1),
        # then relayout into QA: moves with matching partition ranges
        # (half == 1-k) are on-chip copies (deferred so they don't block
        # the next tile's critical ops); cross moves are HWDGE DMAs --
        # keeping them off Pool's SWDGE avoids descriptor-gen queueing
        # behind Pool's q1 adds.
        o = r * T
        na = 2 if r < 2 else 1   # tile 2 holds only ability 4 (rows 0-63)
        for k, qsrc, psrc in ((0, q0c, p0), (1, q1c, p1)):
            qeng = gp if (k == 1 and r < 2) else v
            qeng.tensor_tensor(_ap(qsrc[:], o, slp),
                               _ap(psrc[:], o, slp),
                               _ap(apc2[:], r * (T + 1), slp), op=Alu.add)
            for half in range(na):
                if half == 1 - k:
                    continue
                sy.dma_start(_qa_dst(k, 2 * r + half),
                             _ap_p(qsrc[:], 64 * half, 64, o, [[1, T]]))

    for r in range(2):
        pred_front(r)
        pred_scan(r)
        pred_q(r)
    pred_front(2)
    pred_scan(2)
    # k=0 identity copies for tiles 0/1, emitted here so they sit in the
    # Act queue ahead of tile 2's q consumers but after its Ln/lp ops
    for r in range(2):
        sc.copy(_qa_dst(0, 2 * r + 1),
                _ap_p(q0c[:], 64, 64, r * T, [[1, T]]))
    pred_q(2)
    # deferred identity copies: k=1 planes for tiles 0/1 on Pool (after
    # both q1 adds), and tile 2's late a4 k=1 move on Act
    for r in range(2):
        gp.tensor_copy(out=_qa_dst(1, 2 * r),
                       in_=_ap_p(q1c[:], 0, 64, r * T, [[1, T]]))
    sc.copy(_qa_dst(1, 4), _ap_p(q1c[:], 0, 64, 2 * T, [[1, T]]))
    es_pr.close()
    es_al.close()
    es_tree.close()
    es_in.close()

    # ---------------- collapse over abilities ----------------
    col2 = ctx.enter_context(tc.tile_pool(name="col2", bufs=1))

    MX = col2.tile([128, T], F32, tag="MX")
    DF = col2.tile([128, A_LEV * T], FP16, tag="DF")
    EX = col2.tile([128, A_LEV * T], FP16, tag="EX")
    SM = col2.tile([128, T], F32, tag="SM")
    un = col2.tile([128, T], F32, tag="un")
    mt = col2.tile([128, 2 * T], F32, tag="mt")
    mth = col2.tile([128, 2 * T], FP16, tag="mth")
    psp = ctx.enter_context(tc.tile_pool(name="psp", bufs=1, space="PSUM"))
    un1s = psp.tile([64, T], F32, tag="un1s")
    dl = col2.tile([64, T], F32, tag="dl")
    ed = col2.tile([64, T], F32, tag="ed")
    sp = col2.tile([64, T], F32, tag="sp")
    OI = col2.tile([64, 2 * T], F32, tag="OI")
    # t-chunked 3-engine pipeline over the collapse.  The a0-3 max tree
    # reads only QA03, so it runs while ability 4's relayout is in flight;
    # only MX/DF wait for QA4.  Partitions [0:64) hold k=1, [64:128) k=0,
    # so dl = un1 - un0 and out0 = -softplus(dl), out1 = dl - softplus(dl).
    NCH = 4
    HT = T // NCH

    def cmax(t0):
        hl = [[1, HT]]
        v.tensor_tensor(_ap(mt[:], t0, [[T, 2], [1, HT]]),
                        _ap(QA03[:], t0, [[T, 2], [1, HT]]),
                        _ap(QA03[:], 2 * T + t0, [[T, 2], [1, HT]]),
                        op=Alu.max)
        v.tensor_tensor(_ap(mt[:], t0, hl), _ap(mt[:], t0, hl),
                        _ap(mt[:], T + t0, hl), op=Alu.max)

    def cdf(t0):
        hl = [[1, HT]]
        v.tensor_tensor(_ap(MX[:], t0, hl), _ap(mt[:], t0, hl),
                        _ap(QA4[:], t0, hl), op=Alu.max)
        v.tensor_tensor(_ap(DF[:], t0, [[T, 4], [1, HT]]),
                        _ap(QA03[:], t0, [[T, 4], [1, HT]]),
                        _ap(MX[:], t0, [[0, 4], [1, HT]]),
                        op=Alu.subtract)
        v.tensor_tensor(_ap(DF[:], 4 * T + t0, hl),
                        _ap(QA4[:], t0, hl),
                        _ap(MX[:], t0, hl), op=Alu.subtract)
        sc.activation(_ap(EX[:], t0, [[T, A_LEV], [1, HT]]),
                      _ap(DF[:], t0, [[T, A_LEV], [1, HT]]), Act.Exp)

    def csum(t0):
        hl = [[1, HT]]
        gp.tensor_tensor(_ap(mth[:], t0, [[T, 2], [1, HT]]),
                         _ap(EX[:], t0, [[T, 2], [1, HT]]),
                         _ap(EX[:], 2 * T + t0, [[T, 2], [1, HT]]),
                         op=Alu.add)
        gp.tensor_tensor(_ap(mth[:], t0, hl), _ap(mth[:], t0, hl),
                         _ap(mth[:], T + t0, hl), op=Alu.add)
        v.tensor_tensor(_ap(SM[:], t0, hl), _ap(mth[:], t0, hl),
                        _ap(EX[:], 4 * T + t0, hl), op=Alu.add)
        sc.activation(_ap(SM[:], t0, hl), _ap(SM[:], t0, hl), Act.Ln)
        v.tensor_tensor(_ap(un[:], t0, hl), _ap(MX[:], t0, hl),
                        _ap(SM[:], t0, hl), op=Alu.add)
        # partition shift via idle PE: un1s[j] = un[64+j] (k=0 half down)
        nc.tensor.matmul(_ap_p(un1s[:], 0, 64, t0, hl),
                         SHt[:, 0:64], _ap_p(un[:], 0, 128, t0, hl),
                         start=True, stop=True)

    def ctail(t0, HL, oq):
        hl = [[1, HL]]
        v.tensor_tensor(_ap_p(dl[:], 0, 64, t0, hl),
                        _ap_p(un[:], 0, 64, t0, hl),
                        _ap_p(un1s[:], 0, 64, t0, hl), op=Alu.subtract)
        sc.activation(_ap_p(ed[:], 0, 64, t0, hl),
                      _ap_p(dl[:], 0, 64, t0, hl), Act.Exp)
        sc.activation(_ap_p(sp[:], 0, 64, t0, hl),
                      _ap_p(ed[:], 0, 64, t0, hl), Act.Ln, bias=1.0)
        gp.tensor_scalar(_ap_p(OI[:], 0, 64, 2 * t0, [[2, HL]]),
                         _ap_p(sp[:], 0, 64, t0, hl),
                         -1.0, None, Alu.mult)
        v.tensor_tensor(_ap_p(OI[:], 0, 64, 2 * t0 + 1, [[2, HL]]),
                        _ap_p(dl[:], 0, 64, t0, hl),
                        _ap_p(sp[:], 0, 64, t0, hl), op=Alu.subtract)
        oq.dma_start(bass.AP(O[:].tensor, 2 * t0, [[2 * T, 64], [1, 2 * HL]]),
                     _ap_p(OI[:], 0, 64, 2 * t0, [[1, 2 * HL]]))

    for c in range(NCH):
        cmax(c * HT)
    for c in range(NCH):
        cdf(c * HT)
    for c in range(NCH):
        csum(c * HT)
    for c in range(NCH):
        ctail(c * HT, HT, sy)


def _steer_act_tables(arch):
    """Keep Exp/Ln claimed by one table set (see kernel v1)."""
    from concourse import hw_specs
    tabs = hw_specs.get_activation_tables(arch)
    for name, funcs in tabs.items():
        if name == "natural_log_exp_and_others":
            continue
        funcs.discard(Act.Exp)
        funcs.discard(Act.Ln)


def _build_program():
    nc = bacc.Bacc()
    _steer_act_tables(nc.m.arch)
    U0 = nc.declare_dram_parameter("U0", [RT * 128, T], FP16, isOutput=False)
    U1 = nc.declare_dram_parameter("U1", [RT * 128, T], FP16, isOutput=False)
    PG = nc.declare_dram_parameter("PG", [RT * 128, T], FP16, isOutput=False)
    DM = nc.declare_dram_parameter("DM", [RT * 128, T], FP16, isOutput=False)
    Y = nc.declare_dram_parameter("Y", [RT * 128, T], FP16, isOutput=False)
    K = nc.declare_dram_parameter("K", [RT * 128, 16], F32, isOutput=False)
    SH = nc.declare_dram_parameter("SH", [128, 128], F32, isOutput=False)
    O = nc.declare_dram_parameter("O", [BL, T, 2], F32, isOutput=True)
    with ExitStack() as ctx:
        tc = ctx.enter_context(tile.TileContext(nc))
        with nc.allow_low_precision(reason="fp16 HMM chain; validated vs gate"):
            _emit(ctx, tc, nc, U0, U1, PG, DM, Y, K, SH, O)
    if not nc.is_finalized():
        nc.finalize()
    return nc


def _pad_rows(x, dtype=np.float32, fill=0.0):
    out = np.full((RT * 128, x.shape[1]), fill, dtype=dtype)
    out[:ROWS] = x
    return out


def kernel(corr, ytrue, problem, kc, dyn_emb, obs_logits_problem,
           obs_logits_kc, ability_levels, traj, trans_ind, pred_ind):
    global _last_results, _cached_nc
    import ml_dtypes
    fp16 = np.float16

    corr = np.asarray(corr, dtype=np.float32)
    ytrue = np.asarray(ytrue, dtype=np.float32)
    problem = np.asarray(problem)
    kc = np.asarray(kc)
    dyn_emb = np.asarray(dyn_emb, dtype=np.float32)
    obs_logits_problem = np.asarray(obs_logits_problem, dtype=np.float32)
    obs_logits_kc = np.asarray(obs_logits_kc, dtype=np.float32)
    ability = np.asarray(ability_levels, dtype=np.float32)

    obs_core = obs_logits_problem[problem] + obs_logits_kc[kc][:, None, :]
    dyn = dyn_emb[kc]
    sig = lambda x: 1.0 / (1.0 + np.exp(-x.astype(np.float64)))
    lL, lF, lI0 = dyn[:, 0], dyn[:, 1], dyn[:, 2]
    AT00, AT01 = sig(-lL), sig(lF)
    AT10, AT11 = sig(lL), sig(-lF)
    al = [AT00, AT01, AT10, AT11]
    alpha = [al[2 * (i // 2)] * al[i % 2] for i in range(4)]
    # alpha_cp = AT_c0*AT_0p: (c,p): c0 entry = AT[c][0] = al[2c], AT[0][p]=al[p]
    alpha = [al[2 * (i // 2)] * al[i % 2] for i in range(4)]
    beta = [al[2 * (i // 2) + 1] * al[2 + i % 2] for i in range(4)]
    Kfull = np.stack(al + alpha + beta +
                     [sig(-lI0), sig(lI0), np.zeros_like(lL),
                      np.zeros_like(lL)], axis=1).astype(np.float32)  # (B,16)

    # permute the T axis within each 8-block so storage slot s holds
    # natural step j = bitrev3(s); Y stays natural (cumsum order).
    perm = (np.arange(T) & ~7) + np.tile(
        np.array([0, 4, 2, 6, 1, 5, 3, 7]), T // 8)

    in_maps = []
    for c in range(N_CORES):
        sl = slice(c * BL, (c + 1) * BL)
        g = obs_core[sl, :, 0][None, :, perm] + ability[:, None, None]
        s = obs_core[sl, :, 1][None, :, perm] - ability[:, None, None]
        ct = np.broadcast_to(corr[sl][:, perm][None], (A_LEV, BL, T))
        yt = np.broadcast_to(ytrue[sl][None], (A_LEV, BL, T))
        # observation likelihood diagonals, normalized to sum 1 per step
        c2 = 2.0 * ct - 1.0
        u0r = sig(c2 * g)
        u1r = sig(-c2 * s)
        zu = u0r + u1r
        u0n = (u0r / zu).astype(np.float32)
        pg = sig(g)
        dmv = (sig(-s) - pg).astype(np.float32)
        kt = np.broadcast_to(Kfull[sl][None], (A_LEV, BL, 16))
        kpad = _pad_rows(kt.reshape(ROWS, 16), np.float32)
        kpad[ROWS:] = 0.5            # benign transition probs on padded rows
        shm = np.zeros((128, 128), np.float32)
        shm[np.arange(64) + 64, np.arange(64)] = 1.0
        shm[np.arange(64), np.arange(64) + 64] = 1.0
        # pad rows get benign 0.5 probabilities so no inf/nan ever forms
        # there (the PE half-swap matmuls contract over all partitions and
        # 0 * nan would poison valid lanes)
        in_maps.append({
            "U0": _pad_rows(u0n.reshape(ROWS, T), fp16, 0.5),
            "U1": _pad_rows((1.0 - u0n).reshape(ROWS, T), fp16, 0.5),
            "PG": _pad_rows(pg.reshape(ROWS, T).astype(np.float32), fp16, 0.5),
            "DM": _pad_rows(dmv.reshape(ROWS, T), fp16),
            "Y": _pad_rows(yt.reshape(ROWS, T), fp16),
            "K": kpad,
            "SH": shm,
        })

    if _cached_nc is None:
        _cached_nc = _build_program()

    res = run_bass_kernel_spmd(
        _cached_nc, in_maps, list(range(N_CORES)),
        trace=bool(os.environ.get("BASS_TRACE")),
    )
    _last_results = res
    out = np.concatenate([res.results[i]["O"] for i in range(N_CORES)], axis=0)
    return out.astype(np.float32)



# revision 80
# speedup vs baseline: 1.0236x; 1.0053x over previous
"""BKT model kernel v2 for Trainium2 (8 NeuronCores, Bass/Tile).

Exact 2-state HMM reformulation of the reference's 2^n-trajectory fastBKT
(see kernel v1 docstring).  v2 restructures for the DVE cost model:

- fp16 for the whole matrix chain (obs probs, level matrices, tree products,
  alphas, predictions).  The chain is contractive and sum-normalized, so
  fp16's 2^-11 rounding keeps the final error ~2e-3 << the 2e-2 gate;
  subnormal flushes only hit entries whose contribution is negligible.
- planar 2x2-entry planes (one buffer region per matrix entry) so
  tensor_tensor ops read/write packed last dims -> DVE 2x mode; per-partition
  transition constants ride tensor_scalar (2x/4x) and Act-engine scale APs.
- within-block (8-step) products use the A^T = gamma*I + 1 v^T structure at
  level 1, a "parity-split" pair layout for levels 2-3, and a 3-stage vector
  down-sweep for the per-step alphas.
- the 64-block scan is radix-8: in-group Hillis-Steele matrix prefixes,
  a tiny 8-group matrix scan, then one batched mat-vec to get per-block
  start alphas.
- Act engine absorbs sigmoids/copies/lns (including the bit-reversal
  unpermute via 4-free-dim APs); Pool absorbs reductions off the DVE path.
- log-predictions, cumsum and the ability-collapse stay f32.

Sharding: data-parallel over students (B=512 -> 64 per core); 5 ability
levels x 64 students = 320 rows padded to 3 x 128-partition tiles.
"""

import os
import numpy as np
from contextlib import ExitStack

import concourse.bass as bass
import concourse.bacc as bacc
import concourse.mybir as mybir
from concourse import tile
from concourse.bass_utils import run_bass_kernel_spmd

F32 = mybir.dt.float32
FP16 = mybir.dt.float16
Alu = mybir.AluOpType
Act = mybir.ActivationFunctionType
AX = mybir.AxisListType

N_CORES = 8
B_FULL = 512
T = 512
A_LEV = 5
BL = B_FULL // N_CORES          # students per core = 64
ROWS = A_LEV * BL               # valid rows per core = 320
RT = 3                          # row tiles of 128 (384 rows incl. pad)
NBT = RT * 64                   # blocks spanning tiles = 192
W = RT * T                      # full-plane free width = 1536
H = W // 2                      # half width = 768
ABILITY = np.array([-2.0, -1.0, 0.0, 1.0, 2.0], dtype=np.float32)

_last_results = None
_cached_nc = None


def _ap(base, off, dims):
    """Custom AP on the same tensor as `base`, keeping its partition dim."""
    return bass.AP(base.tensor, base.offset + off, [list(base.ap[0])] + dims)


def _ap_p(base, poff, pcount, off, dims):
    p = list(base.ap[0])
    pstride = p[0]
    return bass.AP(
        base.tensor, base.offset + poff * pstride + off, [[pstride, pcount]] + dims
    )


def _emit(ctx, tc, nc, U0, U1, PG, DM, Y, K, SH, O):
    v = nc.vector
    sc = nc.scalar
    gp = nc.gpsimd
    sy = nc.sync

    keep = ctx.enter_context(tc.tile_pool(name="keep", bufs=1))

    # ---------------- input DMAs ----------------
    # U0/U1 are the normalized per-step observation likelihood diagonals
    # (host-side sigmoids, slot-ordered); PG = P(y=1|unlearned) and
    # DM = P(y=1|learned) - PG feed the predictions.  K first (tiny, the
    # M planes need its scalars), then U0/U1 per row-tile on the HWDGE
    # queue; PG/DM/Y trail on Pool's SWDGE (needed only by the preds).
    es_in = ExitStack()
    io = es_in.enter_context(tc.tile_pool(name="io", bufs=1))
    u0 = io.tile([128, W], FP16, tag="U0")
    u1 = io.tile([128, W], FP16, tag="U1")
    pgs = keep.tile([128, W], FP16, tag="PG")
    dm = keep.tile([128, W], FP16, tag="DM")
    Yt = keep.tile([128, W], FP16, tag="Y")
    Kt = keep.tile([128, RT * 16], F32, tag="K")
    gp.dma_start(_ap(Kt[:], 0, [[16, RT], [1, 16]]),
                 bass.AP(K[:].tensor, 0, [[16, 128], [128 * 16, RT], [1, 16]]))
    for r in range(RT):
        for dram, sb in ((U0, u0), (U1, u1)):
            sy.dma_start(_ap(sb[:], r * T, [[1, T]]),
                         bass.AP(dram[:].tensor, r * 128 * T,
                                 [[T, 128], [1, T]]))
    for r in range(RT):
        for dram, sb in ((PG, pgs), (DM, dm)):
            gp.dma_start(_ap(sb[:], r * T, [[1, T]]),
                         bass.AP(dram[:].tensor, r * 128 * T,
                                 [[T, 128], [1, T]]))
    gp.dma_start(_ap(Yt[:], 0, [[T, RT], [1, T]]),
                 bass.AP(Y[:].tensor, 0, [[T, 128], [128 * T, RT], [1, T]]))
    # partition-half swap matrix SW[i, j] = 1 iff |i-j| == 64: PE matmuls
    # with it (or its left half) replace SBUF->SBUF partition-shift DMAs
    SHt = keep.tile([128, 128], F32, tag="SH")
    sy.dma_start(SHt[:], bass.AP(SH[:].tensor, 0, [[128, 128], [1, 128]]))

    def KC(col):
        """Per-partition scalar AP for K column `col` of row-tile r -- but all
        tiles share the op; K scalars differ per tile, so ops over multi-tile
        widths must pass per-tile slices.  Helper returns slice for tile r."""
        return Kt[:, col:col + 1]

    # K layout (16 cols per tile r at r*16):
    # 0..3 : A^T entries AT00, AT01, AT10, AT11
    # 4..7 : alpha_cp = AT_c0*AT_0p   (order 00,01,10,11)
    # 8..11: beta_cp  = AT_c1*AT_1p
    # 12,13: alpha1 init (s(-lI0), s(lI0))

    # ---------------- split u-halves ----------------
    # U0/U1 arrive from the host with the T axis permuted within each
    # 8-block: storage slot s holds natural step j = bitrev3(s), i.e. slot
    # order j = (0,4,2,6,1,5,3,7).  Slots 0..3 are exactly the even-j
    # "parity-split" order j_even(m) = 4*(m&1)+2*(m>>1) the M planes want,
    # slots 4..7 the odds.  M-plane reads are packed (stride-1 runs of 4)
    # -> DVE 4x, and each row-tile r starts as soon as its U DMAs land.
    Me = keep.tile([128, 4 * H], FP16, tag="Me")
    Mo = keep.tile([128, 4 * H], FP16, tag="Mo")
    ME = [Me[:, i * H:(i + 1) * H] for i in range(4)]
    MO = [Mo[:, i * H:(i + 1) * H] for i in range(4)]

    def m_plane(dst_i, usrc, kcol, joff):
        # dst pos = r*256 + b*4 + m  <-  src pos = r*512 + b*8 + 4*joff + m
        for r in range(RT):
            v.tensor_scalar_mul(
                _ap(dst_i, r * 256, [[4, 64], [1, 4]]),
                _ap(usrc[:], r * T + 4 * joff, [[8, 64], [1, 4]]),
                Kt[:, r * 16 + kcol:r * 16 + kcol + 1])
    for i, (us, kc_) in enumerate(((u0, 0), (u1, 1), (u0, 2), (u1, 3))):
        m_plane(ME[i], us, kc_, 0)
        m_plane(MO[i], us, kc_, 1)

    # ---------------- tree level 1: U2 = Modd @ Meven ----------------
    # U2_cp[B',m] = Mo_c0*Me_0p + Mo_c1*Me_1p, elementwise over (B', m);
    # planes are contiguous so everything is packed (2x fp16).
    es_tree = ExitStack()
    tr = es_tree.enter_context(tc.tile_pool(name="tr", bufs=1))
    U2 = tr.tile([128, 4 * H], FP16, tag="U2")
    g1 = tr.tile([128, 4 * H], FP16, tag="g1")
    g2 = tr.tile([128, 4 * H], FP16, tag="g2")
    for c in range(2):
        # dims (p, B'm): B-side Mo_c0 bcast over p; A-side Me_0p planes
        v.tensor_tensor(_ap(g1[:], 2 * c * H, [[H, 2], [1, H]]),
                        _ap(Mo[:], 2 * c * H, [[0, 2], [1, H]]),
                        _ap(Me[:], 0, [[H, 2], [1, H]]), op=Alu.mult)
        v.tensor_tensor(_ap(g2[:], 2 * c * H, [[H, 2], [1, H]]),
                        _ap(Mo[:], (2 * c + 1) * H, [[0, 2], [1, H]]),
                        _ap(Me[:], 2 * H, [[H, 2], [1, H]]), op=Alu.mult)
    v.tensor_tensor(U2[:], g1[:], g2[:], op=Alu.add)

    # prediction-side mask, chunked so it fills Pool gaps greedily
    Ym = keep.tile([128, W], mybir.dt.uint32, tag="Ym")
    for ch in range(6):
        gp.tensor_scalar(_ap(Ym[:], ch * (W // 6), [[1, W // 6]]),
                         _ap(Yt[:], ch * (W // 6), [[1, W // 6]]),
                         0.5, None, Alu.is_ge)

    # ---------------- tree level 2: U4 ----------------
    # U2 pair-evens at slots {0,1} (contig), odds at {2,3}.
    # U4_cp[B', n] = U2o_c0[B',n]*U2e_0p[B',n] + U2o_c1[B',n]*U2e_1p[B',n]
    # U2 planes: pos(i, B', m) = i*H + B'*4 + m ; even-read: m in {0,1}:
    # [[4,NBT],[1,2]]; odd-read: +2.
    U4 = tr.tile([128, 4 * 2 * NBT], FP16, tag="U4")   # planes cp x (B',n)
    t1 = tr.tile([128, 4 * 2 * NBT], FP16, tag="t1")
    t2 = tr.tile([128, 4 * 2 * NBT], FP16, tag="t2")
    # per c (ISA max 3 free dims), iterate (p, B', n):
    # B-side: U2odd_c{k} at plane (2c+k), slots {2,3}: pos = (2c+k)*H+B'*4+2+n
    # A-side: U2even_{k}p at plane (2k+p), slots {0,1}
    # out t: pos = (2c+p)*2*NBT + B'*2 + n
    for c in range(2):
        dims_out = [[2 * NBT, 2], [2, NBT], [1, 2]]
        v.tensor_tensor(
            _ap(t1[:], c * 2 * 2 * NBT, dims_out),
            _ap(U2[:], 2 * c * H + 2, [[0, 2], [4, NBT], [1, 2]]),
            _ap(U2[:], 0, [[H, 2], [4, NBT], [1, 2]]),
            op=Alu.mult)
        v.tensor_tensor(
            _ap(t2[:], c * 2 * 2 * NBT, dims_out),
            _ap(U2[:], (2 * c + 1) * H + 2, [[0, 2], [4, NBT], [1, 2]]),
            _ap(U2[:], 2 * H, [[H, 2], [4, NBT], [1, 2]]),
            op=Alu.mult)
    v.tensor_tensor(U4[:], t1[:], t2[:], op=Alu.add)

    # ---------------- tree level 3: U8 ----------------
    # U4 planes (B', n) interleaved; strided n-reads (1x), packed add.
    U8 = tr.tile([128, 4 * NBT], FP16, tag="U8")       # planes cp x B'
    t3 = tr.tile([128, 4 * NBT], FP16, tag="t3")
    t4 = tr.tile([128, 4 * NBT], FP16, tag="t4")
    od = [[2 * NBT, 2], [NBT, 2], [1, NBT]]
    v.tensor_tensor(_ap(t3[:], 0, od),
                    _ap(U4[:], 1, [[2 * 2 * NBT, 2], [0, 2], [2, NBT]]),
                    _ap(U4[:], 0, [[0, 2], [2 * NBT, 2], [2, NBT]]),
                    op=Alu.mult)
    v.tensor_tensor(_ap(t4[:], 0, od),
                    _ap(U4[:], 2 * NBT + 1,
                        [[2 * 2 * NBT, 2], [0, 2], [2, NBT]]),
                    _ap(U4[:], 4 * NBT, [[0, 2], [2 * NBT, 2], [2, NBT]]),
                    op=Alu.mult)
    v.tensor_tensor(U8[:], t3[:], t4[:], op=Alu.add)

    # normalize U8 (sum of 4 entries -> 1) to keep radix-8 chains in range
    zn = tr.tile([128, NBT], FP16, tag="zn")
    rz = tr.tile([128, NBT], FP16, tag="rz")
    zn2 = tr.tile([128, NBT], FP16, tag="zn2")
    v.tensor_tensor(_ap(zn[:], 0, [[1, NBT]]),
                    _ap(U8[:], 0, [[1, NBT]]),
                    _ap(U8[:], NBT, [[1, NBT]]), op=Alu.add)
    v.tensor_tensor(_ap(zn2[:], 0, [[1, NBT]]),
                    _ap(U8[:], 2 * NBT, [[1, NBT]]),
                    _ap(U8[:], 3 * NBT, [[1, NBT]]), op=Alu.add)
    v.tensor_tensor(_ap(zn[:], 0, [[1, NBT]]),
                    _ap(zn[:], 0, [[1, NBT]]),
                    _ap(zn2[:], 0, [[1, NBT]]), op=Alu.add)
    v.reciprocal(rz[:], zn[:])
    v.tensor_tensor(_ap(U8[:], 0, [[NBT, 4], [1, NBT]]),
                    _ap(U8[:], 0, [[NBT, 4], [1, NBT]]),
                    _ap(rz[:], 0, [[0, 4], [1, NBT]]), op=Alu.mult)

    # ---------------- radix-8 block scan ----------------
    # blocks b in [0,64) per tile; groups g of 8 blocks (8 groups/tile).
    # Step A: in-group inclusive matrix prefixes P[g, j] (HS shifts 1,2,4).
    # P stored planar like U8: planes cp x (B' = tile*64 + 8g + j).
    es_blk = ExitStack()
    bs = es_blk.enter_context(tc.tile_pool(name="bs", bufs=1))
    P = U8
    for h in (1, 2, 4):
        Pn = bs.tile([128, 4 * NBT], FP16, tag=f"P{h}")
        s1 = bs.tile([128, 4 * NBT], FP16, tag=f"s1_{h}")
        s2 = bs.tile([128, 4 * NBT], FP16, tag=f"s2_{h}")
        n = 8 - h
        # C[i] = P[i] * P[i-h] for i in [h,8) within each group
        # per c: dims (p, g, i); B-side P_c{k}[i] at plane (2c+k)
        go = [[NBT, 2], [8, NBT // 8], [1, n]]
        for c in range(2):
            v.tensor_tensor(
                _ap(s1[:], c * 2 * NBT + h, go),
                _ap(P[:], 2 * c * NBT + h, [[0, 2], [8, NBT // 8], [1, n]]),
                _ap(P[:], 0, [[NBT, 2], [8, NBT // 8], [1, n]]),
                op=Alu.mult)
            v.tensor_tensor(
                _ap(s2[:], c * 2 * NBT + h, go),
                _ap(P[:], (2 * c + 1) * NBT + h,
                    [[0, 2], [8, NBT // 8], [1, n]]),
                _ap(P[:], 2 * NBT, [[NBT, 2], [8, NBT // 8], [1, n]]),
                op=Alu.mult)
        v.tensor_tensor(_ap(Pn[:], h, [[NBT, 4], [8, NBT // 8], [1, n]]),
                        _ap(s1[:], h, [[NBT, 4], [8, NBT // 8], [1, n]]),
                        _ap(s2[:], h, [[NBT, 4], [8, NBT // 8], [1, n]]),
                        op=Alu.add)
        # heads [0,h) copy through (DVE: keeps the chain on one queue --
        # an Act round-trip here costs ~2 sem hops + 185ns SBUF latency)
        v.tensor_copy(out=_ap(Pn[:], 0, [[NBT, 4], [8, NBT // 8], [1, h]]),
                      in_=_ap(P[:], 0, [[NBT, 4], [8, NBT // 8], [1, h]]))
        P = Pn

    # Step B: group totals Tg = P[g,7]; normalize; tiny inclusive scan
    # over the 8 groups per tile (HS 1,2,4); then vg = Escan[g-1] @ alpha1.
    # Tg planar: planes cp x (tile r, g): width 4 * 24.
    NG = RT * 8
    Tg = bs.tile([128, 4 * NG], FP16, tag="Tg")
    v.tensor_copy(out=_ap(Tg[:], 0, [[NG, 4], [1, NG]]),
                  in_=_ap(P[:], 7, [[NBT, 4], [8, NG]]))
    # normalize Tg
    zg = bs.tile([128, NG], FP16, tag="zg")
    rg = bs.tile([128, NG], FP16, tag="rg")
    zg2 = bs.tile([128, NG], FP16, tag="zg2")
    v.tensor_tensor(zg[:], _ap(Tg[:], 0, [[1, NG]]),
                    _ap(Tg[:], NG, [[1, NG]]), op=Alu.add)
    v.tensor_tensor(zg2[:], _ap(Tg[:], 2 * NG, [[1, NG]]),
                    _ap(Tg[:], 3 * NG, [[1, NG]]), op=Alu.add)
    v.tensor_tensor(zg[:], zg[:], zg2[:], op=Alu.add)
    v.reciprocal(rg[:], zg[:])
    v.tensor_tensor(_ap(Tg[:], 0, [[NG, 4], [1, NG]]),
                    _ap(Tg[:], 0, [[NG, 4], [1, NG]]),
                    _ap(rg[:], 0, [[0, 4], [1, NG]]), op=Alu.mult)
    E = Tg
    for h in (1, 2, 4):
        En = bs.tile([128, 4 * NG], FP16, tag=f"E{h}")
        e1 = bs.tile([128, 4 * NG], FP16, tag=f"e1_{h}")
        e2 = bs.tile([128, 4 * NG], FP16, tag=f"e2_{h}")
        n = 8 - h
        go = [[NG, 2], [8, RT], [1, n]]
        for c in range(2):
            v.tensor_tensor(
                _ap(e1[:], c * 2 * NG + h, go),
                _ap(E[:], 2 * c * NG + h, [[0, 2], [8, RT], [1, n]]),
                _ap(E[:], 0, [[NG, 2], [8, RT], [1, n]]),
                op=Alu.mult)
            v.tensor_tensor(
                _ap(e2[:], c * 2 * NG + h, go),
                _ap(E[:], (2 * c + 1) * NG + h, [[0, 2], [8, RT], [1, n]]),
                _ap(E[:], 2 * NG, [[NG, 2], [8, RT], [1, n]]),
                op=Alu.mult)
        v.tensor_tensor(_ap(En[:], h, [[NG, 4], [8, RT], [1, n]]),
                        _ap(e1[:], h, [[NG, 4], [8, RT], [1, n]]),
                        _ap(e2[:], h, [[NG, 4], [8, RT], [1, n]]),
                        op=Alu.add)
        v.tensor_copy(out=_ap(En[:], 0, [[NG, 4], [8, RT], [1, h]]),
                      in_=_ap(E[:], 0, [[NG, 4], [8, RT], [1, h]]))
        E = En

    # vg[g] = E[g-1] @ alpha1 for g>=1 ; vg[0] = alpha1.  alpha1 per-tile
    # scalars K cols 12,13.  v-planes: vg0/vg1 width NG.
    vg = bs.tile([128, 2 * NG], FP16, tag="vg")
    vt = bs.tile([128, 2 * NG], FP16, tag="vt")
    for r in range(RT):
        a0 = Kt[:, r * 16 + 12:r * 16 + 13]
        a1 = Kt[:, r * 16 + 13:r * 16 + 14]
        for comp in range(2):
            # vg[comp][r, g] = E_{comp,0}[g-1]*a0 + E_{comp,1}[g-1]*a1
            seg7 = [[1, 7]]
            v.tensor_scalar_mul(
                _ap(vt[:], comp * NG + r * 8 + 1, seg7),
                _ap(E[:], (2 * comp + 1) * NG + r * 8, seg7), a1)
            v.scalar_tensor_tensor(
                _ap(vg[:], comp * NG + r * 8 + 1, seg7),
                _ap(E[:], (2 * comp) * NG + r * 8, seg7), a0,
                _ap(vt[:], comp * NG + r * 8 + 1, seg7), Alu.mult, Alu.add)
        v.tensor_copy(out=_ap(vg[:], r * 8, [[NG, 2], [1, 1]]),
                      in_=_ap(Kt[:], r * 16 + 12, [[1, 2], [0, 1]]))
    # normalize vg
    zv = bs.tile([128, NG], FP16, tag="zv")
    rv = bs.tile([128, NG], FP16, tag="rv")
    v.tensor_tensor(zv[:], _ap(vg[:], 0, [[1, NG]]),
                    _ap(vg[:], NG, [[1, NG]]), op=Alu.add)
    v.reciprocal(rv[:], zv[:])
    v.tensor_tensor(_ap(vg[:], 0, [[NG, 2], [1, NG]]),
                    _ap(vg[:], 0, [[NG, 2], [1, NG]]),
                    _ap(rv[:], 0, [[0, 2], [1, NG]]), op=Alu.mult)

    # Step C: w_b for all blocks.  w[8g] = vg[g]; w[8g+j] = P[g,j-1] @ vg[g].
    # w planes: w0/w1 width NBT (B'-indexed).
    wb = tr.tile([128, 2 * NBT], FP16, tag="wb")
    wt1 = bs.tile([128, 2 * NBT], FP16, tag="wt1")
    wt2 = bs.tile([128, 2 * NBT], FP16, tag="wt2")
    # dims (comp, g, j in 1..7): w_c = P_c0[g,j-1]*vg_0[g] + P_c1[g,j-1]*vg_1[g]
    wo = [[NBT, 2], [8, NBT // 8], [1, 7]]
    v.tensor_tensor(
        _ap(wt1[:], 1, wo),
        _ap(P[:], 0, [[2 * NBT, 2], [8, NBT // 8], [1, 7]]),
        _ap(vg[:], 0, [[0, 2], [1, NBT // 8], [0, 7]]),
        op=Alu.mult)
    v.tensor_tensor(
        _ap(wt2[:], 1, wo),
        _ap(P[:], NBT, [[2 * NBT, 2], [8, NBT // 8], [1, 7]]),
        _ap(vg[:], NG, [[0, 2], [1, NBT // 8], [0, 7]]),
        op=Alu.mult)
    v.tensor_tensor(_ap(wb[:], 1, wo), _ap(wt1[:], 1, wo),
                    _ap(wt2[:], 1, wo), op=Alu.add)
    v.tensor_copy(out=_ap(wb[:], 0, [[NBT, 2], [8, NBT // 8], [1, 1]]),
                  in_=_ap(vg[:], 0, [[NG, 2], [1, NBT // 8], [0, 1]]))
    # normalize w
    zw = bs.tile([128, NBT], FP16, tag="zw")
    rw = bs.tile([128, NBT], FP16, tag="rw")
    v.tensor_tensor(zw[:], _ap(wb[:], 0, [[1, NBT]]),
                    _ap(wb[:], NBT, [[1, NBT]]), op=Alu.add)
    v.reciprocal(rw[:], zw[:])
    v.tensor_tensor(_ap(wb[:], 0, [[NBT, 2], [1, NBT]]),
                    _ap(wb[:], 0, [[NBT, 2], [1, NBT]]),
                    _ap(rw[:], 0, [[0, 2], [1, NBT]]), op=Alu.mult)
    es_blk.close()

    # ---------------- within-block down-sweep ----------------
    # Alpha planes AL0/AL1, width W, slot layout (B', s: 8),
    # s = bitrev3(j): even slots 0..3 hold j = 0,4,2,6; odd 4..7: 1,5,3,7.
    es_al = ExitStack()
    alp = es_al.enter_context(tc.tile_pool(name="alp", bufs=1))
    AL = keep.tile([128, 2 * W], FP16, tag="AL")  # AL0 | AL1
    a1t = alp.tile([128, 2 * NBT], FP16, tag="a1t")
    a2t = alp.tile([128, 2 * NBT], FP16, tag="a2t")
    # slot 0 (j=0) = w
    sc.copy(_ap(AL[:], 0, [[W, 2], [8, NBT], [1, 1]]),
            _ap(wb[:], 0, [[NBT, 2], [1, NBT], [0, 1]]))
    # stage 1: slot 1 (j=4) = U4[node0] @ w ; U4 node0 = strided n=0 reads
    v.tensor_tensor(
        _ap(a1t[:], 0, [[NBT, 2], [1, NBT]]),
        _ap(U4[:], 0, [[2 * 2 * NBT, 2], [2, NBT]]),
        _ap(wb[:], 0, [[0, 2], [1, NBT]]), op=Alu.mult)
    v.tensor_tensor(
        _ap(a2t[:], 0, [[NBT, 2], [1, NBT]]),
        _ap(U4[:], 2 * NBT, [[2 * 2 * NBT, 2], [2, NBT]]),
        _ap(wb[:], NBT, [[0, 2], [1, NBT]]), op=Alu.mult)
    v.tensor_tensor(_ap(AL[:], 1, [[W, 2], [8, NBT]]),
                    _ap(a1t[:], 0, [[NBT, 2], [1, NBT]]),
                    _ap(a2t[:], 0, [[NBT, 2], [1, NBT]]), op=Alu.add)
    # stage 2: slots 2,3 (j=2,6) = U2[pair-even p1] @ AL[slots 0,1]
    # U2 even-pair slots {0,1}: pos = i*H + B'*4 + m, m in {0,1}
    b1 = alp.tile([128, 2 * 2 * NBT], FP16, tag="b1")
    b2 = alp.tile([128, 2 * 2 * NBT], FP16, tag="b2")
    s2o = [[2 * NBT, 2], [2, NBT], [1, 2]]
    v.tensor_tensor(
        _ap(b1[:], 0, s2o),
        _ap(U2[:], 0, [[2 * H, 2], [4, NBT], [1, 2]]),
        _ap(AL[:], 0, [[0, 2], [8, NBT], [1, 2]]), op=Alu.mult)
    v.tensor_tensor(
        _ap(b2[:], 0, s2o),
        _ap(U2[:], H, [[2 * H, 2], [4, NBT], [1, 2]]),
        _ap(AL[:], W, [[0, 2], [8, NBT], [1, 2]]), op=Alu.mult)
    v.tensor_tensor(_ap(AL[:], 2, [[W, 2], [8, NBT], [1, 2]]),
                    _ap(b1[:], 0, s2o), _ap(b2[:], 0, s2o), op=Alu.add)
    # stage 3: odd slots 4..7 (j=1,5,3,7) = M_even @ AL[even slots]
    c1 = alp.tile([128, 2 * W // 2], FP16, tag="c1")
    c2 = alp.tile([128, 2 * W // 2], FP16, tag="c2")
    # per row-tile so tile-0 predictions can start before tiles 1-2 finish
    for r in range(RT):
        ob4 = r * 256
        oa = r * T
        s3o = [[H, 2], [4, 64], [1, 4]]
        v.tensor_tensor(
            _ap(c1[:], ob4, s3o),
            _ap(Me[:], ob4, [[2 * H, 2], [4, 64], [1, 4]]),
            _ap(AL[:], oa, [[0, 2], [8, 64], [1, 4]]), op=Alu.mult)
        v.tensor_tensor(
            _ap(c2[:], ob4, s3o),
            _ap(Me[:], H + ob4, [[2 * H, 2], [4, 64], [1, 4]]),
            _ap(AL[:], W + oa, [[0, 2], [8, 64], [1, 4]]), op=Alu.mult)
        v.tensor_tensor(_ap(AL[:], 4 + oa, [[W, 2], [8, 64], [1, 4]]),
                        _ap(c1[:], ob4, s3o), _ap(c2[:], ob4, s3o),
                        op=Alu.add)

    # -------- predictions + lp + cumsum + q, pipelined per row-tile --------
    # per tile r: DVE (Za, Ra, rr, q1) -> Act (q0, ln-unpermute) -> DVE
    # (mask, copy-pred, scan, q-adds) -> relayout DMAs; tiles overlap engines.
    es_pr = ExitStack()
    pr = es_pr.enter_context(tc.tile_pool(name="pr", bufs=1))
    Za = pr.tile([128, W], FP16, tag="Za")
    Ra = pr.tile([128, W], FP16, tag="Ra")
    rr = pr.tile([128, W], FP16, tag="rr")
    q1 = pr.tile([128, W], FP16, tag="q1")
    p1 = keep.tile([128, W], F32, tag="p1")
    p0 = keep.tile([128, W], F32, tag="p0")
    lp = keep.tile([128, W], F32, tag="lp")
    # apc2 has one zero column before each tile's T cumsum columns so the
    # q = p + apc[t-1] add runs full-T with no single-element edge copies.
    apc2 = keep.tile([128, W + RT], F32, tag="apc2")
    q1c = keep.tile([128, W], F32, tag="q1c")
    q0c = keep.tile([128, W], F32, tag="q0c")
    # ability planes split in two tiles so the a0-3 max tree isn't blocked
    # on ability 4's late relayout; partitions [0:64) hold k=1, [64:128) k=0
    # (k=1 lower so tile2's identity move is the later-computed q1).
    QA03 = keep.tile([128, 4 * T], F32, tag="QA03")
    QA4 = keep.tile([128, T], F32, tag="QA4")
    psq = ctx.enter_context(tc.tile_pool(name="psq", bufs=1, space="PSUM"))
    gp.memset(_ap(apc2[:], 0, [[T + 1, RT]]), 0.0)
    slp = [[1, T]]

    def pred_front(r):
        o = r * T
        v.tensor_tensor(_ap(Za[:], o, slp), _ap(AL[:], o, slp),
                        _ap(AL[:], W + o, slp), op=Alu.add)
        v.reciprocal(_ap(Ra[:], o, slp), _ap(Za[:], o, slp))
        v.tensor_tensor(_ap(rr[:], o, slp), _ap(AL[:], W + o, slp),
                        _ap(Ra[:], o, slp), op=Alu.mult)
        v.tensor_tensor(_ap(q1[:], o, slp), _ap(rr[:], o, slp),
                        _ap(dm[:], o, slp), op=Alu.mult)
        v.tensor_tensor(_ap(q1[:], o, slp), _ap(q1[:], o, slp),
                        _ap(pgs[:], o, slp), op=Alu.add)
        # ln with unpermute slot->natural (split by j0); p0 = ln(1 - q1)
        # fuses the complement into the activation's scale/bias
        for j0 in range(2):
            sc.activation(
                _ap(p1[:], o + j0, [[8, 64], [2, 2], [4, 2]]),
                _ap(q1[:], o + 4 * j0, [[8, 64], [2, 2], [1, 2]]),
                Act.Ln)
            sc.activation(
                _ap(p0[:], o + j0, [[8, 64], [2, 2], [4, 2]]),
                _ap(q1[:], o + 4 * j0, [[8, 64], [2, 2], [1, 2]]),
                Act.Ln, bias=1.0, scale=-1.0)

    def pred_scan(r):
        o = r * T
        sc.copy(_ap(lp[:], o, slp), _ap(p0[:], o, slp))
        v.copy_predicated(_ap(lp[:], o, slp), _ap(Ym[:], o, slp),
                          _ap(p1[:], o, slp))
        v.tensor_tensor_scan(_ap(apc2[:], r * (T + 1) + 1, slp),
                             _ap(lp[:], o, slp),
                             _ap(lp[:], o, slp),
                             0.0, Alu.add, Alu.bypass)

    def _qa_dst(k, a):
        base, off = (QA4, 0) if a == 4 else (QA03, a * T)
        return _ap_p(base[:], 64 * (1 - k), 64, off, [[1, T]])

    def pred_q(r):
        # q_k = p_k + apc[t-1] (k=0 on DVE, k=1 on Pool for tiles 0/1),
        # then relayout into QA: moves with matching partition ranges
        # (half == 1-k) are on-chip copies (deferred so they don't block
        # the next tile's critical ops); cross moves are HWDGE DMAs --
        # keeping them off Pool's SWDGE avoids descriptor-gen queueing
        # behind Pool's q1 adds.
        o = r * T
        na = 2 if r < 2 else 1   # tile 2 holds only ability 4 (rows 0-63)
        for k, qsrc, psrc in ((0, q0c, p0), (1, q1c, p1)):
            qeng = gp if k == 1 else v
            qeng.tensor_tensor(_ap(qsrc[:], o, slp),
                               _ap(psrc[:], o, slp),
                               _ap(apc2[:], r * (T + 1), slp), op=Alu.add)
            for half in range(na):
                if half == 1 - k:
                    continue
                sy.dma_start(_qa_dst(k, 2 * r + half),
                             _ap_p(qsrc[:], 64 * half, 64, o, [[1, T]]))

    for r in range(2):
        pred_front(r)
        pred_scan(r)
        pred_q(r)
    pred_front(2)
    pred_scan(2)
    # k=0 identity copies for tiles 0/1, emitted here so they sit in the
    # Act queue ahead of tile 2's q consumers but after its Ln/lp ops
    for r in range(2):
        sc.copy(_qa_dst(0, 2 * r + 1),
                _ap_p(q0c[:], 64, 64, r * T, [[1, T]]))
    pred_q(2)
    # deferred identity copies: k=1 planes for tiles 0/1 on Pool (after
    # both q1 adds), and tile 2's late a4 k=1 move on Act
    for r in range(2):
        gp.tensor_copy(out=_qa_dst(1, 2 * r),
                       in_=_ap_p(q1c[:], 0, 64, r * T, [[1, T]]))
    sc.copy(_qa_dst(1, 4), _ap_p(q1c[:], 0, 64, 2 * T, [[1, T]]))
    es_pr.close()
    es_al.close()
    es_tree.close()
    es_in.close()

    # ---------------- collapse over abilities ----------------
    col2 = ctx.enter_context(tc.tile_pool(name="col2", bufs=1))

    MX = col2.tile([128, T], F32, tag="MX")
    DF = col2.tile([128, A_LEV * T], FP16, tag="DF")
    EX = col2.tile([128, A_LEV * T], FP16, tag="EX")
    SM = col2.tile([128, T], F32, tag="SM")
    un = col2.tile([128, T], F32, tag="un")
    mt = col2.tile([128, 2 * T], F32, tag="mt")
    mth = col2.tile([128, 2 * T], FP16, tag="mth")
    psp = ctx.enter_context(tc.tile_pool(name="psp", bufs=1, space="PSUM"))
    un1s = psp.tile([64, T], F32, tag="un1s")
    dl = col2.tile([64, T], F32, tag="dl")
    ed = col2.tile([64, T], F32, tag="ed")
    sp = col2.tile([64, T], F32, tag="sp")
    OI = col2.tile([64, 2 * T], F32, tag="OI")
    # t-chunked 3-engine pipeline over the collapse.  The a0-3 max tree
    # reads only QA03, so it runs while ability 4's relayout is in flight;
    # only MX/DF wait for QA4.  Partitions [0:64) hold k=1, [64:128) k=0,
    # so dl = un1 - un0 and out0 = -softplus(dl), out1 = dl - softplus(dl).
    NCH = 4
    HT = T // NCH

    def cmaxA(t0):
        # max(a0, a1): depends only on tile 0's relayout -- runs in the
        # DVE gap while tile 1's cross DMA is still in flight
        hl = [[1, HT]]
        v.tensor_tensor(_ap(mt[:], t0, hl),
                        _ap(QA03[:], t0, hl),
                        _ap(QA03[:], T + t0, hl), op=Alu.max)

    def cmax(t0):
        hl = [[1, HT]]
        v.tensor_tensor(_ap(mt[:], T + t0, hl),
                        _ap(QA03[:], 2 * T + t0, hl),
                        _ap(QA03[:], 3 * T + t0, hl), op=Alu.max)
        v.tensor_tensor(_ap(mt[:], t0, hl), _ap(mt[:], t0, hl),
                        _ap(mt[:], T + t0, hl), op=Alu.max)

    def cdf(t0):
        hl = [[1, HT]]
        v.tensor_tensor(_ap(MX[:], t0, hl), _ap(mt[:], t0, hl),
                        _ap(QA4[:], t0, hl), op=Alu.max)
        v.tensor_tensor(_ap(DF[:], t0, [[T, 4], [1, HT]]),
                        _ap(QA03[:], t0, [[T, 4], [1, HT]]),
                        _ap(MX[:], t0, [[0, 4], [1, HT]]),
                        op=Alu.subtract)
        v.tensor_tensor(_ap(DF[:], 4 * T + t0, hl),
                        _ap(QA4[:], t0, hl),
                        _ap(MX[:], t0, hl), op=Alu.subtract)
        sc.activation(_ap(EX[:], t0, [[T, A_LEV], [1, HT]]),
                      _ap(DF[:], t0, [[T, A_LEV], [1, HT]]), Act.Exp)

    def csum(t0):
        hl = [[1, HT]]
        gp.tensor_tensor(_ap(mth[:], t0, [[T, 2], [1, HT]]),
                         _ap(EX[:], t0, [[T, 2], [1, HT]]),
                         _ap(EX[:], 2 * T + t0, [[T, 2], [1, HT]]),
                         op=Alu.add)
        gp.tensor_tensor(_ap(mth[:], t0, hl), _ap(mth[:], t0, hl),
                         _ap(mth[:], T + t0, hl), op=Alu.add)
        v.tensor_tensor(_ap(SM[:], t0, hl), _ap(mth[:], t0, hl),
                        _ap(EX[:], 4 * T + t0, hl), op=Alu.add)
        sc.activation(_ap(SM[:], t0, hl), _ap(SM[:], t0, hl), Act.Ln)
        v.tensor_tensor(_ap(un[:], t0, hl), _ap(MX[:], t0, hl),
                        _ap(SM[:], t0, hl), op=Alu.add)
        # partition shift via idle PE: un1s[j] = un[64+j] (k=0 half down)
        nc.tensor.matmul(_ap_p(un1s[:], 0, 64, t0, hl),
                         SHt[:, 0:64], _ap_p(un[:], 0, 128, t0, hl),
                         start=True, stop=True)

    def ctail(t0, HL, oq):
        hl = [[1, HL]]
        v.tensor_tensor(_ap_p(dl[:], 0, 64, t0, hl),
                        _ap_p(un[:], 0, 64, t0, hl),
                        _ap_p(un1s[:], 0, 64, t0, hl), op=Alu.subtract)
        sc.activation(_ap_p(ed[:], 0, 64, t0, hl),
                      _ap_p(dl[:], 0, 64, t0, hl), Act.Exp)
        sc.activation(_ap_p(sp[:], 0, 64, t0, hl),
                      _ap_p(ed[:], 0, 64, t0, hl), Act.Ln, bias=1.0)
        gp.tensor_scalar(_ap_p(OI[:], 0, 64, 2 * t0, [[2, HL]]),
                         _ap_p(sp[:], 0, 64, t0, hl),
                         -1.0, None, Alu.mult)
        v.tensor_tensor(_ap_p(OI[:], 0, 64, 2 * t0 + 1, [[2, HL]]),
                        _ap_p(dl[:], 0, 64, t0, hl),
                        _ap_p(sp[:], 0, 64, t0, hl), op=Alu.subtract)
        if oq is not None:          # one output DMA per chunk pair
            ot = t0 + HL - 2 * HT
            oq.dma_start(
                bass.AP(O[:].tensor, 2 * ot, [[2 * T, 64], [1, 4 * HT]]),
                _ap_p(OI[:], 0, 64, 2 * ot, [[1, 4 * HT]]))

    for c in range(NCH):
        cmaxA(c * HT)
    for c in range(NCH):
        cmax(c * HT)
    for c in range(NCH):
        cdf(c * HT)
    for c in range(NCH):
        csum(c * HT)
    for c in range(NCH):
        ctail(c * HT, HT, sy if c % 2 == 1 else None)


def _steer_act_tables(arch):
    """Keep Exp/Ln claimed by one table set (see kernel v1)."""
    from concourse import hw_specs
    tabs = hw_specs.get_activation_tables(arch)
    for name, funcs in tabs.items():
        if name == "natural_log_exp_and_others":
            continue
        funcs.discard(Act.Exp)
        funcs.discard(Act.Ln)


def _build_program():
    nc = bacc.Bacc()
    _steer_act_tables(nc.m.arch)
    U0 = nc.declare_dram_parameter("U0", [RT * 128, T], FP16, isOutput=False)
    U1 = nc.declare_dram_parameter("U1", [RT * 128, T], FP16, isOutput=False)
    PG = nc.declare_dram_parameter("PG", [RT * 128, T], FP16, isOutput=False)
    DM = nc.declare_dram_parameter("DM", [RT * 128, T], FP16, isOutput=False)
    Y = nc.declare_dram_parameter("Y", [RT * 128, T], FP16, isOutput=False)
    K = nc.declare_dram_parameter("K", [RT * 128, 16], F32, isOutput=False)
    SH = nc.declare_dram_parameter("SH", [128, 128], F32, isOutput=False)
    O = nc.declare_dram_parameter("O", [BL, T, 2], F32, isOutput=True)
    with ExitStack() as ctx:
        tc = ctx.enter_context(tile.TileContext(nc))
        with nc.allow_low_precision(reason="fp16 HMM chain; validated vs gate"):
            _emit(ctx, tc, nc, U0, U1, PG, DM, Y, K, SH, O)
    if not nc.is_finalized():
        nc.finalize()
    return nc


def _pad_rows(x, dtype=np.float32, fill=0.0):
    out = np.full((RT * 128, x.shape[1]), fill, dtype=dtype)
    out[:ROWS] = x
    return out


def kernel(corr, ytrue, problem, kc, dyn_emb, obs_logits_problem,
           obs_logits_kc, ability_levels, traj, trans_ind, pred_ind):
    global _last_results, _cached_nc
    import ml_dtypes
    fp16 = np.float16

    corr = np.asarray(corr, dtype=np.float32)
    ytrue = np.asarray(ytrue, dtype=np.float32)
    problem = np.asarray(problem)
    kc = np.asarray(kc)
    dyn_emb = np.asarray(dyn_emb, dtype=np.float32)
    obs_logits_problem = np.asarray(obs_logits_problem, dtype=np.float32)
    obs_logits_kc = np.asarray(obs_logits_kc, dtype=np.float32)
    ability = np.asarray(ability_levels, dtype=np.float32)

    obs_core = obs_logits_problem[problem] + obs_logits_kc[kc][:, None, :]
    dyn = dyn_emb[kc]
    sig = lambda x: 1.0 / (1.0 + np.exp(-x.astype(np.float64)))
    lL, lF, lI0 = dyn[:, 0], dyn[:, 1], dyn[:, 2]
    AT00, AT01 = sig(-lL), sig(lF)
    AT10, AT11 = sig(lL), sig(-lF)
    al = [AT00, AT01, AT10, AT11]
    alpha = [al[2 * (i // 2)] * al[i % 2] for i in range(4)]
    # alpha_cp = AT_c0*AT_0p: (c,p): c0 entry = AT[c][0] = al[2c], AT[0][p]=al[p]
    alpha = [al[2 * (i // 2)] * al[i % 2] for i in range(4)]
    beta = [al[2 * (i // 2) + 1] * al[2 + i % 2] for i in range(4)]
    Kfull = np.stack(al + alpha + beta +
                     [sig(-lI0), sig(lI0), np.zeros_like(lL),
                      np.zeros_like(lL)], axis=1).astype(np.float32)  # (B,16)

    # permute the T axis within each 8-block so storage slot s holds
    # natural step j = bitrev3(s); Y stays natural (cumsum order).
    perm = (np.arange(T) & ~7) + np.tile(
        np.array([0, 4, 2, 6, 1, 5, 3, 7]), T // 8)

    in_maps = []
    for c in range(N_CORES):
        sl = slice(c * BL, (c + 1) * BL)
        g = obs_core[sl, :, 0][None, :, perm] + ability[:, None, None]
        s = obs_core[sl, :, 1][None, :, perm] - ability[:, None, None]
        ct = np.broadcast_to(corr[sl][:, perm][None], (A_LEV, BL, T))
        yt = np.broadcast_to(ytrue[sl][None], (A_LEV, BL, T))
        # observation likelihood diagonals, normalized to sum 1 per step
        c2 = 2.0 * ct - 1.0
        u0r = sig(c2 * g)
        u1r = sig(-c2 * s)
        zu = u0r + u1r
        u0n = (u0r / zu).astype(np.float32)
        pg = sig(g)
        dmv = (sig(-s) - pg).astype(np.float32)
        kt = np.broadcast_to(Kfull[sl][None], (A_LEV, BL, 16))
        kpad = _pad_rows(kt.reshape(ROWS, 16), np.float32)
        kpad[ROWS:] = 0.5            # benign transition probs on padded rows
        shm = np.zeros((128, 128), np.float32)
        shm[np.arange(64) + 64, np.arange(64)] = 1.0
        shm[np.arange(64), np.arange(64) + 64] = 1.0
        # pad rows get benign 0.5 probabilities so no inf/nan ever forms
        # there (the PE half-swap matmuls contract over all partitions and
        # 0 * nan would poison valid lanes)
        in_maps.append({
            "U0": _pad_rows(u0n.reshape(ROWS, T), fp16, 0.5),
            "U1": _pad_rows((1.0 - u0n).reshape(ROWS, T), fp16, 0.5),
            "PG": _pad_rows(pg.reshape(ROWS, T).astype(np.float32), fp16, 0.5),
            "DM": _pad_rows(dmv.reshape(ROWS, T), fp16),
            "Y": _pad_rows(yt.reshape(ROWS, T), fp16),
            "K": kpad,
            "SH": shm,
        })

    if _cached_nc is None:
        _cached_nc = _build_program()

    res = run_bass_kernel_spmd(
        _cached_nc, in_maps, list(range(N_CORES)),
        trace=bool(os.environ.get("BASS_TRACE")),
    )
    _last_results = res
    out = np.concatenate([res.results[i]["O"] for i in range(N_CORES)], axis=0)
    return out.astype(np.float32)



# revision 82
# speedup vs baseline: 1.0466x; 1.0225x over previous
"""BKT model kernel v2 for Trainium2 (8 NeuronCores, Bass/Tile).

Exact 2-state HMM reformulation of the reference's 2^n-trajectory fastBKT
(see kernel v1 docstring).  v2 restructures for the DVE cost model:

- fp16 for the whole matrix chain (obs probs, level matrices, tree products,
  alphas, predictions).  The chain is contractive and sum-normalized, so
  fp16's 2^-11 rounding keeps the final error ~2e-3 << the 2e-2 gate;
  subnormal flushes only hit entries whose contribution is negligible.
- planar 2x2-entry planes (one buffer region per matrix entry) so
  tensor_tensor ops read/write packed last dims -> DVE 2x mode; per-partition
  transition constants ride tensor_scalar (2x/4x) and Act-engine scale APs.
- within-block (8-step) products use the A^T = gamma*I + 1 v^T structure at
  level 1, a "parity-split" pair layout for levels 2-3, and a 3-stage vector
  down-sweep for the per-step alphas.
- the 64-block scan is radix-8: in-group Hillis-Steele matrix prefixes,
  a tiny 8-group matrix scan, then one batched mat-vec to get per-block
  start alphas.
- Act engine absorbs sigmoids/copies/lns (including the bit-reversal
  unpermute via 4-free-dim APs); Pool absorbs reductions off the DVE path.
- log-predictions, cumsum and the ability-collapse stay f32.

Sharding: data-parallel over students (B=512 -> 64 per core); 5 ability
levels x 64 students = 320 rows padded to 3 x 128-partition tiles.
"""

import os
import numpy as np
from contextlib import ExitStack

import concourse.bass as bass
import concourse.bacc as bacc
import concourse.mybir as mybir
from concourse import tile
from concourse.bass_utils import run_bass_kernel_spmd

F32 = mybir.dt.float32
FP16 = mybir.dt.float16
Alu = mybir.AluOpType
Act = mybir.ActivationFunctionType
AX = mybir.AxisListType

N_CORES = 8
B_FULL = 512
T = 512
A_LEV = 5
BL = B_FULL // N_CORES          # students per core = 64
ROWS = A_LEV * BL               # valid rows per core = 320
RT = 3                          # row tiles of 128 (384 rows incl. pad)
NBT = RT * 64                   # blocks spanning tiles = 192
W = RT * T                      # full-plane free width = 1536
H = W // 2                      # half width = 768
ABILITY = np.array([-2.0, -1.0, 0.0, 1.0, 2.0], dtype=np.float32)

_last_results = None
_cached_nc = None


def _ap(base, off, dims):
    """Custom AP on the same tensor as `base`, keeping its partition dim."""
    return bass.AP(base.tensor, base.offset + off, [list(base.ap[0])] + dims)


def _ap_p(base, poff, pcount, off, dims):
    p = list(base.ap[0])
    pstride = p[0]
    return bass.AP(
        base.tensor, base.offset + poff * pstride + off, [[pstride, pcount]] + dims
    )


def _emit(ctx, tc, nc, U0, U1, PG, DM, Y, K, SH, O):
    v = nc.vector
    sc = nc.scalar
    gp = nc.gpsimd
    sy = nc.sync

    keep = ctx.enter_context(tc.tile_pool(name="keep", bufs=1))

    # ---------------- input DMAs ----------------
    # U0/U1 are the normalized per-step observation likelihood diagonals
    # (host-side sigmoids, slot-ordered); PG = P(y=1|unlearned) and
    # DM = P(y=1|learned) - PG feed the predictions.  K first (tiny, the
    # M planes need its scalars), then U0/U1 per row-tile on the HWDGE
    # queue; PG/DM/Y trail on Pool's SWDGE (needed only by the preds).
    es_in = ExitStack()
    io = es_in.enter_context(tc.tile_pool(name="io", bufs=1))
    u0 = io.tile([128, W], FP16, tag="U0")
    u1 = io.tile([128, W], FP16, tag="U1")
    pgs = keep.tile([128, W], FP16, tag="PG")
    dm = keep.tile([128, W], FP16, tag="DM")
    Yt = keep.tile([128, W], FP16, tag="Y")
    Kt = keep.tile([128, RT * 16], F32, tag="K")
    gp.dma_start(_ap(Kt[:], 0, [[16, RT], [1, 16]]),
                 bass.AP(K[:].tensor, 0, [[16, 128], [128 * 16, RT], [1, 16]]))
    for r in range(RT):
        for dram, sb in ((U0, u0), (U1, u1)):
            sy.dma_start(_ap(sb[:], r * T, [[1, T]]),
                         bass.AP(dram[:].tensor, r * 128 * T,
                                 [[T, 128], [1, T]]))
    for r in range(RT):
        for dram, sb in ((PG, pgs), (DM, dm)):
            gp.dma_start(_ap(sb[:], r * T, [[1, T]]),
                         bass.AP(dram[:].tensor, r * 128 * T,
                                 [[T, 128], [1, T]]))
    gp.dma_start(_ap(Yt[:], 0, [[T, RT], [1, T]]),
                 bass.AP(Y[:].tensor, 0, [[T, 128], [128 * T, RT], [1, T]]))
    # partition-half swap matrix SW[i, j] = 1 iff |i-j| == 64: PE matmuls
    # with it (or its left half) replace SBUF->SBUF partition-shift DMAs
    SHt = keep.tile([128, 128], F32, tag="SH")
    sy.dma_start(SHt[:], bass.AP(SH[:].tensor, 0, [[128, 128], [1, 128]]))

    def KC(col):
        """Per-partition scalar AP for K column `col` of row-tile r -- but all
        tiles share the op; K scalars differ per tile, so ops over multi-tile
        widths must pass per-tile slices.  Helper returns slice for tile r."""
        return Kt[:, col:col + 1]

    # K layout (16 cols per tile r at r*16):
    # 0..3 : A^T entries AT00, AT01, AT10, AT11
    # 4..7 : alpha_cp = AT_c0*AT_0p   (order 00,01,10,11)
    # 8..11: beta_cp  = AT_c1*AT_1p
    # 12,13: alpha1 init (s(-lI0), s(lI0))

    # ---------------- split u-halves ----------------
    # U0/U1 arrive from the host with the T axis permuted within each
    # 8-block: storage slot s holds natural step j = bitrev3(s), i.e. slot
    # order j = (0,4,2,6,1,5,3,7).  Slots 0..3 are exactly the even-j
    # "parity-split" order j_even(m) = 4*(m&1)+2*(m>>1) the M planes want,
    # slots 4..7 the odds.  M-plane reads are packed (stride-1 runs of 4)
    # -> DVE 4x, and each row-tile r starts as soon as its U DMAs land.
    Me = keep.tile([128, 4 * H], FP16, tag="Me")
    Mo = keep.tile([128, 4 * H], FP16, tag="Mo")
    ME = [Me[:, i * H:(i + 1) * H] for i in range(4)]
    MO = [Mo[:, i * H:(i + 1) * H] for i in range(4)]

    def m_plane(dst_i, usrc, kcol, joff):
        # dst pos = r*256 + b*4 + m  <-  src pos = r*512 + b*8 + 4*joff + m
        for r in range(RT):
            v.tensor_scalar_mul(
                _ap(dst_i, r * 256, [[4, 64], [1, 4]]),
                _ap(usrc[:], r * T + 4 * joff, [[8, 64], [1, 4]]),
                Kt[:, r * 16 + kcol:r * 16 + kcol + 1])
    for i, (us, kc_) in enumerate(((u0, 0), (u1, 1), (u0, 2), (u1, 3))):
        m_plane(ME[i], us, kc_, 0)
        m_plane(MO[i], us, kc_, 1)

    # ---------------- tree level 1: U2 = Modd @ Meven ----------------
    # U2_cp[B',m] = Mo_c0*Me_0p + Mo_c1*Me_1p, elementwise over (B', m);
    # planes are contiguous so everything is packed (2x fp16).
    es_tree = ExitStack()
    tr = es_tree.enter_context(tc.tile_pool(name="tr", bufs=1))
    U2 = tr.tile([128, 4 * H], FP16, tag="U2")
    g1 = tr.tile([128, 4 * H], FP16, tag="g1")
    g2 = tr.tile([128, 4 * H], FP16, tag="g2")
    for c in range(2):
        # dims (p, B'm): B-side Mo_c0 bcast over p; A-side Me_0p planes
        v.tensor_tensor(_ap(g1[:], 2 * c * H, [[H, 2], [1, H]]),
                        _ap(Mo[:], 2 * c * H, [[0, 2], [1, H]]),
                        _ap(Me[:], 0, [[H, 2], [1, H]]), op=Alu.mult)
        v.tensor_tensor(_ap(g2[:], 2 * c * H, [[H, 2], [1, H]]),
                        _ap(Mo[:], (2 * c + 1) * H, [[0, 2], [1, H]]),
                        _ap(Me[:], 2 * H, [[H, 2], [1, H]]), op=Alu.mult)
    v.tensor_tensor(U2[:], g1[:], g2[:], op=Alu.add)

    # prediction-side mask, chunked so it fills Pool gaps greedily
    Ym = keep.tile([128, W], mybir.dt.uint32, tag="Ym")
    for ch in range(6):
        gp.tensor_scalar(_ap(Ym[:], ch * (W // 6), [[1, W // 6]]),
                         _ap(Yt[:], ch * (W // 6), [[1, W // 6]]),
                         0.5, None, Alu.is_ge)

    # ---------------- tree level 2: U4 ----------------
    # U2 pair-evens at slots {0,1} (contig), odds at {2,3}.
    # U4_cp[B', n] = U2o_c0[B',n]*U2e_0p[B',n] + U2o_c1[B',n]*U2e_1p[B',n]
    # U2 planes: pos(i, B', m) = i*H + B'*4 + m ; even-read: m in {0,1}:
    # [[4,NBT],[1,2]]; odd-read: +2.
    U4 = tr.tile([128, 4 * 2 * NBT], FP16, tag="U4")   # planes cp x (B',n)
    t1 = tr.tile([128, 4 * 2 * NBT], FP16, tag="t1")
    t2 = tr.tile([128, 4 * 2 * NBT], FP16, tag="t2")
    # per c (ISA max 3 free dims), iterate (p, B', n):
    # B-side: U2odd_c{k} at plane (2c+k), slots {2,3}: pos = (2c+k)*H+B'*4+2+n
    # A-side: U2even_{k}p at plane (2k+p), slots {0,1}
    # out t: pos = (2c+p)*2*NBT + B'*2 + n
    for c in range(2):
        dims_out = [[2 * NBT, 2], [2, NBT], [1, 2]]
        v.tensor_tensor(
            _ap(t1[:], c * 2 * 2 * NBT, dims_out),
            _ap(U2[:], 2 * c * H + 2, [[0, 2], [4, NBT], [1, 2]]),
            _ap(U2[:], 0, [[H, 2], [4, NBT], [1, 2]]),
            op=Alu.mult)
        v.tensor_tensor(
            _ap(t2[:], c * 2 * 2 * NBT, dims_out),
            _ap(U2[:], (2 * c + 1) * H + 2, [[0, 2], [4, NBT], [1, 2]]),
            _ap(U2[:], 2 * H, [[H, 2], [4, NBT], [1, 2]]),
            op=Alu.mult)
    v.tensor_tensor(U4[:], t1[:], t2[:], op=Alu.add)

    # ---------------- tree level 3: U8 ----------------
    # U4 planes (B', n) interleaved; strided n-reads (1x), packed add.
    U8 = tr.tile([128, 4 * NBT], FP16, tag="U8")       # planes cp x B'
    t3 = tr.tile([128, 4 * NBT], FP16, tag="t3")
    t4 = tr.tile([128, 4 * NBT], FP16, tag="t4")
    od = [[2 * NBT, 2], [NBT, 2], [1, NBT]]
    v.tensor_tensor(_ap(t3[:], 0, od),
                    _ap(U4[:], 1, [[2 * 2 * NBT, 2], [0, 2], [2, NBT]]),
                    _ap(U4[:], 0, [[0, 2], [2 * NBT, 2], [2, NBT]]),
                    op=Alu.mult)
    v.tensor_tensor(_ap(t4[:], 0, od),
                    _ap(U4[:], 2 * NBT + 1,
                        [[2 * 2 * NBT, 2], [0, 2], [2, NBT]]),
                    _ap(U4[:], 4 * NBT, [[0, 2], [2 * NBT, 2], [2, NBT]]),
                    op=Alu.mult)
    v.tensor_tensor(U8[:], t3[:], t4[:], op=Alu.add)

    # normalize U8 (sum of 4 entries -> 1) to keep radix-8 chains in range
    zn = tr.tile([128, NBT], FP16, tag="zn")
    rz = tr.tile([128, NBT], FP16, tag="rz")
    zn2 = tr.tile([128, NBT], FP16, tag="zn2")
    v.tensor_tensor(_ap(zn[:], 0, [[1, NBT]]),
                    _ap(U8[:], 0, [[1, NBT]]),
                    _ap(U8[:], NBT, [[1, NBT]]), op=Alu.add)
    v.tensor_tensor(_ap(zn2[:], 0, [[1, NBT]]),
                    _ap(U8[:], 2 * NBT, [[1, NBT]]),
                    _ap(U8[:], 3 * NBT, [[1, NBT]]), op=Alu.add)
    v.tensor_tensor(_ap(zn[:], 0, [[1, NBT]]),
                    _ap(zn[:], 0, [[1, NBT]]),
                    _ap(zn2[:], 0, [[1, NBT]]), op=Alu.add)
    v.reciprocal(rz[:], zn[:])
    v.tensor_tensor(_ap(U8[:], 0, [[NBT, 4], [1, NBT]]),
                    _ap(U8[:], 0, [[NBT, 4], [1, NBT]]),
                    _ap(rz[:], 0, [[0, 4], [1, NBT]]), op=Alu.mult)

    # ---------------- radix-8 block scan ----------------
    # blocks b in [0,64) per tile; groups g of 8 blocks (8 groups/tile).
    # Step A: in-group inclusive matrix prefixes P[g, j] (HS shifts 1,2,4).
    # P stored planar like U8: planes cp x (B' = tile*64 + 8g + j).
    es_blk = ExitStack()
    bs = es_blk.enter_context(tc.tile_pool(name="bs", bufs=1))
    P = U8
    for h in (1, 2, 4):
        Pn = bs.tile([128, 4 * NBT], FP16, tag=f"P{h}")
        s1 = bs.tile([128, 4 * NBT], FP16, tag=f"s1_{h}")
        s2 = bs.tile([128, 4 * NBT], FP16, tag=f"s2_{h}")
        n = 8 - h
        # C[i] = P[i] * P[i-h] for i in [h,8) within each group
        # per c: dims (p, g, i); B-side P_c{k}[i] at plane (2c+k)
        go = [[NBT, 2], [8, NBT // 8], [1, n]]
        for c in range(2):
            v.tensor_tensor(
                _ap(s1[:], c * 2 * NBT + h, go),
                _ap(P[:], 2 * c * NBT + h, [[0, 2], [8, NBT // 8], [1, n]]),
                _ap(P[:], 0, [[NBT, 2], [8, NBT // 8], [1, n]]),
                op=Alu.mult)
            v.tensor_tensor(
                _ap(s2[:], c * 2 * NBT + h, go),
                _ap(P[:], (2 * c + 1) * NBT + h,
                    [[0, 2], [8, NBT // 8], [1, n]]),
                _ap(P[:], 2 * NBT, [[NBT, 2], [8, NBT // 8], [1, n]]),
                op=Alu.mult)
        v.tensor_tensor(_ap(Pn[:], h, [[NBT, 4], [8, NBT // 8], [1, n]]),
                        _ap(s1[:], h, [[NBT, 4], [8, NBT // 8], [1, n]]),
                        _ap(s2[:], h, [[NBT, 4], [8, NBT // 8], [1, n]]),
                        op=Alu.add)
        # heads [0,h) copy through (DVE: keeps the chain on one queue --
        # an Act round-trip here costs ~2 sem hops + 185ns SBUF latency)
        v.tensor_copy(out=_ap(Pn[:], 0, [[NBT, 4], [8, NBT // 8], [1, h]]),
                      in_=_ap(P[:], 0, [[NBT, 4], [8, NBT // 8], [1, h]]))
        P = Pn

    # Step B: group totals Tg = P[g,7]; normalize; tiny inclusive scan
    # over the 8 groups per tile (HS 1,2,4); then vg = Escan[g-1] @ alpha1.
    # Tg planar: planes cp x (tile r, g): width 4 * 24.
    NG = RT * 8
    Tg = bs.tile([128, 4 * NG], FP16, tag="Tg")
    v.tensor_copy(out=_ap(Tg[:], 0, [[NG, 4], [1, NG]]),
                  in_=_ap(P[:], 7, [[NBT, 4], [8, NG]]))
    # normalize Tg
    zg = bs.tile([128, NG], FP16, tag="zg")
    rg = bs.tile([128, NG], FP16, tag="rg")
    zg2 = bs.tile([128, NG], FP16, tag="zg2")
    v.tensor_tensor(zg[:], _ap(Tg[:], 0, [[1, NG]]),
                    _ap(Tg[:], NG, [[1, NG]]), op=Alu.add)
    v.tensor_tensor(zg2[:], _ap(Tg[:], 2 * NG, [[1, NG]]),
                    _ap(Tg[:], 3 * NG, [[1, NG]]), op=Alu.add)
    v.tensor_tensor(zg[:], zg[:], zg2[:], op=Alu.add)
    v.reciprocal(rg[:], zg[:])
    v.tensor_tensor(_ap(Tg[:], 0, [[NG, 4], [1, NG]]),
                    _ap(Tg[:], 0, [[NG, 4], [1, NG]]),
                    _ap(rg[:], 0, [[0, 4], [1, NG]]), op=Alu.mult)
    E = Tg
    for h in (1, 2, 4):
        En = bs.tile([128, 4 * NG], FP16, tag=f"E{h}")
        e1 = bs.tile([128, 4 * NG], FP16, tag=f"e1_{h}")
        e2 = bs.tile([128, 4 * NG], FP16, tag=f"e2_{h}")
        n = 8 - h
        go = [[NG, 2], [8, RT], [1, n]]
        for c in range(2):
            v.tensor_tensor(
                _ap(e1[:], c * 2 * NG + h, go),
                _ap(E[:], 2 * c * NG + h, [[0, 2], [8, RT], [1, n]]),
                _ap(E[:], 0, [[NG, 2], [8, RT], [1, n]]),
                op=Alu.mult)
            v.tensor_tensor(
                _ap(e2[:], c * 2 * NG + h, go),
                _ap(E[:], (2 * c + 1) * NG + h, [[0, 2], [8, RT], [1, n]]),
                _ap(E[:], 2 * NG, [[NG, 2], [8, RT], [1, n]]),
                op=Alu.mult)
        v.tensor_tensor(_ap(En[:], h, [[NG, 4], [8, RT], [1, n]]),
                        _ap(e1[:], h, [[NG, 4], [8, RT], [1, n]]),
                        _ap(e2[:], h, [[NG, 4], [8, RT], [1, n]]),
                        op=Alu.add)
        v.tensor_copy(out=_ap(En[:], 0, [[NG, 4], [8, RT], [1, h]]),
                      in_=_ap(E[:], 0, [[NG, 4], [8, RT], [1, h]]))
        E = En

    # vg[g] = E[g-1] @ alpha1 for g>=1 ; vg[0] = alpha1.  alpha1 per-tile
    # scalars K cols 12,13.  v-planes: vg0/vg1 width NG.
    vg = bs.tile([128, 2 * NG], FP16, tag="vg")
    vt = bs.tile([128, 2 * NG], FP16, tag="vt")
    for r in range(RT):
        a0 = Kt[:, r * 16 + 12:r * 16 + 13]
        a1 = Kt[:, r * 16 + 13:r * 16 + 14]
        for comp in range(2):
            # vg[comp][r, g] = E_{comp,0}[g-1]*a0 + E_{comp,1}[g-1]*a1
            seg7 = [[1, 7]]
            v.tensor_scalar_mul(
                _ap(vt[:], comp * NG + r * 8 + 1, seg7),
                _ap(E[:], (2 * comp + 1) * NG + r * 8, seg7), a1)
            v.scalar_tensor_tensor(
                _ap(vg[:], comp * NG + r * 8 + 1, seg7),
                _ap(E[:], (2 * comp) * NG + r * 8, seg7), a0,
                _ap(vt[:], comp * NG + r * 8 + 1, seg7), Alu.mult, Alu.add)
        v.tensor_copy(out=_ap(vg[:], r * 8, [[NG, 2], [1, 1]]),
                      in_=_ap(Kt[:], r * 16 + 12, [[1, 2], [0, 1]]))
    # normalize vg
    zv = bs.tile([128, NG], FP16, tag="zv")
    rv = bs.tile([128, NG], FP16, tag="rv")
    v.tensor_tensor(zv[:], _ap(vg[:], 0, [[1, NG]]),
                    _ap(vg[:], NG, [[1, NG]]), op=Alu.add)
    v.reciprocal(rv[:], zv[:])
    v.tensor_tensor(_ap(vg[:], 0, [[NG, 2], [1, NG]]),
                    _ap(vg[:], 0, [[NG, 2], [1, NG]]),
                    _ap(rv[:], 0, [[0, 2], [1, NG]]), op=Alu.mult)

    # Step C: w_b for all blocks.  w[8g] = vg[g]; w[8g+j] = P[g,j-1] @ vg[g].
    # w planes: w0/w1 width NBT (B'-indexed).
    wb = tr.tile([128, 2 * NBT], FP16, tag="wb")
    wt1 = bs.tile([128, 2 * NBT], FP16, tag="wt1")
    wt2 = bs.tile([128, 2 * NBT], FP16, tag="wt2")
    # dims (comp, g, j in 1..7): w_c = P_c0[g,j-1]*vg_0[g] + P_c1[g,j-1]*vg_1[g]
    wo = [[NBT, 2], [8, NBT // 8], [1, 7]]
    v.tensor_tensor(
        _ap(wt1[:], 1, wo),
        _ap(P[:], 0, [[2 * NBT, 2], [8, NBT // 8], [1, 7]]),
        _ap(vg[:], 0, [[0, 2], [1, NBT // 8], [0, 7]]),
        op=Alu.mult)
    v.tensor_tensor(
        _ap(wt2[:], 1, wo),
        _ap(P[:], NBT, [[2 * NBT, 2], [8, NBT // 8], [1, 7]]),
        _ap(vg[:], NG, [[0, 2], [1, NBT // 8], [0, 7]]),
        op=Alu.mult)
    v.tensor_tensor(_ap(wb[:], 1, wo), _ap(wt1[:], 1, wo),
                    _ap(wt2[:], 1, wo), op=Alu.add)
    v.tensor_copy(out=_ap(wb[:], 0, [[NBT, 2], [8, NBT // 8], [1, 1]]),
                  in_=_ap(vg[:], 0, [[NG, 2], [1, NBT // 8], [0, 1]]))
    # normalize w
    zw = bs.tile([128, NBT], FP16, tag="zw")
    rw = bs.tile([128, NBT], FP16, tag="rw")
    v.tensor_tensor(zw[:], _ap(wb[:], 0, [[1, NBT]]),
                    _ap(wb[:], NBT, [[1, NBT]]), op=Alu.add)
    v.reciprocal(rw[:], zw[:])
    v.tensor_tensor(_ap(wb[:], 0, [[NBT, 2], [1, NBT]]),
                    _ap(wb[:], 0, [[NBT, 2], [1, NBT]]),
                    _ap(rw[:], 0, [[0, 2], [1, NBT]]), op=Alu.mult)
    es_blk.close()

    # ---------------- within-block down-sweep ----------------
    # Alpha planes AL0/AL1, width W, slot layout (B', s: 8),
    # s = bitrev3(j): even slots 0..3 hold j = 0,4,2,6; odd 4..7: 1,5,3,7.
    es_al = ExitStack()
    alp = es_al.enter_context(tc.tile_pool(name="alp", bufs=1))
    AL = keep.tile([128, 2 * W], FP16, tag="AL")  # AL0 | AL1
    a1t = alp.tile([128, 2 * NBT], FP16, tag="a1t")
    a2t = alp.tile([128, 2 * NBT], FP16, tag="a2t")
    # slot 0 (j=0) = w
    sc.copy(_ap(AL[:], 0, [[W, 2], [8, NBT], [1, 1]]),
            _ap(wb[:], 0, [[NBT, 2], [1, NBT], [0, 1]]))
    # stage 1: slot 1 (j=4) = U4[node0] @ w ; U4 node0 = strided n=0 reads
    v.tensor_tensor(
        _ap(a1t[:], 0, [[NBT, 2], [1, NBT]]),
        _ap(U4[:], 0, [[2 * 2 * NBT, 2], [2, NBT]]),
        _ap(wb[:], 0, [[0, 2], [1, NBT]]), op=Alu.mult)
    v.tensor_tensor(
        _ap(a2t[:], 0, [[NBT, 2], [1, NBT]]),
        _ap(U4[:], 2 * NBT, [[2 * 2 * NBT, 2], [2, NBT]]),
        _ap(wb[:], NBT, [[0, 2], [1, NBT]]), op=Alu.mult)
    v.tensor_tensor(_ap(AL[:], 1, [[W, 2], [8, NBT]]),
                    _ap(a1t[:], 0, [[NBT, 2], [1, NBT]]),
                    _ap(a2t[:], 0, [[NBT, 2], [1, NBT]]), op=Alu.add)
    # stage 2: slots 2,3 (j=2,6) = U2[pair-even p1] @ AL[slots 0,1]
    # U2 even-pair slots {0,1}: pos = i*H + B'*4 + m, m in {0,1}
    b1 = alp.tile([128, 2 * 2 * NBT], FP16, tag="b1")
    b2 = alp.tile([128, 2 * 2 * NBT], FP16, tag="b2")
    s2o = [[2 * NBT, 2], [2, NBT], [1, 2]]
    v.tensor_tensor(
        _ap(b1[:], 0, s2o),
        _ap(U2[:], 0, [[2 * H, 2], [4, NBT], [1, 2]]),
        _ap(AL[:], 0, [[0, 2], [8, NBT], [1, 2]]), op=Alu.mult)
    v.tensor_tensor(
        _ap(b2[:], 0, s2o),
        _ap(U2[:], H, [[2 * H, 2], [4, NBT], [1, 2]]),
        _ap(AL[:], W, [[0, 2], [8, NBT], [1, 2]]), op=Alu.mult)
    v.tensor_tensor(_ap(AL[:], 2, [[W, 2], [8, NBT], [1, 2]]),
                    _ap(b1[:], 0, s2o), _ap(b2[:], 0, s2o), op=Alu.add)
    # stage 3: odd slots 4..7 (j=1,5,3,7) = M_even @ AL[even slots]
    c1 = alp.tile([128, 2 * W // 2], FP16, tag="c1")
    c2 = alp.tile([128, 2 * W // 2], FP16, tag="c2")
    # per row-tile so tile-0 predictions can start before tiles 1-2 finish
    for r in range(RT):
        ob4 = r * 256
        oa = r * T
        s3o = [[H, 2], [4, 64], [1, 4]]
        v.tensor_tensor(
            _ap(c1[:], ob4, s3o),
            _ap(Me[:], ob4, [[2 * H, 2], [4, 64], [1, 4]]),
            _ap(AL[:], oa, [[0, 2], [8, 64], [1, 4]]), op=Alu.mult)
        v.tensor_tensor(
            _ap(c2[:], ob4, s3o),
            _ap(Me[:], H + ob4, [[2 * H, 2], [4, 64], [1, 4]]),
            _ap(AL[:], W + oa, [[0, 2], [8, 64], [1, 4]]), op=Alu.mult)
        v.tensor_tensor(_ap(AL[:], 4 + oa, [[W, 2], [8, 64], [1, 4]]),
                        _ap(c1[:], ob4, s3o), _ap(c2[:], ob4, s3o),
                        op=Alu.add)

    # -------- predictions + lp + cumsum + q, pipelined per row-tile --------
    # per tile r: DVE (Za, Ra, rr, q1) -> Act (q0, ln-unpermute) -> DVE
    # (mask, copy-pred, scan, q-adds) -> relayout DMAs; tiles overlap engines.
    es_pr = ExitStack()
    pr = es_pr.enter_context(tc.tile_pool(name="pr", bufs=1))
    Za = pr.tile([128, W], FP16, tag="Za")
    Ra = pr.tile([128, W], FP16, tag="Ra")
    rr = pr.tile([128, W], FP16, tag="rr")
    q1 = pr.tile([128, W], FP16, tag="q1")
    p1 = keep.tile([128, W], F32, tag="p1")
    p0 = keep.tile([128, W], F32, tag="p0")
    lp = keep.tile([128, W], F32, tag="lp")
    # apc2 has one zero column before each tile's T cumsum columns so the
    # q = p + apc[t-1] add runs full-T with no single-element edge copies.
    apc2 = keep.tile([128, W + RT], F32, tag="apc2")
    q1c = keep.tile([128, W], F32, tag="q1c")
    q0c = keep.tile([128, W], F32, tag="q0c")
    # ability planes split in two tiles so the a0-3 max tree isn't blocked
    # on ability 4's late relayout; partitions [0:64) hold k=1, [64:128) k=0
    # (k=1 lower so tile2's identity move is the later-computed q1).
    QA03 = keep.tile([128, 4 * T], F32, tag="QA03")
    QA4 = keep.tile([128, T], F32, tag="QA4")
    psq = ctx.enter_context(tc.tile_pool(name="psq", bufs=1, space="PSUM"))
    gp.memset(_ap(apc2[:], 0, [[T + 1, RT]]), 0.0)
    slp = [[1, T]]

    def pred_front(r):
        o = r * T
        v.tensor_tensor(_ap(Za[:], o, slp), _ap(AL[:], o, slp),
                        _ap(AL[:], W + o, slp), op=Alu.add)
        v.reciprocal(_ap(Ra[:], o, slp), _ap(Za[:], o, slp))
        v.tensor_tensor(_ap(rr[:], o, slp), _ap(AL[:], W + o, slp),
                        _ap(Ra[:], o, slp), op=Alu.mult)
        v.tensor_tensor(_ap(q1[:], o, slp), _ap(rr[:], o, slp),
                        _ap(dm[:], o, slp), op=Alu.mult)
        v.tensor_tensor(_ap(q1[:], o, slp), _ap(q1[:], o, slp),
                        _ap(pgs[:], o, slp), op=Alu.add)
        # ln with unpermute slot->natural (split by j0); p0 = ln(1 - q1)
        # fuses the complement into the activation's scale/bias
        for j0 in range(2):
            sc.activation(
                _ap(p1[:], o + j0, [[8, 64], [2, 2], [4, 2]]),
                _ap(q1[:], o + 4 * j0, [[8, 64], [2, 2], [1, 2]]),
                Act.Ln)
            sc.activation(
                _ap(p0[:], o + j0, [[8, 64], [2, 2], [4, 2]]),
                _ap(q1[:], o + 4 * j0, [[8, 64], [2, 2], [1, 2]]),
                Act.Ln, bias=1.0, scale=-1.0)

    def pred_scan(r):
        o = r * T
        sc.copy(_ap(lp[:], o, slp), _ap(p0[:], o, slp))
        v.copy_predicated(_ap(lp[:], o, slp), _ap(Ym[:], o, slp),
                          _ap(p1[:], o, slp))
        v.tensor_tensor_scan(_ap(apc2[:], r * (T + 1) + 1, slp),
                             _ap(lp[:], o, slp),
                             _ap(lp[:], o, slp),
                             0.0, Alu.add, Alu.bypass)

    def _qa_dst(k, a):
        base, off = (QA4, 0) if a == 4 else (QA03, a * T)
        return _ap_p(base[:], 64 * (1 - k), 64, off, [[1, T]])

    def pred_q(r):
        # q_k = p_k + apc[t-1] (k=0 on DVE, k=1 on Pool for tiles 0/1),
        # then relayout into QA: moves with matching partition ranges
        # (half == 1-k) are on-chip copies (deferred so they don't block
        # the next tile's critical ops); cross moves are HWDGE DMAs --
        # keeping them off Pool's SWDGE avoids descriptor-gen queueing
        # behind Pool's q1 adds.
        o = r * T
        na = 2 if r < 2 else 1   # tile 2 holds only ability 4 (rows 0-63)
        for k, qsrc, psrc in ((0, q0c, p0), (1, q1c, p1)):
            qeng = gp if k == 1 else v
            qeng.tensor_tensor(_ap(qsrc[:], o, slp),
                               _ap(psrc[:], o, slp),
                               _ap(apc2[:], r * (T + 1), slp), op=Alu.add)
            for half in range(na):
                if half == 1 - k:
                    continue
                sy.dma_start(_qa_dst(k, 2 * r + half),
                             _ap_p(qsrc[:], 64 * half, 64, o, [[1, T]]))

    for r in range(2):
        pred_front(r)
        pred_scan(r)
        pred_q(r)
    pred_front(2)
    pred_scan(2)
    # k=0 identity copies for tiles 0/1, emitted here so they sit in the
    # Act queue ahead of tile 2's q consumers but after its Ln/lp ops
    for r in range(2):
        sc.copy(_qa_dst(0, 2 * r + 1),
                _ap_p(q0c[:], 64, 64, r * T, [[1, T]]))
    pred_q(2)
    # deferred identity copies: k=1 planes for tiles 0/1 on Pool (after
    # both q1 adds), and tile 2's late a4 k=1 move on Act
    for r in range(2):
        gp.tensor_copy(out=_qa_dst(1, 2 * r),
                       in_=_ap_p(q1c[:], 0, 64, r * T, [[1, T]]))
    sc.copy(_qa_dst(1, 4), _ap_p(q1c[:], 0, 64, 2 * T, [[1, T]]))
    es_pr.close()
    es_al.close()
    es_tree.close()
    es_in.close()

    # ---------------- collapse over abilities ----------------
    col2 = ctx.enter_context(tc.tile_pool(name="col2", bufs=1))

    MX = col2.tile([128, T], F32, tag="MX")
    DF = col2.tile([128, A_LEV * T], FP16, tag="DF")
    EX = col2.tile([128, A_LEV * T], FP16, tag="EX")
    SM = col2.tile([128, T], F32, tag="SM")
    un = col2.tile([128, T], F32, tag="un")
    mt = col2.tile([128, 2 * T], F32, tag="mt")
    mth = col2.tile([128, 2 * T], FP16, tag="mth")
    psp = ctx.enter_context(tc.tile_pool(name="psp", bufs=1, space="PSUM"))
    un1s0 = psp.tile([64, T // 4], F32, tag="un1s0")
    un1s1 = psp.tile([64, T // 4], F32, tag="un1s1")
    un1s2 = psp.tile([64, T // 4], F32, tag="un1s2")
    un1s3 = psp.tile([64, T // 4], F32, tag="un1s3")
    un1s = [un1s0, un1s1, un1s2, un1s3]
    dl = col2.tile([64, T], F32, tag="dl")
    ed = col2.tile([64, T], F32, tag="ed")
    sp = col2.tile([64, T], F32, tag="sp")
    OI = col2.tile([64, 2 * T], F32, tag="OI")
    # t-chunked 3-engine pipeline over the collapse.  The a0-3 max tree
    # reads only QA03, so it runs while ability 4's relayout is in flight;
    # only MX/DF wait for QA4.  Partitions [0:64) hold k=1, [64:128) k=0,
    # so dl = un1 - un0 and out0 = -softplus(dl), out1 = dl - softplus(dl).
    NCH = 4
    HT = T // NCH

    def cmaxA(t0):
        # max(a0, a1): depends only on tile 0's relayout -- runs in the
        # DVE gap while tile 1's cross DMA is still in flight
        hl = [[1, HT]]
        v.tensor_tensor(_ap(mt[:], t0, hl),
                        _ap(QA03[:], t0, hl),
                        _ap(QA03[:], T + t0, hl), op=Alu.max)

    def cmax(t0):
        hl = [[1, HT]]
        v.tensor_tensor(_ap(mt[:], T + t0, hl),
                        _ap(QA03[:], 2 * T + t0, hl),
                        _ap(QA03[:], 3 * T + t0, hl), op=Alu.max)
        v.tensor_tensor(_ap(mt[:], t0, hl), _ap(mt[:], t0, hl),
                        _ap(mt[:], T + t0, hl), op=Alu.max)

    def cdf(t0):
        hl = [[1, HT]]
        v.tensor_tensor(_ap(MX[:], t0, hl), _ap(mt[:], t0, hl),
                        _ap(QA4[:], t0, hl), op=Alu.max)
        v.tensor_tensor(_ap(DF[:], t0, [[T, 4], [1, HT]]),
                        _ap(QA03[:], t0, [[T, 4], [1, HT]]),
                        _ap(MX[:], t0, [[0, 4], [1, HT]]),
                        op=Alu.subtract)
        v.tensor_tensor(_ap(DF[:], 4 * T + t0, hl),
                        _ap(QA4[:], t0, hl),
                        _ap(MX[:], t0, hl), op=Alu.subtract)
        sc.activation(_ap(EX[:], t0, [[T, A_LEV], [1, HT]]),
                      _ap(DF[:], t0, [[T, A_LEV], [1, HT]]), Act.Exp)

    def csum(t0):
        hl = [[1, HT]]
        gp.tensor_tensor(_ap(mth[:], t0, [[T, 2], [1, HT]]),
                         _ap(EX[:], t0, [[T, 2], [1, HT]]),
                         _ap(EX[:], 2 * T + t0, [[T, 2], [1, HT]]),
                         op=Alu.add)
        gp.tensor_tensor(_ap(mth[:], t0, hl), _ap(mth[:], t0, hl),
                         _ap(mth[:], T + t0, hl), op=Alu.add)
        v.tensor_tensor(_ap(SM[:], t0, hl), _ap(mth[:], t0, hl),
                        _ap(EX[:], 4 * T + t0, hl), op=Alu.add)
        sc.activation(_ap(SM[:], t0, hl), _ap(SM[:], t0, hl), Act.Ln)
        v.tensor_tensor(_ap(un[:], t0, hl), _ap(MX[:], t0, hl),
                        _ap(SM[:], t0, hl), op=Alu.add)
        # partition shift via idle PE: un1s[j] = un[64+j] (k=0 half down);
        # one PSUM tile per chunk so consumers don't serialize on tile deps
        nc.tensor.matmul(_ap_p(un1s[t0 // HT][:], 0, 64, 0, hl),
                         SHt[:, 0:64], _ap_p(un[:], 0, 128, t0, hl),
                         start=True, stop=True)

    def ctail(t0, HL, oq):
        hl = [[1, HL]]
        v.tensor_tensor(_ap_p(dl[:], 0, 64, t0, hl),
                        _ap_p(un[:], 0, 64, t0, hl),
                        _ap_p(un1s[t0 // (T // 4)][:], 0, 64, 0, hl),
                        op=Alu.subtract)
        sc.activation(_ap_p(ed[:], 0, 64, t0, hl),
                      _ap_p(dl[:], 0, 64, t0, hl), Act.Exp)
        sc.activation(_ap_p(sp[:], 0, 64, t0, hl),
                      _ap_p(ed[:], 0, 64, t0, hl), Act.Ln, bias=1.0)
        gp.tensor_scalar(_ap_p(OI[:], 0, 64, 2 * t0, [[2, HL]]),
                         _ap_p(sp[:], 0, 64, t0, hl),
                         -1.0, None, Alu.mult)
        v.tensor_tensor(_ap_p(OI[:], 0, 64, 2 * t0 + 1, [[2, HL]]),
                        _ap_p(dl[:], 0, 64, t0, hl),
                        _ap_p(sp[:], 0, 64, t0, hl), op=Alu.subtract)
        if oq is not None:          # one output DMA per chunk pair
            ot = t0 + HL - 2 * HT
            oq.dma_start(
                bass.AP(O[:].tensor, 2 * ot, [[2 * T, 64], [1, 4 * HT]]),
                _ap_p(OI[:], 0, 64, 2 * ot, [[1, 4 * HT]]))

    for c in range(NCH):
        cmaxA(c * HT)
    for c in range(NCH):
        cmax(c * HT)
    for c in range(NCH):
        cdf(c * HT)
    for c in range(NCH):
        csum(c * HT)
    for c in range(NCH):
        ctail(c * HT, HT, sy if c % 2 == 1 else None)


def _steer_act_tables(arch):
    """Keep Exp/Ln claimed by one table set (see kernel v1)."""
    from concourse import hw_specs
    tabs = hw_specs.get_activation_tables(arch)
    for name, funcs in tabs.items():
        if name == "natural_log_exp_and_others":
            continue
        funcs.discard(Act.Exp)
        funcs.discard(Act.Ln)


def _build_program():
    nc = bacc.Bacc()
    _steer_act_tables(nc.m.arch)
    U0 = nc.declare_dram_parameter("U0", [RT * 128, T], FP16, isOutput=False)
    U1 = nc.declare_dram_parameter("U1", [RT * 128, T], FP16, isOutput=False)
    PG = nc.declare_dram_parameter("PG", [RT * 128, T], FP16, isOutput=False)
    DM = nc.declare_dram_parameter("DM", [RT * 128, T], FP16, isOutput=False)
    Y = nc.declare_dram_parameter("Y", [RT * 128, T], FP16, isOutput=False)
    K = nc.declare_dram_parameter("K", [RT * 128, 16], F32, isOutput=False)
    SH = nc.declare_dram_parameter("SH", [128, 128], F32, isOutput=False)
    O = nc.declare_dram_parameter("O", [BL, T, 2], F32, isOutput=True)
    with ExitStack() as ctx:
        tc = ctx.enter_context(tile.TileContext(nc))
        with nc.allow_low_precision(reason="fp16 HMM chain; validated vs gate"):
            _emit(ctx, tc, nc, U0, U1, PG, DM, Y, K, SH, O)
    if not nc.is_finalized():
        nc.finalize()
    return nc


def _pad_rows(x, dtype=np.float32, fill=0.0):
    out = np.full((RT * 128, x.shape[1]), fill, dtype=dtype)
    out[:ROWS] = x
    return out


def kernel(corr, ytrue, problem, kc, dyn_emb, obs_logits_problem,
           obs_logits_kc, ability_levels, traj, trans_ind, pred_ind):
    global _last_results, _cached_nc
    import ml_dtypes
    fp16 = np.float16

    corr = np.asarray(corr, dtype=np.float32)
    ytrue = np.asarray(ytrue, dtype=np.float32)
    problem = np.asarray(problem)
    kc = np.asarray(kc)
    dyn_emb = np.asarray(dyn_emb, dtype=np.float32)
    obs_logits_problem = np.asarray(obs_logits_problem, dtype=np.float32)
    obs_logits_kc = np.asarray(obs_logits_kc, dtype=np.float32)
    ability = np.asarray(ability_levels, dtype=np.float32)

    obs_core = obs_logits_problem[problem] + obs_logits_kc[kc][:, None, :]
    dyn = dyn_emb[kc]
    sig = lambda x: 1.0 / (1.0 + np.exp(-x.astype(np.float64)))
    lL, lF, lI0 = dyn[:, 0], dyn[:, 1], dyn[:, 2]
    AT00, AT01 = sig(-lL), sig(lF)
    AT10, AT11 = sig(lL), sig(-lF)
    al = [AT00, AT01, AT10, AT11]
    alpha = [al[2 * (i // 2)] * al[i % 2] for i in range(4)]
    # alpha_cp = AT_c0*AT_0p: (c,p): c0 entry = AT[c][0] = al[2c], AT[0][p]=al[p]
    alpha = [al[2 * (i // 2)] * al[i % 2] for i in range(4)]
    beta = [al[2 * (i // 2) + 1] * al[2 + i % 2] for i in range(4)]
    Kfull = np.stack(al + alpha + beta +
                     [sig(-lI0), sig(lI0), np.zeros_like(lL),
                      np.zeros_like(lL)], axis=1).astype(np.float32)  # (B,16)

    # permute the T axis within each 8-block so storage slot s holds
    # natural step j = bitrev3(s); Y stays natural (cumsum order).
    perm = (np.arange(T) & ~7) + np.tile(
        np.array([0, 4, 2, 6, 1, 5, 3, 7]), T // 8)

    in_maps = []
    for c in range(N_CORES):
        sl = slice(c * BL, (c + 1) * BL)
        g = obs_core[sl, :, 0][None, :, perm] + ability[:, None, None]
        s = obs_core[sl, :, 1][None, :, perm] - ability[:, None, None]
        ct = np.broadcast_to(corr[sl][:, perm][None], (A_LEV, BL, T))
        yt = np.broadcast_to(ytrue[sl][None], (A_LEV, BL, T))
        # observation likelihood diagonals, normalized to sum 1 per step
        c2 = 2.0 * ct - 1.0
        u0r = sig(c2 * g)
        u1r = sig(-c2 * s)
        zu = u0r + u1r
        u0n = (u0r / zu).astype(np.float32)
        pg = sig(g)
        dmv = (sig(-s) - pg).astype(np.float32)
        kt = np.broadcast_to(Kfull[sl][None], (A_LEV, BL, 16))
        kpad = _pad_rows(kt.reshape(ROWS, 16), np.float32)
        kpad[ROWS:] = 0.5            # benign transition probs on padded rows
        shm = np.zeros((128, 128), np.float32)
        shm[np.arange(64) + 64, np.arange(64)] = 1.0
        shm[np.arange(64), np.arange(64) + 64] = 1.0
        # pad rows get benign 0.5 probabilities so no inf/nan ever forms
        # there (the PE half-swap matmuls contract over all partitions and
        # 0 * nan would poison valid lanes)
        in_maps.append({
            "U0": _pad_rows(u0n.reshape(ROWS, T), fp16, 0.5),
            "U1": _pad_rows((1.0 - u0n).reshape(ROWS, T), fp16, 0.5),
            "PG": _pad_rows(pg.reshape(ROWS, T).astype(np.float32), fp16, 0.5),
            "DM": _pad_rows(dmv.reshape(ROWS, T), fp16),
            "Y": _pad_rows(yt.reshape(ROWS, T), fp16),
            "K": kpad,
            "SH": shm,
        })

    if _cached_nc is None:
        _cached_nc = _build_program()

    res = run_bass_kernel_spmd(
        _cached_nc, in_maps, list(range(N_CORES)),
        trace=bool(os.environ.get("BASS_TRACE")),
    )
    _last_results = res
    out = np.concatenate([res.results[i]["O"] for i in range(N_CORES)], axis=0)
    return out.astype(np.float32)



# revision 83
# speedup vs baseline: 1.0509x; 1.0040x over previous
"""BKT model kernel v2 for Trainium2 (8 NeuronCores, Bass/Tile).

Exact 2-state HMM reformulation of the reference's 2^n-trajectory fastBKT
(see kernel v1 docstring).  v2 restructures for the DVE cost model:

- fp16 for the whole matrix chain (obs probs, level matrices, tree products,
  alphas, predictions).  The chain is contractive and sum-normalized, so
  fp16's 2^-11 rounding keeps the final error ~2e-3 << the 2e-2 gate;
  subnormal flushes only hit entries whose contribution is negligible.
- planar 2x2-entry planes (one buffer region per matrix entry) so
  tensor_tensor ops read/write packed last dims -> DVE 2x mode; per-partition
  transition constants ride tensor_scalar (2x/4x) and Act-engine scale APs.
- within-block (8-step) products use the A^T = gamma*I + 1 v^T structure at
  level 1, a "parity-split" pair layout for levels 2-3, and a 3-stage vector
  down-sweep for the per-step alphas.
- the 64-block scan is radix-8: in-group Hillis-Steele matrix prefixes,
  a tiny 8-group matrix scan, then one batched mat-vec to get per-block
  start alphas.
- Act engine absorbs sigmoids/copies/lns (including the bit-reversal
  unpermute via 4-free-dim APs); Pool absorbs reductions off the DVE path.
- log-predictions, cumsum and the ability-collapse stay f32.

Sharding: data-parallel over students (B=512 -> 64 per core); 5 ability
levels x 64 students = 320 rows padded to 3 x 128-partition tiles.
"""

import os
import numpy as np
from contextlib import ExitStack

import concourse.bass as bass
import concourse.bacc as bacc
import concourse.mybir as mybir
from concourse import tile
from concourse.bass_utils import run_bass_kernel_spmd

F32 = mybir.dt.float32
FP16 = mybir.dt.float16
Alu = mybir.AluOpType
Act = mybir.ActivationFunctionType
AX = mybir.AxisListType

N_CORES = 8
B_FULL = 512
T = 512
A_LEV = 5
BL = B_FULL // N_CORES          # students per core = 64
ROWS = A_LEV * BL               # valid rows per core = 320
RT = 3                          # row tiles of 128 (384 rows incl. pad)
NBT = RT * 64                   # blocks spanning tiles = 192
W = RT * T                      # full-plane free width = 1536
H = W // 2                      # half width = 768
ABILITY = np.array([-2.0, -1.0, 0.0, 1.0, 2.0], dtype=np.float32)

_last_results = None
_cached_nc = None


def _ap(base, off, dims):
    """Custom AP on the same tensor as `base`, keeping its partition dim."""
    return bass.AP(base.tensor, base.offset + off, [list(base.ap[0])] + dims)


def _ap_p(base, poff, pcount, off, dims):
    p = list(base.ap[0])
    pstride = p[0]
    return bass.AP(
        base.tensor, base.offset + poff * pstride + off, [[pstride, pcount]] + dims
    )


def _emit(ctx, tc, nc, U0, U1, PG, DM, Y, K, SH, O):
    v = nc.vector
    sc = nc.scalar
    gp = nc.gpsimd
    sy = nc.sync

    keep = ctx.enter_context(tc.tile_pool(name="keep", bufs=1))

    # ---------------- input DMAs ----------------
    # U0/U1 are the normalized per-step observation likelihood diagonals
    # (host-side sigmoids, slot-ordered); PG = P(y=1|unlearned) and
    # DM = P(y=1|learned) - PG feed the predictions.  K first (tiny, the
    # M planes need its scalars), then U0/U1 per row-tile on the HWDGE
    # queue; PG/DM/Y trail on Pool's SWDGE (needed only by the preds).
    es_in = ExitStack()
    io = es_in.enter_context(tc.tile_pool(name="io", bufs=1))
    u0 = io.tile([128, W], FP16, tag="U0")
    u1 = io.tile([128, W], FP16, tag="U1")
    pgs = keep.tile([128, W], FP16, tag="PG")
    dm = keep.tile([128, W], FP16, tag="DM")
    Yt = keep.tile([128, W], FP16, tag="Y")
    Kt = keep.tile([128, RT * 16], F32, tag="K")
    gp.dma_start(_ap(Kt[:], 0, [[16, RT], [1, 16]]),
                 bass.AP(K[:].tensor, 0, [[16, 128], [128 * 16, RT], [1, 16]]))
    for r in range(RT):
        for dram, sb in ((U0, u0), (U1, u1)):
            sy.dma_start(_ap(sb[:], r * T, [[1, T]]),
                         bass.AP(dram[:].tensor, r * 128 * T,
                                 [[T, 128], [1, T]]))
    for r in range(RT):
        for dram, sb in ((PG, pgs), (DM, dm)):
            gp.dma_start(_ap(sb[:], r * T, [[1, T]]),
                         bass.AP(dram[:].tensor, r * 128 * T,
                                 [[T, 128], [1, T]]))
    gp.dma_start(_ap(Yt[:], 0, [[T, RT], [1, T]]),
                 bass.AP(Y[:].tensor, 0, [[T, 128], [128 * T, RT], [1, T]]))
    # partition-half swap matrix SW[i, j] = 1 iff |i-j| == 64: PE matmuls
    # with it (or its left half) replace SBUF->SBUF partition-shift DMAs
    SHt = keep.tile([128, 128], F32, tag="SH")
    sy.dma_start(SHt[:], bass.AP(SH[:].tensor, 0, [[128, 128], [1, 128]]))

    def KC(col):
        """Per-partition scalar AP for K column `col` of row-tile r -- but all
        tiles share the op; K scalars differ per tile, so ops over multi-tile
        widths must pass per-tile slices.  Helper returns slice for tile r."""
        return Kt[:, col:col + 1]

    # K layout (16 cols per tile r at r*16):
    # 0..3 : A^T entries AT00, AT01, AT10, AT11
    # 4..7 : alpha_cp = AT_c0*AT_0p   (order 00,01,10,11)
    # 8..11: beta_cp  = AT_c1*AT_1p
    # 12,13: alpha1 init (s(-lI0), s(lI0))

    # ---------------- split u-halves ----------------
    # U0/U1 arrive from the host with the T axis permuted within each
    # 8-block: storage slot s holds natural step j = bitrev3(s), i.e. slot
    # order j = (0,4,2,6,1,5,3,7).  Slots 0..3 are exactly the even-j
    # "parity-split" order j_even(m) = 4*(m&1)+2*(m>>1) the M planes want,
    # slots 4..7 the odds.  M-plane reads are packed (stride-1 runs of 4)
    # -> DVE 4x, and each row-tile r starts as soon as its U DMAs land.
    Me = keep.tile([128, 4 * H], FP16, tag="Me")
    Mo = keep.tile([128, 4 * H], FP16, tag="Mo")
    ME = [Me[:, i * H:(i + 1) * H] for i in range(4)]
    MO = [Mo[:, i * H:(i + 1) * H] for i in range(4)]

    def m_plane(dst_i, usrc, kcol, joff):
        # dst pos = r*256 + b*4 + m  <-  src pos = r*512 + b*8 + 4*joff + m
        for r in range(RT):
            v.tensor_scalar_mul(
                _ap(dst_i, r * 256, [[4, 64], [1, 4]]),
                _ap(usrc[:], r * T + 4 * joff, [[8, 64], [1, 4]]),
                Kt[:, r * 16 + kcol:r * 16 + kcol + 1])
    for i, (us, kc_) in enumerate(((u0, 0), (u1, 1), (u0, 2), (u1, 3))):
        m_plane(ME[i], us, kc_, 0)
        m_plane(MO[i], us, kc_, 1)

    # ---------------- tree level 1: U2 = Modd @ Meven ----------------
    # U2_cp[B',m] = Mo_c0*Me_0p + Mo_c1*Me_1p, elementwise over (B', m);
    # planes are contiguous so everything is packed (2x fp16).
    es_tree = ExitStack()
    tr = es_tree.enter_context(tc.tile_pool(name="tr", bufs=1))
    U2 = tr.tile([128, 4 * H], FP16, tag="U2")
    g1 = tr.tile([128, 4 * H], FP16, tag="g1")
    g2 = tr.tile([128, 4 * H], FP16, tag="g2")
    for c in range(2):
        # dims (p, B'm): B-side Mo_c0 bcast over p; A-side Me_0p planes
        v.tensor_tensor(_ap(g1[:], 2 * c * H, [[H, 2], [1, H]]),
                        _ap(Mo[:], 2 * c * H, [[0, 2], [1, H]]),
                        _ap(Me[:], 0, [[H, 2], [1, H]]), op=Alu.mult)
        v.tensor_tensor(_ap(g2[:], 2 * c * H, [[H, 2], [1, H]]),
                        _ap(Mo[:], (2 * c + 1) * H, [[0, 2], [1, H]]),
                        _ap(Me[:], 2 * H, [[H, 2], [1, H]]), op=Alu.mult)
    v.tensor_tensor(U2[:], g1[:], g2[:], op=Alu.add)

    # prediction-side mask, chunked so it fills Pool gaps greedily
    Ym = keep.tile([128, W], mybir.dt.uint32, tag="Ym")
    for ch in range(6):
        gp.tensor_scalar(_ap(Ym[:], ch * (W // 6), [[1, W // 6]]),
                         _ap(Yt[:], ch * (W // 6), [[1, W // 6]]),
                         0.5, None, Alu.is_ge)

    # ---------------- tree level 2: U4 ----------------
    # U2 pair-evens at slots {0,1} (contig), odds at {2,3}.
    # U4_cp[B', n] = U2o_c0[B',n]*U2e_0p[B',n] + U2o_c1[B',n]*U2e_1p[B',n]
    # U2 planes: pos(i, B', m) = i*H + B'*4 + m ; even-read: m in {0,1}:
    # [[4,NBT],[1,2]]; odd-read: +2.
    U4 = tr.tile([128, 4 * 2 * NBT], FP16, tag="U4")   # planes cp x (B',n)
    t1 = tr.tile([128, 4 * 2 * NBT], FP16, tag="t1")
    t2 = tr.tile([128, 4 * 2 * NBT], FP16, tag="t2")
    # per c (ISA max 3 free dims), iterate (p, B', n):
    # B-side: U2odd_c{k} at plane (2c+k), slots {2,3}: pos = (2c+k)*H+B'*4+2+n
    # A-side: U2even_{k}p at plane (2k+p), slots {0,1}
    # out t: pos = (2c+p)*2*NBT + B'*2 + n
    for c in range(2):
        dims_out = [[2 * NBT, 2], [2, NBT], [1, 2]]
        v.tensor_tensor(
            _ap(t1[:], c * 2 * 2 * NBT, dims_out),
            _ap(U2[:], 2 * c * H + 2, [[0, 2], [4, NBT], [1, 2]]),
            _ap(U2[:], 0, [[H, 2], [4, NBT], [1, 2]]),
            op=Alu.mult)
        v.tensor_tensor(
            _ap(t2[:], c * 2 * 2 * NBT, dims_out),
            _ap(U2[:], (2 * c + 1) * H + 2, [[0, 2], [4, NBT], [1, 2]]),
            _ap(U2[:], 2 * H, [[H, 2], [4, NBT], [1, 2]]),
            op=Alu.mult)
    v.tensor_tensor(U4[:], t1[:], t2[:], op=Alu.add)

    # ---------------- tree level 3: U8 ----------------
    # U4 planes (B', n) interleaved; strided n-reads (1x), packed add.
    U8 = tr.tile([128, 4 * NBT], FP16, tag="U8")       # planes cp x B'
    t3 = tr.tile([128, 4 * NBT], FP16, tag="t3")
    t4 = tr.tile([128, 4 * NBT], FP16, tag="t4")
    od = [[2 * NBT, 2], [NBT, 2], [1, NBT]]
    v.tensor_tensor(_ap(t3[:], 0, od),
                    _ap(U4[:], 1, [[2 * 2 * NBT, 2], [0, 2], [2, NBT]]),
                    _ap(U4[:], 0, [[0, 2], [2 * NBT, 2], [2, NBT]]),
                    op=Alu.mult)
    v.tensor_tensor(_ap(t4[:], 0, od),
                    _ap(U4[:], 2 * NBT + 1,
                        [[2 * 2 * NBT, 2], [0, 2], [2, NBT]]),
                    _ap(U4[:], 4 * NBT, [[0, 2], [2 * NBT, 2], [2, NBT]]),
                    op=Alu.mult)
    v.tensor_tensor(U8[:], t3[:], t4[:], op=Alu.add)

    # normalize U8 (sum of 4 entries -> 1) to keep radix-8 chains in range
    zn = tr.tile([128, NBT], FP16, tag="zn")
    rz = tr.tile([128, NBT], FP16, tag="rz")
    zn2 = tr.tile([128, NBT], FP16, tag="zn2")
    v.tensor_tensor(_ap(zn[:], 0, [[1, NBT]]),
                    _ap(U8[:], 0, [[1, NBT]]),
                    _ap(U8[:], NBT, [[1, NBT]]), op=Alu.add)
    v.tensor_tensor(_ap(zn2[:], 0, [[1, NBT]]),
                    _ap(U8[:], 2 * NBT, [[1, NBT]]),
                    _ap(U8[:], 3 * NBT, [[1, NBT]]), op=Alu.add)
    v.tensor_tensor(_ap(zn[:], 0, [[1, NBT]]),
                    _ap(zn[:], 0, [[1, NBT]]),
                    _ap(zn2[:], 0, [[1, NBT]]), op=Alu.add)
    v.reciprocal(rz[:], zn[:])
    v.tensor_tensor(_ap(U8[:], 0, [[NBT, 4], [1, NBT]]),
                    _ap(U8[:], 0, [[NBT, 4], [1, NBT]]),
                    _ap(rz[:], 0, [[0, 4], [1, NBT]]), op=Alu.mult)

    # ---------------- radix-8 block scan ----------------
    # blocks b in [0,64) per tile; groups g of 8 blocks (8 groups/tile).
    # Step A: in-group inclusive matrix prefixes P[g, j] (HS shifts 1,2,4).
    # P stored planar like U8: planes cp x (B' = tile*64 + 8g + j).
    es_blk = ExitStack()
    bs = es_blk.enter_context(tc.tile_pool(name="bs", bufs=1))
    P = U8
    for h in (1, 2, 4):
        Pn = bs.tile([128, 4 * NBT], FP16, tag=f"P{h}")
        s1 = bs.tile([128, 4 * NBT], FP16, tag=f"s1_{h}")
        s2 = bs.tile([128, 4 * NBT], FP16, tag=f"s2_{h}")
        n = 8 - h
        # C[i] = P[i] * P[i-h] for i in [h,8) within each group
        # per c: dims (p, g, i); B-side P_c{k}[i] at plane (2c+k)
        go = [[NBT, 2], [8, NBT // 8], [1, n]]
        for c in range(2):
            v.tensor_tensor(
                _ap(s1[:], c * 2 * NBT + h, go),
                _ap(P[:], 2 * c * NBT + h, [[0, 2], [8, NBT // 8], [1, n]]),
                _ap(P[:], 0, [[NBT, 2], [8, NBT // 8], [1, n]]),
                op=Alu.mult)
            v.tensor_tensor(
                _ap(s2[:], c * 2 * NBT + h, go),
                _ap(P[:], (2 * c + 1) * NBT + h,
                    [[0, 2], [8, NBT // 8], [1, n]]),
                _ap(P[:], 2 * NBT, [[NBT, 2], [8, NBT // 8], [1, n]]),
                op=Alu.mult)
        v.tensor_tensor(_ap(Pn[:], h, [[NBT, 4], [8, NBT // 8], [1, n]]),
                        _ap(s1[:], h, [[NBT, 4], [8, NBT // 8], [1, n]]),
                        _ap(s2[:], h, [[NBT, 4], [8, NBT // 8], [1, n]]),
                        op=Alu.add)
        # heads [0,h) copy through (DVE: keeps the chain on one queue --
        # an Act round-trip here costs ~2 sem hops + 185ns SBUF latency)
        v.tensor_copy(out=_ap(Pn[:], 0, [[NBT, 4], [8, NBT // 8], [1, h]]),
                      in_=_ap(P[:], 0, [[NBT, 4], [8, NBT // 8], [1, h]]))
        P = Pn

    # Step B: group totals Tg = P[g,7]; normalize; tiny inclusive scan
    # over the 8 groups per tile (HS 1,2,4); then vg = Escan[g-1] @ alpha1.
    # Tg planar: planes cp x (tile r, g): width 4 * 24.
    NG = RT * 8
    Tg = bs.tile([128, 4 * NG], FP16, tag="Tg")
    v.tensor_copy(out=_ap(Tg[:], 0, [[NG, 4], [1, NG]]),
                  in_=_ap(P[:], 7, [[NBT, 4], [8, NG]]))
    # normalize Tg
    zg = bs.tile([128, NG], FP16, tag="zg")
    rg = bs.tile([128, NG], FP16, tag="rg")
    zg2 = bs.tile([128, NG], FP16, tag="zg2")
    v.tensor_tensor(zg[:], _ap(Tg[:], 0, [[1, NG]]),
                    _ap(Tg[:], NG, [[1, NG]]), op=Alu.add)
    v.tensor_tensor(zg2[:], _ap(Tg[:], 2 * NG, [[1, NG]]),
                    _ap(Tg[:], 3 * NG, [[1, NG]]), op=Alu.add)
    v.tensor_tensor(zg[:], zg[:], zg2[:], op=Alu.add)
    v.reciprocal(rg[:], zg[:])
    v.tensor_tensor(_ap(Tg[:], 0, [[NG, 4], [1, NG]]),
                    _ap(Tg[:], 0, [[NG, 4], [1, NG]]),
                    _ap(rg[:], 0, [[0, 4], [1, NG]]), op=Alu.mult)
    E = Tg
    for h in (1, 2, 4):
        En = bs.tile([128, 4 * NG], FP16, tag=f"E{h}")
        e1 = bs.tile([128, 4 * NG], FP16, tag=f"e1_{h}")
        e2 = bs.tile([128, 4 * NG], FP16, tag=f"e2_{h}")
        n = 8 - h
        go = [[NG, 2], [8, RT], [1, n]]
        for c in range(2):
            v.tensor_tensor(
                _ap(e1[:], c * 2 * NG + h, go),
                _ap(E[:], 2 * c * NG + h, [[0, 2], [8, RT], [1, n]]),
                _ap(E[:], 0, [[NG, 2], [8, RT], [1, n]]),
                op=Alu.mult)
            v.tensor_tensor(
                _ap(e2[:], c * 2 * NG + h, go),
                _ap(E[:], (2 * c + 1) * NG + h, [[0, 2], [8, RT], [1, n]]),
                _ap(E[:], 2 * NG, [[NG, 2], [8, RT], [1, n]]),
                op=Alu.mult)
        v.tensor_tensor(_ap(En[:], h, [[NG, 4], [8, RT], [1, n]]),
                        _ap(e1[:], h, [[NG, 4], [8, RT], [1, n]]),
                        _ap(e2[:], h, [[NG, 4], [8, RT], [1, n]]),
                        op=Alu.add)
        v.tensor_copy(out=_ap(En[:], 0, [[NG, 4], [8, RT], [1, h]]),
                      in_=_ap(E[:], 0, [[NG, 4], [8, RT], [1, h]]))
        E = En

    # vg[g] = E[g-1] @ alpha1 for g>=1 ; vg[0] = alpha1.  alpha1 per-tile
    # scalars K cols 12,13.  v-planes: vg0/vg1 width NG.
    vg = bs.tile([128, 2 * NG], FP16, tag="vg")
    vt = bs.tile([128, 2 * NG], FP16, tag="vt")
    for r in range(RT):
        a0 = Kt[:, r * 16 + 12:r * 16 + 13]
        a1 = Kt[:, r * 16 + 13:r * 16 + 14]
        for comp in range(2):
            # vg[comp][r, g] = E_{comp,0}[g-1]*a0 + E_{comp,1}[g-1]*a1
            seg7 = [[1, 7]]
            v.tensor_scalar_mul(
                _ap(vt[:], comp * NG + r * 8 + 1, seg7),
                _ap(E[:], (2 * comp + 1) * NG + r * 8, seg7), a1)
            v.scalar_tensor_tensor(
                _ap(vg[:], comp * NG + r * 8 + 1, seg7),
                _ap(E[:], (2 * comp) * NG + r * 8, seg7), a0,
                _ap(vt[:], comp * NG + r * 8 + 1, seg7), Alu.mult, Alu.add)
        v.tensor_copy(out=_ap(vg[:], r * 8, [[NG, 2], [1, 1]]),
                      in_=_ap(Kt[:], r * 16 + 12, [[1, 2], [0, 1]]))
    # normalize vg
    zv = bs.tile([128, NG], FP16, tag="zv")
    rv = bs.tile([128, NG], FP16, tag="rv")
    v.tensor_tensor(zv[:], _ap(vg[:], 0, [[1, NG]]),
                    _ap(vg[:], NG, [[1, NG]]), op=Alu.add)
    v.reciprocal(rv[:], zv[:])
    v.tensor_tensor(_ap(vg[:], 0, [[NG, 2], [1, NG]]),
                    _ap(vg[:], 0, [[NG, 2], [1, NG]]),
                    _ap(rv[:], 0, [[0, 2], [1, NG]]), op=Alu.mult)

    # Step C: w_b for all blocks.  w[8g] = vg[g]; w[8g+j] = P[g,j-1] @ vg[g].
    # w planes: w0/w1 width NBT (B'-indexed).
    wb = tr.tile([128, 2 * NBT], FP16, tag="wb")
    wt1 = bs.tile([128, 2 * NBT], FP16, tag="wt1")
    wt2 = bs.tile([128, 2 * NBT], FP16, tag="wt2")
    # dims (comp, g, j in 1..7): w_c = P_c0[g,j-1]*vg_0[g] + P_c1[g,j-1]*vg_1[g]
    wo = [[NBT, 2], [8, NBT // 8], [1, 7]]
    v.tensor_tensor(
        _ap(wt1[:], 1, wo),
        _ap(P[:], 0, [[2 * NBT, 2], [8, NBT // 8], [1, 7]]),
        _ap(vg[:], 0, [[0, 2], [1, NBT // 8], [0, 7]]),
        op=Alu.mult)
    v.tensor_tensor(
        _ap(wt2[:], 1, wo),
        _ap(P[:], NBT, [[2 * NBT, 2], [8, NBT // 8], [1, 7]]),
        _ap(vg[:], NG, [[0, 2], [1, NBT // 8], [0, 7]]),
        op=Alu.mult)
    v.tensor_tensor(_ap(wb[:], 1, wo), _ap(wt1[:], 1, wo),
                    _ap(wt2[:], 1, wo), op=Alu.add)
    v.tensor_copy(out=_ap(wb[:], 0, [[NBT, 2], [8, NBT // 8], [1, 1]]),
                  in_=_ap(vg[:], 0, [[NG, 2], [1, NBT // 8], [0, 1]]))
    # normalize w
    zw = bs.tile([128, NBT], FP16, tag="zw")
    rw = bs.tile([128, NBT], FP16, tag="rw")
    v.tensor_tensor(zw[:], _ap(wb[:], 0, [[1, NBT]]),
                    _ap(wb[:], NBT, [[1, NBT]]), op=Alu.add)
    v.reciprocal(rw[:], zw[:])
    v.tensor_tensor(_ap(wb[:], 0, [[NBT, 2], [1, NBT]]),
                    _ap(wb[:], 0, [[NBT, 2], [1, NBT]]),
                    _ap(rw[:], 0, [[0, 2], [1, NBT]]), op=Alu.mult)
    es_blk.close()

    # ---------------- within-block down-sweep ----------------
    # Alpha planes AL0/AL1, width W, slot layout (B', s: 8),
    # s = bitrev3(j): even slots 0..3 hold j = 0,4,2,6; odd 4..7: 1,5,3,7.
    es_al = ExitStack()
    alp = es_al.enter_context(tc.tile_pool(name="alp", bufs=1))
    AL = keep.tile([128, 2 * W], FP16, tag="AL")  # AL0 | AL1
    a1t = alp.tile([128, 2 * NBT], FP16, tag="a1t")
    a2t = alp.tile([128, 2 * NBT], FP16, tag="a2t")
    # slot 0 (j=0) = w
    sc.copy(_ap(AL[:], 0, [[W, 2], [8, NBT], [1, 1]]),
            _ap(wb[:], 0, [[NBT, 2], [1, NBT], [0, 1]]))
    # stage 1: slot 1 (j=4) = U4[node0] @ w ; U4 node0 = strided n=0 reads
    v.tensor_tensor(
        _ap(a1t[:], 0, [[NBT, 2], [1, NBT]]),
        _ap(U4[:], 0, [[2 * 2 * NBT, 2], [2, NBT]]),
        _ap(wb[:], 0, [[0, 2], [1, NBT]]), op=Alu.mult)
    v.tensor_tensor(
        _ap(a2t[:], 0, [[NBT, 2], [1, NBT]]),
        _ap(U4[:], 2 * NBT, [[2 * 2 * NBT, 2], [2, NBT]]),
        _ap(wb[:], NBT, [[0, 2], [1, NBT]]), op=Alu.mult)
    v.tensor_tensor(_ap(AL[:], 1, [[W, 2], [8, NBT]]),
                    _ap(a1t[:], 0, [[NBT, 2], [1, NBT]]),
                    _ap(a2t[:], 0, [[NBT, 2], [1, NBT]]), op=Alu.add)
    # stage 2: slots 2,3 (j=2,6) = U2[pair-even p1] @ AL[slots 0,1]
    # U2 even-pair slots {0,1}: pos = i*H + B'*4 + m, m in {0,1}
    b1 = alp.tile([128, 2 * 2 * NBT], FP16, tag="b1")
    b2 = alp.tile([128, 2 * 2 * NBT], FP16, tag="b2")
    s2o = [[2 * NBT, 2], [2, NBT], [1, 2]]
    v.tensor_tensor(
        _ap(b1[:], 0, s2o),
        _ap(U2[:], 0, [[2 * H, 2], [4, NBT], [1, 2]]),
        _ap(AL[:], 0, [[0, 2], [8, NBT], [1, 2]]), op=Alu.mult)
    v.tensor_tensor(
        _ap(b2[:], 0, s2o),
        _ap(U2[:], H, [[2 * H, 2], [4, NBT], [1, 2]]),
        _ap(AL[:], W, [[0, 2], [8, NBT], [1, 2]]), op=Alu.mult)
    v.tensor_tensor(_ap(AL[:], 2, [[W, 2], [8, NBT], [1, 2]]),
                    _ap(b1[:], 0, s2o), _ap(b2[:], 0, s2o), op=Alu.add)
    # stage 3: odd slots 4..7 (j=1,5,3,7) = M_even @ AL[even slots]
    c1 = alp.tile([128, 2 * W // 2], FP16, tag="c1")
    c2 = alp.tile([128, 2 * W // 2], FP16, tag="c2")
    # per row-tile so tile-0 predictions can start before tiles 1-2 finish
    for r in range(RT):
        ob4 = r * 256
        oa = r * T
        s3o = [[H, 2], [4, 64], [1, 4]]
        v.tensor_tensor(
            _ap(c1[:], ob4, s3o),
            _ap(Me[:], ob4, [[2 * H, 2], [4, 64], [1, 4]]),
            _ap(AL[:], oa, [[0, 2], [8, 64], [1, 4]]), op=Alu.mult)
        v.tensor_tensor(
            _ap(c2[:], ob4, s3o),
            _ap(Me[:], H + ob4, [[2 * H, 2], [4, 64], [1, 4]]),
            _ap(AL[:], W + oa, [[0, 2], [8, 64], [1, 4]]), op=Alu.mult)
        v.tensor_tensor(_ap(AL[:], 4 + oa, [[W, 2], [8, 64], [1, 4]]),
                        _ap(c1[:], ob4, s3o), _ap(c2[:], ob4, s3o),
                        op=Alu.add)

    # -------- predictions + lp + cumsum + q, pipelined per row-tile --------
    # per tile r: DVE (Za, Ra, rr, q1) -> Act (q0, ln-unpermute) -> DVE
    # (mask, copy-pred, scan, q-adds) -> relayout DMAs; tiles overlap engines.
    es_pr = ExitStack()
    pr = es_pr.enter_context(tc.tile_pool(name="pr", bufs=1))
    Za = pr.tile([128, W], FP16, tag="Za")
    Ra = pr.tile([128, W], FP16, tag="Ra")
    rr = pr.tile([128, W], FP16, tag="rr")
    q1 = pr.tile([128, W], FP16, tag="q1")
    p1 = keep.tile([128, W], F32, tag="p1")
    p0 = keep.tile([128, W], F32, tag="p0")
    lp = keep.tile([128, W], F32, tag="lp")
    # apc2 has one zero column before each tile's T cumsum columns so the
    # q = p + apc[t-1] add runs full-T with no single-element edge copies.
    apc2 = keep.tile([128, W + RT], F32, tag="apc2")
    q1c = keep.tile([128, W], F32, tag="q1c")
    q0c = keep.tile([128, W], F32, tag="q0c")
    # ability planes split in two tiles so the a0-3 max tree isn't blocked
    # on ability 4's late relayout; partitions [0:64) hold k=1, [64:128) k=0
    # (k=1 lower so tile2's identity move is the later-computed q1).
    QA03 = keep.tile([128, 4 * T], F32, tag="QA03")
    QA4 = keep.tile([128, T], F32, tag="QA4")
    psq = ctx.enter_context(tc.tile_pool(name="psq", bufs=1, space="PSUM"))
    gp.memset(_ap(apc2[:], 0, [[T + 1, RT]]), 0.0)
    slp = [[1, T]]

    def pred_front(r):
        o = r * T
        v.tensor_tensor(_ap(Za[:], o, slp), _ap(AL[:], o, slp),
                        _ap(AL[:], W + o, slp), op=Alu.add)
        v.reciprocal(_ap(Ra[:], o, slp), _ap(Za[:], o, slp))
        v.tensor_tensor(_ap(rr[:], o, slp), _ap(AL[:], W + o, slp),
                        _ap(Ra[:], o, slp), op=Alu.mult)
        v.tensor_tensor(_ap(q1[:], o, slp), _ap(rr[:], o, slp),
                        _ap(dm[:], o, slp), op=Alu.mult)
        v.tensor_tensor(_ap(q1[:], o, slp), _ap(q1[:], o, slp),
                        _ap(pgs[:], o, slp), op=Alu.add)
        # ln with unpermute slot->natural (split by j0); p0 = ln(1 - q1)
        # fuses the complement into the activation's scale/bias
        for j0 in range(2):
            sc.activation(
                _ap(p1[:], o + j0, [[8, 64], [2, 2], [4, 2]]),
                _ap(q1[:], o + 4 * j0, [[8, 64], [2, 2], [1, 2]]),
                Act.Ln)
            sc.activation(
                _ap(p0[:], o + j0, [[8, 64], [2, 2], [4, 2]]),
                _ap(q1[:], o + 4 * j0, [[8, 64], [2, 2], [1, 2]]),
                Act.Ln, bias=1.0, scale=-1.0)

    def pred_scan(r):
        o = r * T
        sc.copy(_ap(lp[:], o, slp), _ap(p0[:], o, slp))
        v.copy_predicated(_ap(lp[:], o, slp), _ap(Ym[:], o, slp),
                          _ap(p1[:], o, slp))
        v.tensor_tensor_scan(_ap(apc2[:], r * (T + 1) + 1, slp),
                             _ap(lp[:], o, slp),
                             _ap(lp[:], o, slp),
                             0.0, Alu.add, Alu.bypass)

    def _qa_dst(k, a):
        base, off = (QA4, 0) if a == 4 else (QA03, a * T)
        return _ap_p(base[:], 64 * (1 - k), 64, off, [[1, T]])

    def pred_q(r):
        # q_k = p_k + apc[t-1] (k=0 on DVE, k=1 on Pool for tiles 0/1),
        # then relayout into QA: moves with matching partition ranges
        # (half == 1-k) are on-chip copies (deferred so they don't block
        # the next tile's critical ops); cross moves are HWDGE DMAs --
        # keeping them off Pool's SWDGE avoids descriptor-gen queueing
        # behind Pool's q1 adds.
        o = r * T
        na = 2 if r < 2 else 1   # tile 2 holds only ability 4 (rows 0-63)
        for k, qsrc, psrc in ((0, q0c, p0), (1, q1c, p1)):
            qeng = gp if k == 1 else v
            qeng.tensor_tensor(_ap(qsrc[:], o, slp),
                               _ap(psrc[:], o, slp),
                               _ap(apc2[:], r * (T + 1), slp), op=Alu.add)
            for half in range(na):
                if half == 1 - k:
                    continue
                sy.dma_start(_qa_dst(k, 2 * r + half),
                             _ap_p(qsrc[:], 64 * half, 64, o, [[1, T]]))

    for r in range(2):
        pred_front(r)
        pred_scan(r)
        pred_q(r)
    pred_front(2)
    pred_scan(2)
    # k=0 identity copies for tiles 0/1, emitted here so they sit in the
    # Act queue ahead of tile 2's q consumers but after its Ln/lp ops
    for r in range(2):
        sc.copy(_qa_dst(0, 2 * r + 1),
                _ap_p(q0c[:], 64, 64, r * T, [[1, T]]))
    pred_q(2)
    # deferred identity copies: k=1 planes for tiles 0/1 on Pool (after
    # both q1 adds), and tile 2's late a4 k=1 move on Act
    for r in range(2):
        gp.tensor_copy(out=_qa_dst(1, 2 * r),
                       in_=_ap_p(q1c[:], 0, 64, r * T, [[1, T]]))
    sc.copy(_qa_dst(1, 4), _ap_p(q1c[:], 0, 64, 2 * T, [[1, T]]))
    es_pr.close()
    es_al.close()
    es_tree.close()
    es_in.close()

    # ---------------- collapse over abilities ----------------
    col2 = ctx.enter_context(tc.tile_pool(name="col2", bufs=1))

    MX = col2.tile([128, T], F32, tag="MX")
    DF = col2.tile([128, A_LEV * T], FP16, tag="DF")
    EX = col2.tile([128, A_LEV * T], FP16, tag="EX")
    SM = col2.tile([128, T], F32, tag="SM")
    un = col2.tile([128, T], F32, tag="un")
    mt = col2.tile([128, 2 * T], F32, tag="mt")
    mth = col2.tile([128, 2 * T], FP16, tag="mth")
    psp = ctx.enter_context(tc.tile_pool(name="psp", bufs=1, space="PSUM"))
    un1s0 = psp.tile([64, T // 4], F32, tag="un1s0")
    un1s1 = psp.tile([64, T // 4], F32, tag="un1s1")
    un1s2 = psp.tile([64, T // 4], F32, tag="un1s2")
    un1s3 = psp.tile([64, T // 4], F32, tag="un1s3")
    un1s = [un1s0, un1s1, un1s2, un1s3]
    dl = col2.tile([64, T], F32, tag="dl")
    ed = col2.tile([64, T], F32, tag="ed")
    sp = col2.tile([64, T], F32, tag="sp")
    OI = col2.tile([64, 2 * T], F32, tag="OI")
    # t-chunked 3-engine pipeline over the collapse.  The a0-3 max tree
    # reads only QA03, so it runs while ability 4's relayout is in flight;
    # only MX/DF wait for QA4.  Partitions [0:64) hold k=1, [64:128) k=0,
    # so dl = un1 - un0 and out0 = -softplus(dl), out1 = dl - softplus(dl).
    NCH = 4
    HT = T // NCH

    def cmaxA(t0):
        # max(a0, a1): depends only on tile 0's relayout -- runs in the
        # DVE gap while tile 1's cross DMA is still in flight
        hl = [[1, HT]]
        v.tensor_tensor(_ap(mt[:], t0, hl),
                        _ap(QA03[:], t0, hl),
                        _ap(QA03[:], T + t0, hl), op=Alu.max)

    def cmax(t0):
        hl = [[1, HT]]
        v.tensor_tensor(_ap(mt[:], T + t0, hl),
                        _ap(QA03[:], 2 * T + t0, hl),
                        _ap(QA03[:], 3 * T + t0, hl), op=Alu.max)
        v.tensor_tensor(_ap(mt[:], t0, hl), _ap(mt[:], t0, hl),
                        _ap(mt[:], T + t0, hl), op=Alu.max)

    def cdf(t0):
        hl = [[1, HT]]
        v.tensor_tensor(_ap(MX[:], t0, hl), _ap(mt[:], t0, hl),
                        _ap(QA4[:], t0, hl), op=Alu.max)
        v.tensor_tensor(_ap(DF[:], t0, [[T, 4], [1, HT]]),
                        _ap(QA03[:], t0, [[T, 4], [1, HT]]),
                        _ap(MX[:], t0, [[0, 4], [1, HT]]),
                        op=Alu.subtract)
        v.tensor_tensor(_ap(DF[:], 4 * T + t0, hl),
                        _ap(QA4[:], t0, hl),
                        _ap(MX[:], t0, hl), op=Alu.subtract)
        sc.activation(_ap(EX[:], t0, [[T, A_LEV], [1, HT]]),
                      _ap(DF[:], t0, [[T, A_LEV], [1, HT]]), Act.Exp)

    def csum(t0):
        hl = [[1, HT]]
        v.tensor_tensor(_ap(mth[:], t0, [[T, 2], [1, HT]]),
                        _ap(EX[:], t0, [[T, 2], [1, HT]]),
                        _ap(EX[:], 2 * T + t0, [[T, 2], [1, HT]]),
                        op=Alu.add)
        v.tensor_tensor(_ap(mth[:], t0, hl), _ap(mth[:], t0, hl),
                        _ap(mth[:], T + t0, hl), op=Alu.add)
        v.tensor_tensor(_ap(SM[:], t0, hl), _ap(mth[:], t0, hl),
                        _ap(EX[:], 4 * T + t0, hl), op=Alu.add)
        sc.activation(_ap(SM[:], t0, hl), _ap(SM[:], t0, hl), Act.Ln)
        v.tensor_tensor(_ap(un[:], t0, hl), _ap(MX[:], t0, hl),
                        _ap(SM[:], t0, hl), op=Alu.add)
        # partition shift via idle PE: un1s[j] = un[64+j] (k=0 half down);
        # one PSUM tile per chunk so consumers don't serialize on tile deps
        nc.tensor.matmul(_ap_p(un1s[t0 // HT][:], 0, 64, 0, hl),
                         SHt[:, 0:64], _ap_p(un[:], 0, 128, t0, hl),
                         start=True, stop=True)

    def ctail(t0, HL, oq):
        hl = [[1, HL]]
        v.tensor_tensor(_ap_p(dl[:], 0, 64, t0, hl),
                        _ap_p(un[:], 0, 64, t0, hl),
                        _ap_p(un1s[t0 // (T // 4)][:], 0, 64, 0, hl),
                        op=Alu.subtract)
        sc.activation(_ap_p(ed[:], 0, 64, t0, hl),
                      _ap_p(dl[:], 0, 64, t0, hl), Act.Exp)
        sc.activation(_ap_p(sp[:], 0, 64, t0, hl),
                      _ap_p(ed[:], 0, 64, t0, hl), Act.Ln, bias=1.0)
        gp.tensor_scalar(_ap_p(OI[:], 0, 64, 2 * t0, [[2, HL]]),
                         _ap_p(sp[:], 0, 64, t0, hl),
                         -1.0, None, Alu.mult)
        v.tensor_tensor(_ap_p(OI[:], 0, 64, 2 * t0 + 1, [[2, HL]]),
                        _ap_p(dl[:], 0, 64, t0, hl),
                        _ap_p(sp[:], 0, 64, t0, hl), op=Alu.subtract)
        if oq is not None:          # one output DMA per chunk pair
            ot = t0 + HL - 2 * HT
            oq.dma_start(
                bass.AP(O[:].tensor, 2 * ot, [[2 * T, 64], [1, 4 * HT]]),
                _ap_p(OI[:], 0, 64, 2 * ot, [[1, 4 * HT]]))

    for c in range(NCH):
        cmaxA(c * HT)
    for c in range(NCH):
        cmax(c * HT)
    for c in range(NCH):
        cdf(c * HT)
    for c in range(NCH):
        csum(c * HT)
    for c in range(NCH):
        ctail(c * HT, HT, sy if c % 2 == 1 else None)


def _steer_act_tables(arch):
    """Keep Exp/Ln claimed by one table set (see kernel v1)."""
    from concourse import hw_specs
    tabs = hw_specs.get_activation_tables(arch)
    for name, funcs in tabs.items():
        if name == "natural_log_exp_and_others":
            continue
        funcs.discard(Act.Exp)
        funcs.discard(Act.Ln)


def _build_program():
    nc = bacc.Bacc()
    _steer_act_tables(nc.m.arch)
    U0 = nc.declare_dram_parameter("U0", [RT * 128, T], FP16, isOutput=False)
    U1 = nc.declare_dram_parameter("U1", [RT * 128, T], FP16, isOutput=False)
    PG = nc.declare_dram_parameter("PG", [RT * 128, T], FP16, isOutput=False)
    DM = nc.declare_dram_parameter("DM", [RT * 128, T], FP16, isOutput=False)
    Y = nc.declare_dram_parameter("Y", [RT * 128, T], FP16, isOutput=False)
    K = nc.declare_dram_parameter("K", [RT * 128, 16], F32, isOutput=False)
    SH = nc.declare_dram_parameter("SH", [128, 128], F32, isOutput=False)
    O = nc.declare_dram_parameter("O", [BL, T, 2], F32, isOutput=True)
    with ExitStack() as ctx:
        tc = ctx.enter_context(tile.TileContext(nc))
        with nc.allow_low_precision(reason="fp16 HMM chain; validated vs gate"):
            _emit(ctx, tc, nc, U0, U1, PG, DM, Y, K, SH, O)
    if not nc.is_finalized():
        nc.finalize()
    return nc


def _pad_rows(x, dtype=np.float32, fill=0.0):
    out = np.full((RT * 128, x.shape[1]), fill, dtype=dtype)
    out[:ROWS] = x
    return out


def kernel(corr, ytrue, problem, kc, dyn_emb, obs_logits_problem,
           obs_logits_kc, ability_levels, traj, trans_ind, pred_ind):
    global _last_results, _cached_nc
    import ml_dtypes
    fp16 = np.float16

    corr = np.asarray(corr, dtype=np.float32)
    ytrue = np.asarray(ytrue, dtype=np.float32)
    problem = np.asarray(problem)
    kc = np.asarray(kc)
    dyn_emb = np.asarray(dyn_emb, dtype=np.float32)
    obs_logits_problem = np.asarray(obs_logits_problem, dtype=np.float32)
    obs_logits_kc = np.asarray(obs_logits_kc, dtype=np.float32)
    ability = np.asarray(ability_levels, dtype=np.float32)

    obs_core = obs_logits_problem[problem] + obs_logits_kc[kc][:, None, :]
    dyn = dyn_emb[kc]
    sig = lambda x: 1.0 / (1.0 + np.exp(-x.astype(np.float64)))
    lL, lF, lI0 = dyn[:, 0], dyn[:, 1], dyn[:, 2]
    AT00, AT01 = sig(-lL), sig(lF)
    AT10, AT11 = sig(lL), sig(-lF)
    al = [AT00, AT01, AT10, AT11]
    alpha = [al[2 * (i // 2)] * al[i % 2] for i in range(4)]
    # alpha_cp = AT_c0*AT_0p: (c,p): c0 entry = AT[c][0] = al[2c], AT[0][p]=al[p]
    alpha = [al[2 * (i // 2)] * al[i % 2] for i in range(4)]
    beta = [al[2 * (i // 2) + 1] * al[2 + i % 2] for i in range(4)]
    Kfull = np.stack(al + alpha + beta +
                     [sig(-lI0), sig(lI0), np.zeros_like(lL),
                      np.zeros_like(lL)], axis=1).astype(np.float32)  # (B,16)

    # permute the T axis within each 8-block so storage slot s holds
    # natural step j = bitrev3(s); Y stays natural (cumsum order).
    perm = (np.arange(T) & ~7) + np.tile(
        np.array([0, 4, 2, 6, 1, 5, 3, 7]), T // 8)

    in_maps = []
    for c in range(N_CORES):
        sl = slice(c * BL, (c + 1) * BL)
        g = obs_core[sl, :, 0][None, :, perm] + ability[:, None, None]
        s = obs_core[sl, :, 1][None, :, perm] - ability[:, None, None]
        ct = np.broadcast_to(corr[sl][:, perm][None], (A_LEV, BL, T))
        yt = np.broadcast_to(ytrue[sl][None], (A_LEV, BL, T))
        # observation likelihood diagonals, normalized to sum 1 per step
        c2 = 2.0 * ct - 1.0
        u0r = sig(c2 * g)
        u1r = sig(-c2 * s)
        zu = u0r + u1r
        u0n = (u0r / zu).astype(np.float32)
        pg = sig(g)
        dmv = (sig(-s) - pg).astype(np.float32)
        kt = np.broadcast_to(Kfull[sl][None], (A_LEV, BL, 16))
        kpad = _pad_rows(kt.reshape(ROWS, 16), np.float32)
        kpad[ROWS:] = 0.5            # benign transition probs on padded rows
        shm = np.zeros((128, 128), np.float32)
        shm[np.arange(64) + 64, np.arange(64)] = 1.0
        shm[np.arange(64), np.arange(64) + 64] = 1.0
        # pad rows get benign 0.5 probabilities so no inf/nan ever forms
        # there (the PE half-swap matmuls contract over all partitions and
        # 0 * nan would poison valid lanes)
        in_maps.append({
            "U0": _pad_rows(u0n.reshape(ROWS, T), fp16, 0.5),
            "U1": _pad_rows((1.0 - u0n).reshape(ROWS, T), fp16, 0.5),
            "PG": _pad_rows(pg.reshape(ROWS, T).astype(np.float32), fp16, 0.5),
            "DM": _pad_rows(dmv.reshape(ROWS, T), fp16),
            "Y": _pad_rows(yt.reshape(ROWS, T), fp16),
            "K": kpad,
            "SH": shm,
        })

    if _cached_nc is None:
        _cached_nc = _build_program()

    res = run_bass_kernel_spmd(
        _cached_nc, in_maps, list(range(N_CORES)),
        trace=bool(os.environ.get("BASS_TRACE")),
    )
    _last_results = res
    out = np.concatenate([res.results[i]["O"] for i in range(N_CORES)], axis=0)
    return out.astype(np.float32)



# revision 85
# speedup vs baseline: 1.0547x; 1.0037x over previous
"""BKT model kernel v2 for Trainium2 (8 NeuronCores, Bass/Tile).

Exact 2-state HMM reformulation of the reference's 2^n-trajectory fastBKT
(see kernel v1 docstring).  v2 restructures for the DVE cost model:

- fp16 for the whole matrix chain (obs probs, level matrices, tree products,
  alphas, predictions).  The chain is contractive and sum-normalized, so
  fp16's 2^-11 rounding keeps the final error ~2e-3 << the 2e-2 gate;
  subnormal flushes only hit entries whose contribution is negligible.
- planar 2x2-entry planes (one buffer region per matrix entry) so
  tensor_tensor ops read/write packed last dims -> DVE 2x mode; per-partition
  transition constants ride tensor_scalar (2x/4x) and Act-engine scale APs.
- within-block (8-step) products use the A^T = gamma*I + 1 v^T structure at
  level 1, a "parity-split" pair layout for levels 2-3, and a 3-stage vector
  down-sweep for the per-step alphas.
- the 64-block scan is radix-8: in-group Hillis-Steele matrix prefixes,
  a tiny 8-group matrix scan, then one batched mat-vec to get per-block
  start alphas.
- Act engine absorbs sigmoids/copies/lns (including the bit-reversal
  unpermute via 4-free-dim APs); Pool absorbs reductions off the DVE path.
- log-predictions, cumsum and the ability-collapse stay f32.

Sharding: data-parallel over students (B=512 -> 64 per core); 5 ability
levels x 64 students = 320 rows padded to 3 x 128-partition tiles.
"""

import os
import numpy as np
from contextlib import ExitStack

import concourse.bass as bass
import concourse.bacc as bacc
import concourse.mybir as mybir
from concourse import tile
from concourse.bass_utils import run_bass_kernel_spmd

F32 = mybir.dt.float32
FP16 = mybir.dt.float16
Alu = mybir.AluOpType
Act = mybir.ActivationFunctionType
AX = mybir.AxisListType

N_CORES = 8
B_FULL = 512
T = 512
A_LEV = 5
BL = B_FULL // N_CORES          # students per core = 64
ROWS = A_LEV * BL               # valid rows per core = 320
RT = 3                          # row tiles of 128 (384 rows incl. pad)
NBT = RT * 64                   # blocks spanning tiles = 192
W = RT * T                      # full-plane free width = 1536
H = W // 2                      # half width = 768
ABILITY = np.array([-2.0, -1.0, 0.0, 1.0, 2.0], dtype=np.float32)

_last_results = None
_cached_nc = None


def _ap(base, off, dims):
    """Custom AP on the same tensor as `base`, keeping its partition dim."""
    return bass.AP(base.tensor, base.offset + off, [list(base.ap[0])] + dims)


def _ap_p(base, poff, pcount, off, dims):
    p = list(base.ap[0])
    pstride = p[0]
    return bass.AP(
        base.tensor, base.offset + poff * pstride + off, [[pstride, pcount]] + dims
    )


def _emit(ctx, tc, nc, U01, PD, Y, K, SH, O):
    v = nc.vector
    sc = nc.scalar
    gp = nc.gpsimd
    sy = nc.sync

    keep = ctx.enter_context(tc.tile_pool(name="keep", bufs=1))

    # ---------------- input DMAs ----------------
    # U0/U1 are the normalized per-step observation likelihood diagonals
    # (host-side sigmoids, slot-ordered); PG = P(y=1|unlearned) and
    # DM = P(y=1|learned) - PG feed the predictions.  K first (tiny, the
    # M planes need its scalars), then U0/U1 per row-tile on the HWDGE
    # queue; PG/DM/Y trail on Pool's SWDGE (needed only by the preds).
    es_in = ExitStack()
    io = es_in.enter_context(tc.tile_pool(name="io", bufs=1))
    u01 = io.tile([128, 2 * W], FP16, tag="U01")
    pd = keep.tile([128, 2 * W], FP16, tag="PD")
    Yt = keep.tile([128, W], FP16, tag="Y")
    Kt = keep.tile([128, RT * 16], F32, tag="K")
    gp.dma_start(_ap(Kt[:], 0, [[16, RT], [1, 16]]),
                 bass.AP(K[:].tensor, 0, [[16, 128], [128 * 16, RT], [1, 16]]))
    # U0|U1 packed per row on the host: one DMA per row-tile fills both
    # halves of u01 (cols [0,W) = u0, [W,2W) = u1); same for PG|DM.
    for r in range(RT):
        sy.dma_start(_ap(u01[:], r * T, [[W, 2], [1, T]]),
                     bass.AP(U01[:].tensor, r * 128 * 2 * T,
                             [[2 * T, 128], [T, 2], [1, T]]))
    for r in range(RT):
        gp.dma_start(_ap(pd[:], r * T, [[W, 2], [1, T]]),
                     bass.AP(PD[:].tensor, r * 128 * 2 * T,
                             [[2 * T, 128], [T, 2], [1, T]]))
    gp.dma_start(_ap(Yt[:], 0, [[T, RT], [1, T]]),
                 bass.AP(Y[:].tensor, 0, [[T, 128], [128 * T, RT], [1, T]]))
    # partition-half swap matrix SW[i, j] = 1 iff |i-j| == 64: PE matmuls
    # with it (or its left half) replace SBUF->SBUF partition-shift DMAs
    SHt = keep.tile([128, 128], F32, tag="SH")
    sy.dma_start(SHt[:], bass.AP(SH[:].tensor, 0, [[128, 128], [1, 128]]))

    def KC(col):
        """Per-partition scalar AP for K column `col` of row-tile r -- but all
        tiles share the op; K scalars differ per tile, so ops over multi-tile
        widths must pass per-tile slices.  Helper returns slice for tile r."""
        return Kt[:, col:col + 1]

    # K layout (16 cols per tile r at r*16):
    # 0..3 : A^T entries AT00, AT01, AT10, AT11
    # 4..7 : alpha_cp = AT_c0*AT_0p   (order 00,01,10,11)
    # 8..11: beta_cp  = AT_c1*AT_1p
    # 12,13: alpha1 init (s(-lI0), s(lI0))

    # ---------------- split u-halves ----------------
    # U0/U1 arrive from the host with the T axis permuted within each
    # 8-block: storage slot s holds natural step j = bitrev3(s), i.e. slot
    # order j = (0,4,2,6,1,5,3,7).  Slots 0..3 are exactly the even-j
    # "parity-split" order j_even(m) = 4*(m&1)+2*(m>>1) the M planes want,
    # slots 4..7 the odds.  M-plane reads are packed (stride-1 runs of 4)
    # -> DVE 4x, and each row-tile r starts as soon as its U DMAs land.
    Me = keep.tile([128, 4 * H], FP16, tag="Me")
    Mo = keep.tile([128, 4 * H], FP16, tag="Mo")
    ME = [Me[:, i * H:(i + 1) * H] for i in range(4)]
    MO = [Mo[:, i * H:(i + 1) * H] for i in range(4)]

    def m_plane(dst_i, uoff, kcol, joff):
        # dst pos = r*256 + b*4 + m  <-  src pos = r*512 + b*8 + 4*joff + m
        for r in range(RT):
            v.tensor_scalar_mul(
                _ap(dst_i, r * 256, [[4, 64], [1, 4]]),
                _ap(u01[:], uoff + r * T + 4 * joff, [[8, 64], [1, 4]]),
                Kt[:, r * 16 + kcol:r * 16 + kcol + 1])
    for i, (uo, kc_) in enumerate(((0, 0), (W, 1), (0, 2), (W, 3))):
        m_plane(ME[i], uo, kc_, 0)
        m_plane(MO[i], uo, kc_, 1)

    # ---------------- tree level 1: U2 = Modd @ Meven ----------------
    # U2_cp[B',m] = Mo_c0*Me_0p + Mo_c1*Me_1p, elementwise over (B', m);
    # planes are contiguous so everything is packed (2x fp16).
    es_tree = ExitStack()
    tr = es_tree.enter_context(tc.tile_pool(name="tr", bufs=1))
    U2 = tr.tile([128, 4 * H], FP16, tag="U2")
    g1 = tr.tile([128, 4 * H], FP16, tag="g1")
    g2 = tr.tile([128, 4 * H], FP16, tag="g2")
    for c in range(2):
        # dims (p, B'm): B-side Mo_c0 bcast over p; A-side Me_0p planes
        v.tensor_tensor(_ap(g1[:], 2 * c * H, [[H, 2], [1, H]]),
                        _ap(Mo[:], 2 * c * H, [[0, 2], [1, H]]),
                        _ap(Me[:], 0, [[H, 2], [1, H]]), op=Alu.mult)
        v.tensor_tensor(_ap(g2[:], 2 * c * H, [[H, 2], [1, H]]),
                        _ap(Mo[:], (2 * c + 1) * H, [[0, 2], [1, H]]),
                        _ap(Me[:], 2 * H, [[H, 2], [1, H]]), op=Alu.mult)
    v.tensor_tensor(U2[:], g1[:], g2[:], op=Alu.add)

    # prediction-side mask, chunked so it fills Pool gaps greedily
    Ym = keep.tile([128, W], mybir.dt.uint32, tag="Ym")
    for ch in range(6):
        gp.tensor_scalar(_ap(Ym[:], ch * (W // 6), [[1, W // 6]]),
                         _ap(Yt[:], ch * (W // 6), [[1, W // 6]]),
                         0.5, None, Alu.is_ge)

    # ---------------- tree level 2: U4 ----------------
    # U2 pair-evens at slots {0,1} (contig), odds at {2,3}.
    # U4_cp[B', n] = U2o_c0[B',n]*U2e_0p[B',n] + U2o_c1[B',n]*U2e_1p[B',n]
    # U2 planes: pos(i, B', m) = i*H + B'*4 + m ; even-read: m in {0,1}:
    # [[4,NBT],[1,2]]; odd-read: +2.
    U4 = tr.tile([128, 4 * 2 * NBT], FP16, tag="U4")   # planes cp x (B',n)
    t1 = tr.tile([128, 4 * 2 * NBT], FP16, tag="t1")
    t2 = tr.tile([128, 4 * 2 * NBT], FP16, tag="t2")
    # per c (ISA max 3 free dims), iterate (p, B', n):
    # B-side: U2odd_c{k} at plane (2c+k), slots {2,3}: pos = (2c+k)*H+B'*4+2+n
    # A-side: U2even_{k}p at plane (2k+p), slots {0,1}
    # out t: pos = (2c+p)*2*NBT + B'*2 + n
    for c in range(2):
        dims_out = [[2 * NBT, 2], [2, NBT], [1, 2]]
        v.tensor_tensor(
            _ap(t1[:], c * 2 * 2 * NBT, dims_out),
            _ap(U2[:], 2 * c * H + 2, [[0, 2], [4, NBT], [1, 2]]),
            _ap(U2[:], 0, [[H, 2], [4, NBT], [1, 2]]),
            op=Alu.mult)
        v.tensor_tensor(
            _ap(t2[:], c * 2 * 2 * NBT, dims_out),
            _ap(U2[:], (2 * c + 1) * H + 2, [[0, 2], [4, NBT], [1, 2]]),
            _ap(U2[:], 2 * H, [[H, 2], [4, NBT], [1, 2]]),
            op=Alu.mult)
    v.tensor_tensor(U4[:], t1[:], t2[:], op=Alu.add)

    # ---------------- tree level 3: U8 ----------------
    # U4 planes (B', n) interleaved; strided n-reads (1x), packed add.
    U8 = tr.tile([128, 4 * NBT], FP16, tag="U8")       # planes cp x B'
    t3 = tr.tile([128, 4 * NBT], FP16, tag="t3")
    t4 = tr.tile([128, 4 * NBT], FP16, tag="t4")
    od = [[2 * NBT, 2], [NBT, 2], [1, NBT]]
    v.tensor_tensor(_ap(t3[:], 0, od),
                    _ap(U4[:], 1, [[2 * 2 * NBT, 2], [0, 2], [2, NBT]]),
                    _ap(U4[:], 0, [[0, 2], [2 * NBT, 2], [2, NBT]]),
                    op=Alu.mult)
    v.tensor_tensor(_ap(t4[:], 0, od),
                    _ap(U4[:], 2 * NBT + 1,
                        [[2 * 2 * NBT, 2], [0, 2], [2, NBT]]),
                    _ap(U4[:], 4 * NBT, [[0, 2], [2 * NBT, 2], [2, NBT]]),
                    op=Alu.mult)
    v.tensor_tensor(U8[:], t3[:], t4[:], op=Alu.add)

    # normalize U8 (sum of 4 entries -> 1) to keep radix-8 chains in range
    zn = tr.tile([128, NBT], FP16, tag="zn")
    rz = tr.tile([128, NBT], FP16, tag="rz")
    zn2 = tr.tile([128, NBT], FP16, tag="zn2")
    v.tensor_tensor(_ap(zn[:], 0, [[1, NBT]]),
                    _ap(U8[:], 0, [[1, NBT]]),
                    _ap(U8[:], NBT, [[1, NBT]]), op=Alu.add)
    v.tensor_tensor(_ap(zn2[:], 0, [[1, NBT]]),
                    _ap(U8[:], 2 * NBT, [[1, NBT]]),
                    _ap(U8[:], 3 * NBT, [[1, NBT]]), op=Alu.add)
    v.tensor_tensor(_ap(zn[:], 0, [[1, NBT]]),
                    _ap(zn[:], 0, [[1, NBT]]),
                    _ap(zn2[:], 0, [[1, NBT]]), op=Alu.add)
    v.reciprocal(rz[:], zn[:])
    v.tensor_tensor(_ap(U8[:], 0, [[NBT, 4], [1, NBT]]),
                    _ap(U8[:], 0, [[NBT, 4], [1, NBT]]),
                    _ap(rz[:], 0, [[0, 4], [1, NBT]]), op=Alu.mult)

    # ---------------- radix-8 block scan ----------------
    # blocks b in [0,64) per tile; groups g of 8 blocks (8 groups/tile).
    # Step A: in-group inclusive matrix prefixes P[g, j] (HS shifts 1,2,4).
    # P stored planar like U8: planes cp x (B' = tile*64 + 8g + j).
    es_blk = ExitStack()
    bs = es_blk.enter_context(tc.tile_pool(name="bs", bufs=1))
    P = U8
    for h in (1, 2, 4):
        Pn = bs.tile([128, 4 * NBT], FP16, tag=f"P{h}")
        s1 = bs.tile([128, 4 * NBT], FP16, tag=f"s1_{h}")
        s2 = bs.tile([128, 4 * NBT], FP16, tag=f"s2_{h}")
        n = 8 - h
        # C[i] = P[i] * P[i-h] for i in [h,8) within each group
        # per c: dims (p, g, i); B-side P_c{k}[i] at plane (2c+k)
        go = [[NBT, 2], [8, NBT // 8], [1, n]]
        for c in range(2):
            v.tensor_tensor(
                _ap(s1[:], c * 2 * NBT + h, go),
                _ap(P[:], 2 * c * NBT + h, [[0, 2], [8, NBT // 8], [1, n]]),
                _ap(P[:], 0, [[NBT, 2], [8, NBT // 8], [1, n]]),
                op=Alu.mult)
            v.tensor_tensor(
                _ap(s2[:], c * 2 * NBT + h, go),
                _ap(P[:], (2 * c + 1) * NBT + h,
                    [[0, 2], [8, NBT // 8], [1, n]]),
                _ap(P[:], 2 * NBT, [[NBT, 2], [8, NBT // 8], [1, n]]),
                op=Alu.mult)
        v.tensor_tensor(_ap(Pn[:], h, [[NBT, 4], [8, NBT // 8], [1, n]]),
                        _ap(s1[:], h, [[NBT, 4], [8, NBT // 8], [1, n]]),
                        _ap(s2[:], h, [[NBT, 4], [8, NBT // 8], [1, n]]),
                        op=Alu.add)
        # heads [0,h) copy through (DVE: keeps the chain on one queue --
        # an Act round-trip here costs ~2 sem hops + 185ns SBUF latency)
        v.tensor_copy(out=_ap(Pn[:], 0, [[NBT, 4], [8, NBT // 8], [1, h]]),
                      in_=_ap(P[:], 0, [[NBT, 4], [8, NBT // 8], [1, h]]))
        P = Pn

    # Step B: group totals Tg = P[g,7]; normalize; tiny inclusive scan
    # over the 8 groups per tile (HS 1,2,4); then vg = Escan[g-1] @ alpha1.
    # Tg planar: planes cp x (tile r, g): width 4 * 24.
    NG = RT * 8
    Tg = bs.tile([128, 4 * NG], FP16, tag="Tg")
    v.tensor_copy(out=_ap(Tg[:], 0, [[NG, 4], [1, NG]]),
                  in_=_ap(P[:], 7, [[NBT, 4], [8, NG]]))
    # normalize Tg
    zg = bs.tile([128, NG], FP16, tag="zg")
    rg = bs.tile([128, NG], FP16, tag="rg")
    zg2 = bs.tile([128, NG], FP16, tag="zg2")
    v.tensor_tensor(zg[:], _ap(Tg[:], 0, [[1, NG]]),
                    _ap(Tg[:], NG, [[1, NG]]), op=Alu.add)
    v.tensor_tensor(zg2[:], _ap(Tg[:], 2 * NG, [[1, NG]]),
                    _ap(Tg[:], 3 * NG, [[1, NG]]), op=Alu.add)
    v.tensor_tensor(zg[:], zg[:], zg2[:], op=Alu.add)
    v.reciprocal(rg[:], zg[:])
    v.tensor_tensor(_ap(Tg[:], 0, [[NG, 4], [1, NG]]),
                    _ap(Tg[:], 0, [[NG, 4], [1, NG]]),
                    _ap(rg[:], 0, [[0, 4], [1, NG]]), op=Alu.mult)
    E = Tg
    for h in (1, 2, 4):
        En = bs.tile([128, 4 * NG], FP16, tag=f"E{h}")
        e1 = bs.tile([128, 4 * NG], FP16, tag=f"e1_{h}")
        e2 = bs.tile([128, 4 * NG], FP16, tag=f"e2_{h}")
        n = 8 - h
        go = [[NG, 2], [8, RT], [1, n]]
        for c in range(2):
            v.tensor_tensor(
                _ap(e1[:], c * 2 * NG + h, go),
                _ap(E[:], 2 * c * NG + h, [[0, 2], [8, RT], [1, n]]),
                _ap(E[:], 0, [[NG, 2], [8, RT], [1, n]]),
                op=Alu.mult)
            v.tensor_tensor(
                _ap(e2[:], c * 2 * NG + h, go),
                _ap(E[:], (2 * c + 1) * NG + h, [[0, 2], [8, RT], [1, n]]),
                _ap(E[:], 2 * NG, [[NG, 2], [8, RT], [1, n]]),
                op=Alu.mult)
        v.tensor_tensor(_ap(En[:], h, [[NG, 4], [8, RT], [1, n]]),
                        _ap(e1[:], h, [[NG, 4], [8, RT], [1, n]]),
                        _ap(e2[:], h, [[NG, 4], [8, RT], [1, n]]),
                        op=Alu.add)
        v.tensor_copy(out=_ap(En[:], 0, [[NG, 4], [8, RT], [1, h]]),
                      in_=_ap(E[:], 0, [[NG, 4], [8, RT], [1, h]]))
        E = En

    # vg[g] = E[g-1] @ alpha1 for g>=1 ; vg[0] = alpha1.  alpha1 per-tile
    # scalars K cols 12,13.  v-planes: vg0/vg1 width NG.
    vg = bs.tile([128, 2 * NG], FP16, tag="vg")
    vt = bs.tile([128, 2 * NG], FP16, tag="vt")
    for r in range(RT):
        a0 = Kt[:, r * 16 + 12:r * 16 + 13]
        a1 = Kt[:, r * 16 + 13:r * 16 + 14]
        for comp in range(2):
            # vg[comp][r, g] = E_{comp,0}[g-1]*a0 + E_{comp,1}[g-1]*a1
            seg7 = [[1, 7]]
            v.tensor_scalar_mul(
                _ap(vt[:], comp * NG + r * 8 + 1, seg7),
                _ap(E[:], (2 * comp + 1) * NG + r * 8, seg7), a1)
            v.scalar_tensor_tensor(
                _ap(vg[:], comp * NG + r * 8 + 1, seg7),
                _ap(E[:], (2 * comp) * NG + r * 8, seg7), a0,
                _ap(vt[:], comp * NG + r * 8 + 1, seg7), Alu.mult, Alu.add)
        v.tensor_copy(out=_ap(vg[:], r * 8, [[NG, 2], [1, 1]]),
                      in_=_ap(Kt[:], r * 16 + 12, [[1, 2], [0, 1]]))
    # normalize vg
    zv = bs.tile([128, NG], FP16, tag="zv")
    rv = bs.tile([128, NG], FP16, tag="rv")
    v.tensor_tensor(zv[:], _ap(vg[:], 0, [[1, NG]]),
                    _ap(vg[:], NG, [[1, NG]]), op=Alu.add)
    v.reciprocal(rv[:], zv[:])
    v.tensor_tensor(_ap(vg[:], 0, [[NG, 2], [1, NG]]),
                    _ap(vg[:], 0, [[NG, 2], [1, NG]]),
                    _ap(rv[:], 0, [[0, 2], [1, NG]]), op=Alu.mult)

    # Step C: w_b for all blocks.  w[8g] = vg[g]; w[8g+j] = P[g,j-1] @ vg[g].
    # w planes: w0/w1 width NBT (B'-indexed).
    wb = tr.tile([128, 2 * NBT], FP16, tag="wb")
    wt1 = bs.tile([128, 2 * NBT], FP16, tag="wt1")
    wt2 = bs.tile([128, 2 * NBT], FP16, tag="wt2")
    # dims (comp, g, j in 1..7): w_c = P_c0[g,j-1]*vg_0[g] + P_c1[g,j-1]*vg_1[g]
    wo = [[NBT, 2], [8, NBT // 8], [1, 7]]
    v.tensor_tensor(
        _ap(wt1[:], 1, wo),
        _ap(P[:], 0, [[2 * NBT, 2], [8, NBT // 8], [1, 7]]),
        _ap(vg[:], 0, [[0, 2], [1, NBT // 8], [0, 7]]),
        op=Alu.mult)
    v.tensor_tensor(
        _ap(wt2[:], 1, wo),
        _ap(P[:], NBT, [[2 * NBT, 2], [8, NBT // 8], [1, 7]]),
        _ap(vg[:], NG, [[0, 2], [1, NBT // 8], [0, 7]]),
        op=Alu.mult)
    v.tensor_tensor(_ap(wb[:], 1, wo), _ap(wt1[:], 1, wo),
                    _ap(wt2[:], 1, wo), op=Alu.add)
    v.tensor_copy(out=_ap(wb[:], 0, [[NBT, 2], [8, NBT // 8], [1, 1]]),
                  in_=_ap(vg[:], 0, [[NG, 2], [1, NBT // 8], [0, 1]]))
    # normalize w
    zw = bs.tile([128, NBT], FP16, tag="zw")
    rw = bs.tile([128, NBT], FP16, tag="rw")
    v.tensor_tensor(zw[:], _ap(wb[:], 0, [[1, NBT]]),
                    _ap(wb[:], NBT, [[1, NBT]]), op=Alu.add)
    v.reciprocal(rw[:], zw[:])
    v.tensor_tensor(_ap(wb[:], 0, [[NBT, 2], [1, NBT]]),
                    _ap(wb[:], 0, [[NBT, 2], [1, NBT]]),
                    _ap(rw[:], 0, [[0, 2], [1, NBT]]), op=Alu.mult)
    es_blk.close()

    # ---------------- within-block down-sweep ----------------
    # Alpha planes AL0/AL1, width W, slot layout (B', s: 8),
    # s = bitrev3(j): even slots 0..3 hold j = 0,4,2,6; odd 4..7: 1,5,3,7.
    es_al = ExitStack()
    alp = es_al.enter_context(tc.tile_pool(name="alp", bufs=1))
    AL = keep.tile([128, 2 * W], FP16, tag="AL")  # AL0 | AL1
    a1t = alp.tile([128, 2 * NBT], FP16, tag="a1t")
    a2t = alp.tile([128, 2 * NBT], FP16, tag="a2t")
    # slot 0 (j=0) = w
    sc.copy(_ap(AL[:], 0, [[W, 2], [8, NBT], [1, 1]]),
            _ap(wb[:], 0, [[NBT, 2], [1, NBT], [0, 1]]))
    # stage 1: slot 1 (j=4) = U4[node0] @ w ; U4 node0 = strided n=0 reads
    v.tensor_tensor(
        _ap(a1t[:], 0, [[NBT, 2], [1, NBT]]),
        _ap(U4[:], 0, [[2 * 2 * NBT, 2], [2, NBT]]),
        _ap(wb[:], 0, [[0, 2], [1, NBT]]), op=Alu.mult)
    v.tensor_tensor(
        _ap(a2t[:], 0, [[NBT, 2], [1, NBT]]),
        _ap(U4[:], 2 * NBT, [[2 * 2 * NBT, 2], [2, NBT]]),
        _ap(wb[:], NBT, [[0, 2], [1, NBT]]), op=Alu.mult)
    v.tensor_tensor(_ap(AL[:], 1, [[W, 2], [8, NBT]]),
                    _ap(a1t[:], 0, [[NBT, 2], [1, NBT]]),
                    _ap(a2t[:], 0, [[NBT, 2], [1, NBT]]), op=Alu.add)
    # stage 2: slots 2,3 (j=2,6) = U2[pair-even p1] @ AL[slots 0,1]
    # U2 even-pair slots {0,1}: pos = i*H + B'*4 + m, m in {0,1}
    b1 = alp.tile([128, 2 * 2 * NBT], FP16, tag="b1")
    b2 = alp.tile([128, 2 * 2 * NBT], FP16, tag="b2")
    s2o = [[2 * NBT, 2], [2, NBT], [1, 2]]
    v.tensor_tensor(
        _ap(b1[:], 0, s2o),
        _ap(U2[:], 0, [[2 * H, 2], [4, NBT], [1, 2]]),
        _ap(AL[:], 0, [[0, 2], [8, NBT], [1, 2]]), op=Alu.mult)
    v.tensor_tensor(
        _ap(b2[:], 0, s2o),
        _ap(U2[:], H, [[2 * H, 2], [4, NBT], [1, 2]]),
        _ap(AL[:], W, [[0, 2], [8, NBT], [1, 2]]), op=Alu.mult)
    v.tensor_tensor(_ap(AL[:], 2, [[W, 2], [8, NBT], [1, 2]]),
                    _ap(b1[:], 0, s2o), _ap(b2[:], 0, s2o), op=Alu.add)
    # stage 3: odd slots 4..7 (j=1,5,3,7) = M_even @ AL[even slots]
    c1 = alp.tile([128, 2 * W // 2], FP16, tag="c1")
    c2 = alp.tile([128, 2 * W // 2], FP16, tag="c2")
    # per row-tile so tile-0 predictions can start before tiles 1-2 finish
    for r in range(RT):
        ob4 = r * 256
        oa = r * T
        s3o = [[H, 2], [4, 64], [1, 4]]
        v.tensor_tensor(
            _ap(c1[:], ob4, s3o),
            _ap(Me[:], ob4, [[2 * H, 2], [4, 64], [1, 4]]),
            _ap(AL[:], oa, [[0, 2], [8, 64], [1, 4]]), op=Alu.mult)
        v.tensor_tensor(
            _ap(c2[:], ob4, s3o),
            _ap(Me[:], H + ob4, [[2 * H, 2], [4, 64], [1, 4]]),
            _ap(AL[:], W + oa, [[0, 2], [8, 64], [1, 4]]), op=Alu.mult)
        v.tensor_tensor(_ap(AL[:], 4 + oa, [[W, 2], [8, 64], [1, 4]]),
                        _ap(c1[:], ob4, s3o), _ap(c2[:], ob4, s3o),
                        op=Alu.add)

    # -------- predictions + lp + cumsum + q, pipelined per row-tile --------
    # per tile r: DVE (Za, Ra, rr, q1) -> Act (q0, ln-unpermute) -> DVE
    # (mask, copy-pred, scan, q-adds) -> relayout DMAs; tiles overlap engines.
    es_pr = ExitStack()
    pr = es_pr.enter_context(tc.tile_pool(name="pr", bufs=1))
    Za = pr.tile([128, W], FP16, tag="Za")
    Ra = pr.tile([128, W], FP16, tag="Ra")
    rr = pr.tile([128, W], FP16, tag="rr")
    q1 = pr.tile([128, W], FP16, tag="q1")
    p1 = keep.tile([128, W], F32, tag="p1")
    p0 = keep.tile([128, W], F32, tag="p0")
    lp = keep.tile([128, W], F32, tag="lp")
    # apc2 has one zero column before each tile's T cumsum columns so the
    # q = p + apc[t-1] add runs full-T with no single-element edge copies.
    apc2 = keep.tile([128, W + RT], F32, tag="apc2")
    q1c = keep.tile([128, W], F32, tag="q1c")
    q0c = keep.tile([128, W], F32, tag="q0c")
    # ability planes split in two tiles so the a0-3 max tree isn't blocked
    # on ability 4's late relayout; partitions [0:64) hold k=1, [64:128) k=0
    # (k=1 lower so tile2's identity move is the later-computed q1).
    QA03 = keep.tile([128, 4 * T], F32, tag="QA03")
    QA4 = keep.tile([128, T], F32, tag="QA4")
    psq = ctx.enter_context(tc.tile_pool(name="psq", bufs=1, space="PSUM"))
    gp.memset(_ap(apc2[:], 0, [[T + 1, RT]]), 0.0)
    slp = [[1, T]]

    def pred_front(r):
        o = r * T
        v.tensor_tensor(_ap(Za[:], o, slp), _ap(AL[:], o, slp),
                        _ap(AL[:], W + o, slp), op=Alu.add)
        v.reciprocal(_ap(Ra[:], o, slp), _ap(Za[:], o, slp))
        v.tensor_tensor(_ap(rr[:], o, slp), _ap(AL[:], W + o, slp),
                        _ap(Ra[:], o, slp), op=Alu.mult)
        v.tensor_tensor(_ap(q1[:], o, slp), _ap(rr[:], o, slp),
                        _ap(pd[:], W + o, slp), op=Alu.mult)
        v.tensor_tensor(_ap(q1[:], o, slp), _ap(q1[:], o, slp),
                        _ap(pd[:], o, slp), op=Alu.add)
        # ln with unpermute slot->natural (split by j0); p0 = ln(1 - q1)
        # fuses the complement into the activation's scale/bias
        for j0 in range(2):
            sc.activation(
                _ap(p1[:], o + j0, [[8, 64], [2, 2], [4, 2]]),
                _ap(q1[:], o + 4 * j0, [[8, 64], [2, 2], [1, 2]]),
                Act.Ln)
            sc.activation(
                _ap(p0[:], o + j0, [[8, 64], [2, 2], [4, 2]]),
                _ap(q1[:], o + 4 * j0, [[8, 64], [2, 2], [1, 2]]),
                Act.Ln, bias=1.0, scale=-1.0)

    def pred_scan(r):
        o = r * T
        sc.copy(_ap(lp[:], o, slp), _ap(p0[:], o, slp))
        v.copy_predicated(_ap(lp[:], o, slp), _ap(Ym[:], o, slp),
                          _ap(p1[:], o, slp))
        v.tensor_tensor_scan(_ap(apc2[:], r * (T + 1) + 1, slp),
                             _ap(lp[:], o, slp),
                             _ap(lp[:], o, slp),
                             0.0, Alu.add, Alu.bypass)

    def _qa_dst(k, a):
        base, off = (QA4, 0) if a == 4 else (QA03, a * T)
        return _ap_p(base[:], 64 * (1 - k), 64, off, [[1, T]])

    def pred_q(r):
        # q_k = p_k + apc[t-1] (k=0 on DVE, k=1 on Pool for tiles 0/1),
        # then relayout into QA: moves with matching partition ranges
        # (half == 1-k) are on-chip copies (deferred so they don't block
        # the next tile's critical ops); cross moves are HWDGE DMAs --
        # keeping them off Pool's SWDGE avoids descriptor-gen queueing
        # behind Pool's q1 adds.
        o = r * T
        na = 2 if r < 2 else 1   # tile 2 holds only ability 4 (rows 0-63)
        for k, qsrc, psrc in ((0, q0c, p0), (1, q1c, p1)):
            qeng = gp if k == 1 else v
            qeng.tensor_tensor(_ap(qsrc[:], o, slp),
                               _ap(psrc[:], o, slp),
                               _ap(apc2[:], r * (T + 1), slp), op=Alu.add)
            for half in range(na):
                if half == 1 - k:
                    continue
                sy.dma_start(_qa_dst(k, 2 * r + half),
                             _ap_p(qsrc[:], 64 * half, 64, o, [[1, T]]))

    for r in range(2):
        pred_front(r)
        pred_scan(r)
        pred_q(r)
    pred_front(2)
    pred_scan(2)
    # k=0 identity copies for tiles 0/1, emitted here so they sit in the
    # Act queue ahead of tile 2's q consumers but after its Ln/lp ops
    for r in range(2):
        sc.copy(_qa_dst(0, 2 * r + 1),
                _ap_p(q0c[:], 64, 64, r * T, [[1, T]]))
    pred_q(2)
    # deferred identity copies: k=1 planes for tiles 0/1 on Pool (after
    # both q1 adds), and tile 2's late a4 k=1 move on Act
    for r in range(2):
        gp.tensor_copy(out=_qa_dst(1, 2 * r),
                       in_=_ap_p(q1c[:], 0, 64, r * T, [[1, T]]))
    sc.copy(_qa_dst(1, 4), _ap_p(q1c[:], 0, 64, 2 * T, [[1, T]]))
    es_pr.close()
    es_al.close()
    es_tree.close()
    es_in.close()

    # ---------------- collapse over abilities ----------------
    col2 = ctx.enter_context(tc.tile_pool(name="col2", bufs=1))

    MX = col2.tile([128, T], F32, tag="MX")
    DF = col2.tile([128, A_LEV * T], FP16, tag="DF")
    EX = col2.tile([128, A_LEV * T], FP16, tag="EX")
    SM = col2.tile([128, T], F32, tag="SM")
    un = col2.tile([128, T], F32, tag="un")
    mt = col2.tile([128, 2 * T], F32, tag="mt")
    mth = col2.tile([128, 2 * T], FP16, tag="mth")
    psp = ctx.enter_context(tc.tile_pool(name="psp", bufs=1, space="PSUM"))
    un1s0 = psp.tile([64, T // 4], F32, tag="un1s0")
    un1s1 = psp.tile([64, T // 4], F32, tag="un1s1")
    un1s2 = psp.tile([64, T // 4], F32, tag="un1s2")
    un1s3 = psp.tile([64, T // 4], F32, tag="un1s3")
    un1s = [un1s0, un1s1, un1s2, un1s3]
    dl = col2.tile([64, T], F32, tag="dl")
    ed = col2.tile([64, T], F32, tag="ed")
    sp = col2.tile([64, T], F32, tag="sp")
    OI = col2.tile([64, 2 * T], F32, tag="OI")
    # t-chunked 3-engine pipeline over the collapse.  The a0-3 max tree
    # reads only QA03, so it runs while ability 4's relayout is in flight;
    # only MX/DF wait for QA4.  Partitions [0:64) hold k=1, [64:128) k=0,
    # so dl = un1 - un0 and out0 = -softplus(dl), out1 = dl - softplus(dl).
    NCH = 4
    HT = T // NCH

    def cmaxA(t0):
        # max(a0, a1): depends only on tile 0's relayout -- runs in the
        # DVE gap while tile 1's cross DMA is still in flight
        hl = [[1, HT]]
        v.tensor_tensor(_ap(mt[:], t0, hl),
                        _ap(QA03[:], t0, hl),
                        _ap(QA03[:], T + t0, hl), op=Alu.max)

    def cmax(t0):
        hl = [[1, HT]]
        v.tensor_tensor(_ap(mt[:], T + t0, hl),
                        _ap(QA03[:], 2 * T + t0, hl),
                        _ap(QA03[:], 3 * T + t0, hl), op=Alu.max)
        v.tensor_tensor(_ap(mt[:], t0, hl), _ap(mt[:], t0, hl),
                        _ap(mt[:], T + t0, hl), op=Alu.max)

    def cdf(t0):
        hl = [[1, HT]]
        v.tensor_tensor(_ap(MX[:], t0, hl), _ap(mt[:], t0, hl),
                        _ap(QA4[:], t0, hl), op=Alu.max)
        v.tensor_tensor(_ap(DF[:], t0, [[T, 4], [1, HT]]),
                        _ap(QA03[:], t0, [[T, 4], [1, HT]]),
                        _ap(MX[:], t0, [[0, 4], [1, HT]]),
                        op=Alu.subtract)
        v.tensor_tensor(_ap(DF[:], 4 * T + t0, hl),
                        _ap(QA4[:], t0, hl),
                        _ap(MX[:], t0, hl), op=Alu.subtract)
        sc.activation(_ap(EX[:], t0, [[T, A_LEV], [1, HT]]),
                      _ap(DF[:], t0, [[T, A_LEV], [1, HT]]), Act.Exp)

    def csum(t0):
        hl = [[1, HT]]
        v.tensor_tensor(_ap(mth[:], t0, [[T, 2], [1, HT]]),
                        _ap(EX[:], t0, [[T, 2], [1, HT]]),
                        _ap(EX[:], 2 * T + t0, [[T, 2], [1, HT]]),
                        op=Alu.add)
        v.tensor_tensor(_ap(mth[:], t0, hl), _ap(mth[:], t0, hl),
                        _ap(mth[:], T + t0, hl), op=Alu.add)
        v.tensor_tensor(_ap(SM[:], t0, hl), _ap(mth[:], t0, hl),
                        _ap(EX[:], 4 * T + t0, hl), op=Alu.add)
        sc.activation(_ap(SM[:], t0, hl), _ap(SM[:], t0, hl), Act.Ln)
        v.tensor_tensor(_ap(un[:], t0, hl), _ap(MX[:], t0, hl),
                        _ap(SM[:], t0, hl), op=Alu.add)
        # partition shift via idle PE: un1s[j] = un[64+j] (k=0 half down);
        # one PSUM tile per chunk so consumers don't serialize on tile deps
        nc.tensor.matmul(_ap_p(un1s[t0 // HT][:], 0, 64, 0, hl),
                         SHt[:, 0:64], _ap_p(un[:], 0, 128, t0, hl),
                         start=True, stop=True)

    def ctail(t0, HL, oq):
        hl = [[1, HL]]
        v.tensor_tensor(_ap_p(dl[:], 0, 64, t0, hl),
                        _ap_p(un[:], 0, 64, t0, hl),
                        _ap_p(un1s[t0 // (T // 4)][:], 0, 64, 0, hl),
                        op=Alu.subtract)
        sc.activation(_ap_p(ed[:], 0, 64, t0, hl),
                      _ap_p(dl[:], 0, 64, t0, hl), Act.Exp)
        sc.activation(_ap_p(sp[:], 0, 64, t0, hl),
                      _ap_p(ed[:], 0, 64, t0, hl), Act.Ln, bias=1.0)
        gp.tensor_scalar(_ap_p(OI[:], 0, 64, 2 * t0, [[2, HL]]),
                         _ap_p(sp[:], 0, 64, t0, hl),
                         -1.0, None, Alu.mult)
        v.tensor_tensor(_ap_p(OI[:], 0, 64, 2 * t0 + 1, [[2, HL]]),
                        _ap_p(dl[:], 0, 64, t0, hl),
                        _ap_p(sp[:], 0, 64, t0, hl), op=Alu.subtract)
        if oq is not None:          # one output DMA per chunk pair
            ot = t0 + HL - 2 * HT
            oq.dma_start(
                bass.AP(O[:].tensor, 2 * ot, [[2 * T, 64], [1, 4 * HT]]),
                _ap_p(OI[:], 0, 64, 2 * ot, [[1, 4 * HT]]))

    for c in range(NCH):
        cmaxA(c * HT)
    for c in range(NCH):
        cmax(c * HT)
    for c in range(NCH):
        cdf(c * HT)
    for c in range(NCH):
        csum(c * HT)
    for c in range(NCH):
        ctail(c * HT, HT, sy if c % 2 == 1 else None)


def _steer_act_tables(arch):
    """Keep Exp/Ln claimed by one table set (see kernel v1)."""
    from concourse import hw_specs
    tabs = hw_specs.get_activation_tables(arch)
    for name, funcs in tabs.items():
        if name == "natural_log_exp_and_others":
            continue
        funcs.discard(Act.Exp)
        funcs.discard(Act.Ln)


def _build_program():
    nc = bacc.Bacc()
    _steer_act_tables(nc.m.arch)
    U01 = nc.declare_dram_parameter("U01", [RT * 128, 2 * T], FP16,
                                    isOutput=False)
    PD = nc.declare_dram_parameter("PD", [RT * 128, 2 * T], FP16,
                                   isOutput=False)
    Y = nc.declare_dram_parameter("Y", [RT * 128, T], FP16, isOutput=False)
    K = nc.declare_dram_parameter("K", [RT * 128, 16], F32, isOutput=False)
    SH = nc.declare_dram_parameter("SH", [128, 128], F32, isOutput=False)
    O = nc.declare_dram_parameter("O", [BL, T, 2], F32, isOutput=True)
    with ExitStack() as ctx:
        tc = ctx.enter_context(tile.TileContext(nc))
        with nc.allow_low_precision(reason="fp16 HMM chain; validated vs gate"):
            _emit(ctx, tc, nc, U01, PD, Y, K, SH, O)
    if not nc.is_finalized():
        nc.finalize()
    return nc


def _pad_rows(x, dtype=np.float32, fill=0.0):
    out = np.full((RT * 128, x.shape[1]), fill, dtype=dtype)
    out[:ROWS] = x
    return out


def kernel(corr, ytrue, problem, kc, dyn_emb, obs_logits_problem,
           obs_logits_kc, ability_levels, traj, trans_ind, pred_ind):
    global _last_results, _cached_nc
    import ml_dtypes
    fp16 = np.float16

    corr = np.asarray(corr, dtype=np.float32)
    ytrue = np.asarray(ytrue, dtype=np.float32)
    problem = np.asarray(problem)
    kc = np.asarray(kc)
    dyn_emb = np.asarray(dyn_emb, dtype=np.float32)
    obs_logits_problem = np.asarray(obs_logits_problem, dtype=np.float32)
    obs_logits_kc = np.asarray(obs_logits_kc, dtype=np.float32)
    ability = np.asarray(ability_levels, dtype=np.float32)

    obs_core = obs_logits_problem[problem] + obs_logits_kc[kc][:, None, :]
    dyn = dyn_emb[kc]
    sig = lambda x: 1.0 / (1.0 + np.exp(-x.astype(np.float64)))
    lL, lF, lI0 = dyn[:, 0], dyn[:, 1], dyn[:, 2]
    AT00, AT01 = sig(-lL), sig(lF)
    AT10, AT11 = sig(lL), sig(-lF)
    al = [AT00, AT01, AT10, AT11]
    alpha = [al[2 * (i // 2)] * al[i % 2] for i in range(4)]
    # alpha_cp = AT_c0*AT_0p: (c,p): c0 entry = AT[c][0] = al[2c], AT[0][p]=al[p]
    alpha = [al[2 * (i // 2)] * al[i % 2] for i in range(4)]
    beta = [al[2 * (i // 2) + 1] * al[2 + i % 2] for i in range(4)]
    Kfull = np.stack(al + alpha + beta +
                     [sig(-lI0), sig(lI0), np.zeros_like(lL),
                      np.zeros_like(lL)], axis=1).astype(np.float32)  # (B,16)

    # permute the T axis within each 8-block so storage slot s holds
    # natural step j = bitrev3(s); Y stays natural (cumsum order).
    perm = (np.arange(T) & ~7) + np.tile(
        np.array([0, 4, 2, 6, 1, 5, 3, 7]), T // 8)

    in_maps = []
    for c in range(N_CORES):
        sl = slice(c * BL, (c + 1) * BL)
        g = obs_core[sl, :, 0][None, :, perm] + ability[:, None, None]
        s = obs_core[sl, :, 1][None, :, perm] - ability[:, None, None]
        ct = np.broadcast_to(corr[sl][:, perm][None], (A_LEV, BL, T))
        yt = np.broadcast_to(ytrue[sl][None], (A_LEV, BL, T))
        # observation likelihood diagonals, normalized to sum 1 per step
        c2 = 2.0 * ct - 1.0
        u0r = sig(c2 * g)
        u1r = sig(-c2 * s)
        zu = u0r + u1r
        u0n = (u0r / zu).astype(np.float32)
        pg = sig(g)
        dmv = (sig(-s) - pg).astype(np.float32)
        kt = np.broadcast_to(Kfull[sl][None], (A_LEV, BL, 16))
        kpad = _pad_rows(kt.reshape(ROWS, 16), np.float32)
        kpad[ROWS:] = 0.5            # benign transition probs on padded rows
        shm = np.zeros((128, 128), np.float32)
        shm[np.arange(64) + 64, np.arange(64)] = 1.0
        shm[np.arange(64), np.arange(64) + 64] = 1.0
        # pad rows get benign 0.5 probabilities so no inf/nan ever forms
        # there (the PE half-swap matmuls contract over all partitions and
        # 0 * nan would poison valid lanes)
        u01h = np.concatenate([u0n.reshape(ROWS, T),
                               (1.0 - u0n).reshape(ROWS, T)], axis=1)
        pdh = np.concatenate([pg.reshape(ROWS, T).astype(np.float32),
                              dmv.reshape(ROWS, T)], axis=1)
        in_maps.append({
            "U01": _pad_rows(u01h, fp16, 0.5),
            "PD": _pad_rows(pdh, fp16, 0.25),
            "Y": _pad_rows(yt.reshape(ROWS, T), fp16),
            "K": kpad,
            "SH": shm,
        })

    if _cached_nc is None:
        _cached_nc = _build_program()

    res = run_bass_kernel_spmd(
        _cached_nc, in_maps, list(range(N_CORES)),
        trace=bool(os.environ.get("BASS_TRACE")),
    )
    _last_results = res
    out = np.concatenate([res.results[i]["O"] for i in range(N_CORES)], axis=0)
    return out.astype(np.float32)

